# revision 6
# baseline (speedup 1.0000x reference)
"""HGT regressor on 8 Trainium2 NeuronCores (Bass/Tile).

Strategy (graph/data parallel, hint-following):
  - Nodes of each type are partitioned contiguously across the 8 cores
    (a: 12500/core, w: 2500/core, o: 6250/core). Each core owns the edges
    whose *destination* lies in its node shard.
  - Per layer, each core computes K = kqv[:, :128] (raw) and the per-edge-type
    source-side V transform (m_rel folded at source) plus the destination-side
    Q transform (a_rel * p_rel * scale folded into Q) for its own nodes only.
  - The full K / V_et tables are exchanged between layer launches via the host
    (replicated to all cores), i.e. host-mediated all-gather. Q' stays local.
  - Edge phase per core: edges sorted by local destination row, grouped into
    128-node windows; per 128-edge tile: indirect-DMA gathers of K[src],
    V_et[src], Q'_et[dst]; alpha = sum_h(K*Q'); ex = exp(alpha); payload
    [ex*V | ex] is scatter-added into a PSUM window accumulator via a
    one-hot matmul; windows flush densely to a numer/den table in DRAM.
  - Node phase per core: agg = numer/den, gelu, W_o, gated skip, LayerNorm,
    relu, then next-layer projections (or the scalar head in the last layer).
  - Softmax needs no running max: alpha = q'k with these parameter scales is
    O(1); exp cannot overflow, and softmax is shift-invariant anyway.
"""
import os
import sys

sys.path.insert(0, "/opt/trn_rl_repo")

import numpy as np

import concourse.bass as bass
import concourse.mybir as mybir
import concourse.tile as tile
from concourse import bacc

P = 128
H, D, HID = 4, 32, 128
PAY = HID + H  # 132
F32 = mybir.dt.float32
F16 = mybir.dt.float16
BF16 = mybir.dt.bfloat16
I32 = mybir.dt.int32
AF = mybir.ActivationFunctionType
OP = mybir.AluOpType


def _ceil(a, b):
    return (a + b - 1) * b // b if False else -(-a // b) * b


def cdiv(a, b):
    return -(-a // b)


class Cfg:
    """All sizes derived from problem scale; supports mini-scale testing."""

    def __init__(self, NA=100000, NWK=20000, NO=50000, E=150000, C=8):
        self.NA, self.NWK, self.NO, self.E, self.C = NA, NWK, NO, E, C
        assert NA % C == 0 and NWK % C == 0 and NO % C == 0
        self.nac, self.nwc, self.noc = NA // C, NWK // C, NO // C
        self.nap, self.nwp, self.nop = (
            cdiv(self.nac, P) * P,
            cdiv(self.nwc, P) * P,
            cdiv(self.noc, P) * P,
        )
        # local node-row layout (numer/xs/kd rows): [a | w | o], each padded
        self.base_local = (0, self.nap, self.nap + self.nwp)
        self.LOCN = self.nap + self.nwp + self.nop
        self.NWIN = self.LOCN // P
        # per-type tile counts
        self.ntile_a, self.ntile_w, self.ntile_o = (
            self.nap // P,
            self.nwp // P,
            self.nop // P,
        )
        # q' local layout: slots [a-et0, a-et1, w-et2, o-et3]
        self.QB = (0, self.nap, 2 * self.nap, 2 * self.nap + self.nwp)
        self.QTOT = 2 * self.nap + self.nwp + self.nop
        # ve local layout (same bases): slots [a-et2, a-et3, w-et0, o-et1]
        # global kd table layout: [a 0..NA | w | o] + trash
        self.KOFF = (0, NA, NA + NWK)
        self.KD_ROWS = NA + NWK + NO + 1
        # global stacked ve table: [et0 w | et1 o | et2 a | et3 a] + trash
        self.VOFF = (0, NWK, NWK + NO, NWK + NO + NA)
        self.VE_ROWS = NWK + NO + 2 * NA + 1


# edge types: (src_type, dst_type)
ETYPES = ((1, 0), (2, 0), (0, 1), (0, 2))


# ---------------------------------------------------------------------------
# Host-side preprocessing
# ---------------------------------------------------------------------------

def prep_graph(cfg, inputs):
    """Compute per-core edge tile indices. Shared across both layers.

    Returns dict with:
      NT: static tile count (same all cores)
      tws: [NWIN] tiles per window (static across cores)
      tile_idx: [C][NT, P, 4] int32  (kidx, vidx, qidx, dst_local)
    """
    c = cfg
    edges = []  # per et: (src, dst)
    for name_s, name_d in (("src_wa", "dst_wa"), ("src_oa", "dst_oa"),
                           ("src_aw", "dst_aw"), ("src_ao", "dst_ao")):
        edges.append((np.asarray(inputs[name_s]), np.asarray(inputs[name_d])))

    shard_n = (c.nac, c.nwc, c.noc)
    # concat all ets with global indices
    K_TRASH = c.KD_ROWS - 1
    V_TRASH = c.VE_ROWS - 1
    Q_TRASH = c.QTOT  # row appended by host to the q' table

    all_core = []
    koff_by_et = (c.KOFF[1], c.KOFF[2], c.KOFF[0], c.KOFF[0])  # src type offset in kd
    for et, (st, dt) in enumerate(ETYPES):
        src, dst = edges[et]
        kidx = koff_by_et[et] + src
        vidx = c.VOFF[et] + src
        core = dst // shard_n[dt]
        dloc = dst - core * shard_n[dt]  # dst index within its type shard
        # local numer row / q' row
        tb = (c.base_local[0], c.base_local[1], c.base_local[2])[dt]
        row = tb + dloc
        qslot = {0: 0, 1: 1, 2: 2, 3: 3}[et]
        qidx = c.QB[qslot] + dloc
        all_core.append((core, row, kidx, vidx, qidx))

    core_cat = np.concatenate([a[0] for a in all_core])
    row_cat = np.concatenate([a[1] for a in all_core])
    k_cat = np.concatenate([a[2] for a in all_core])
    v_cat = np.concatenate([a[3] for a in all_core])
    q_cat = np.concatenate([a[4] for a in all_core])

    # per-core, per-window edge counts -> static tile structure
    win_cat = row_cat // P
    counts = np.zeros((c.C, c.NWIN), np.int64)
    for cc in range(c.C):
        m = core_cat == cc
        counts[cc] = np.bincount(win_cat[m], minlength=c.NWIN)
    tws = np.maximum(cdiv(counts.max(axis=0), P), 1)  # >=1 tile per window
    NT = int(tws.sum())
    tile_base = np.zeros(c.NWIN, np.int64)
    tile_base[1:] = np.cumsum(tws)[:-1]

    tile_idx = np.zeros((c.C, NT, P, 4), np.int32)
    # fill pads with trash rows -> ex = 0 contributions
    tile_idx[:, :, :, 0] = K_TRASH
    tile_idx[:, :, :, 1] = V_TRASH
    tile_idx[:, :, :, 2] = Q_TRASH
    tile_idx[:, :, :, 3] = 0
    for cc in range(c.C):
        m = core_cat == cc
        rows = row_cat[m]
        order = np.argsort(rows, kind="stable")
        rows = rows[order]
        ks, vs, qs = k_cat[m][order], v_cat[m][order], q_cat[m][order]
        wins = rows // P
        dstl = rows % P
        # position within window
        wstart = np.searchsorted(wins, np.arange(c.NWIN), side="left")
        pos = np.arange(rows.size) - wstart[wins]
        slot_t = pos // P   # tile within window
        slot_p = pos % P    # partition
        gt = tile_base[wins] + slot_t  # global tile id
        tile_idx[cc, gt, slot_p, 0] = ks
        tile_idx[cc, gt, slot_p, 1] = vs
        tile_idx[cc, gt, slot_p, 2] = qs
        tile_idx[cc, gt, slot_p, 3] = dstl
    return {"NT": NT, "tws": tws.astype(np.int64), "tile_idx": tile_idx,
            "tile_base": tile_base}


def blockdiag(M):
    out = np.zeros((HID, HID), np.float32)
    for h in range(H):
        out[h * D:(h + 1) * D, h * D:(h + 1) * D] = M[h]
    return out


def prep_params(cfg, inputs):
    """Fold and lay out all parameters (host, tiny)."""
    scale = np.float32(1.0 / np.sqrt(D))
    a_rel = np.asarray(inputs["a_rel"])
    m_rel = np.asarray(inputs["m_rel"])
    p_rel = np.asarray(inputs["p_rel"])
    prm = {}
    rep = lambda v, w: np.broadcast_to(np.asarray(v, np.float32)[None, :], (P, w)).copy()
    for l in range(2):
        BDaT, BDm = [], []
        for et in range(4):
            a_eff = a_rel[l, et] * (p_rel[l, et] * scale)[:, None, None]
            BDaT.append(blockdiag(a_eff).T.copy())
            BDm.append(blockdiag(m_rel[l, et]))
        prm[f"BDaT{l}"] = np.stack(BDaT)  # [4,128,128]
        prm[f"BDm{l}"] = np.stack(BDm)
        prm[f"Wkqv{l}"] = np.asarray(inputs["W_kqv"])[l]       # [3,128,384]
        prm[f"bkqv{l}"] = np.stack([rep(np.asarray(inputs["b_kqv"])[l, t], 3 * HID) for t in range(3)])
        prm[f"Wo{l}"] = np.asarray(inputs["W_o"])[l]           # [3,128,128]
        g = 1.0 / (1.0 + np.exp(-np.asarray(inputs["skip_p"], np.float64)))  # [2,3]
        prm[f"g{l}"] = g[l].astype(np.float32)
        prm[f"bo{l}"] = np.stack([rep(np.asarray(inputs["b_o"])[l, t] * g[l, t], HID) for t in range(3)])
        prm[f"lng{l}"] = np.stack([rep(np.asarray(inputs["ln_g"])[l, t], HID) for t in range(3)])
        prm[f"lnb{l}"] = np.stack([rep(np.asarray(inputs["ln_b"])[l, t], HID) for t in range(3)])
    # input proj, padded to 128 contraction
    W_in = np.asarray(inputs["W_in"])  # [3,64,128]
    Wp = np.zeros((3, 128, HID), np.float32)
    Wp[:, :64, :] = W_in
    prm["Win"] = Wp
    prm["bin"] = np.stack([rep(np.asarray(inputs["b_in"])[t], HID) for t in range(3)])
    prm["whead"] = np.asarray(inputs["w_head"], np.float32)  # [128,1]
    prm["bh"] = np.full((P, 1), float(np.asarray(inputs["b_head"])[0] + np.asarray(inputs["base"])[0]), np.float32)
    prm["iota"] = np.broadcast_to(np.arange(128, dtype=np.int32)[None, :], (P, 128)).copy()
    prm["ident"] = np.eye(128, dtype=np.float32)
    return prm


# ---------------------------------------------------------------------------
# Builders
# ---------------------------------------------------------------------------

_CONST_N = [0]


def _load_const(nc, cp, ap, shape, dtype=F32):
    _CONST_N[0] += 1
    t = cp.tile(list(shape), dtype, tag=f"cst{_CONST_N[0]}")
    nc.sync.dma_start(t[:], ap)
    return t


def _type_tiles(cfg):
    """Yield (t, i_t, r0) for all node tiles: type, tile-in-type, local row base."""
    out = []
    for t, (ntile, b) in enumerate(
        zip((cfg.ntile_a, cfg.ntile_w, cfg.ntile_o), cfg.base_local)
    ):
        for i in range(ntile):
            out.append((t, i, b + i * P))
    return out


def _kqv_chain(nc, pools, cfg, consts, t, i_t, r0, xs_tile, outs):
    """Emit next-layer projections for one node tile (node-major xs_tile [128,128]).

    Writes kd rows (local), q'_et rows, ve_et rows via DMA to outs dict.
    """
    cp, wp, pp_t, pp_mm = pools["cp"], pools["wp"], pools["pp_t"], pools["pp_mm"]
    kd_o, qp_o, ve_o = outs["kd"], outs["qp"], outs["ve"]
    ident = consts["ident"]

    xsT_ps = pp_t.tile([P, P], F32, tag="tp_ps")
    nc.tensor.transpose(out=xsT_ps[:], in_=xs_tile[:], identity=ident[:])
    xsT = wp.tile([P, P], F32, tag="xsT")
    nc.scalar.copy(out=xsT[:], in_=xsT_ps[:])

    kqv_ps = pp_mm.tile([P, 3 * HID], F32, tag="mm_ps")
    nc.tensor.matmul(out=kqv_ps[:], lhsT=xsT[:], rhs=consts["Wkqv"][t][:],
                     start=True, stop=True)
    kqv = wp.tile([P, 3 * HID], F32, tag="kqv")
    nc.vector.tensor_tensor(out=kqv[:], in0=kqv_ps[:], in1=consts["bkqv"][t][:],
                            op=OP.add)
    nc.sync.dma_start(kd_o[r0:r0 + P, :], kqv[:, :HID])

    qdT_ps = pp_t.tile([P, P], F32, tag="tp_ps")
    nc.tensor.transpose(out=qdT_ps[:], in_=kqv[:, HID:2 * HID], identity=ident[:])
    qdT = wp.tile([P, P], F32, tag="qdT")
    nc.scalar.copy(out=qdT[:], in_=qdT_ps[:])
    vdT_ps = pp_t.tile([P, P], F32, tag="tp_ps")
    nc.tensor.transpose(out=vdT_ps[:], in_=kqv[:, 2 * HID:], identity=ident[:])
    vdT = wp.tile([P, P], F32, tag="vdT")
    nc.scalar.copy(out=vdT[:], in_=vdT_ps[:])

    # per-type (q'-ets, ve-ets, q-slots, v-slots)
    q_ets = ((0, 1), (2,), (3,))[t]
    v_ets = ((2, 3), (0,), (1,))[t]
    q_slots = ((0, 1), (2,), (3,))[t]
    v_slots = ((0, 1), (2,), (3,))[t]
    rt0 = i_t * P
    for et, sl in zip(q_ets, q_slots):
        ps = pp_mm.tile([P, HID], F32, tag="mm_ps")
        nc.tensor.matmul(out=ps[:], lhsT=qdT[:], rhs=consts["BDaT"][et][:],
                         start=True, stop=True)
        sb = wp.tile([P, HID], F32, tag="qp_sb")
        nc.vector.tensor_copy(out=sb[:], in_=ps[:])
        nc.sync.dma_start(qp_o[cfg.QB[sl] + rt0:cfg.QB[sl] + rt0 + P, :], sb[:])
    for et, sl in zip(v_ets, v_slots):
        ps = pp_mm.tile([P, HID], F32, tag="mm_ps")
        nc.tensor.matmul(out=ps[:], lhsT=vdT[:], rhs=consts["BDm"][et][:],
                         start=True, stop=True)
        sb = wp.tile([P, HID], F32, tag="ve_sb")
        nc.vector.tensor_copy(out=sb[:], in_=ps[:])
        nc.sync.dma_start(ve_o[cfg.QB[sl] + rt0:cfg.QB[sl] + rt0 + P, :], sb[:])


def build_l1(cfg):
    """Launch 1: input proj + relu -> xs1; kqv chain -> kd/q'/ve tables."""
    nc = bacc.Bacc("TRN2", target_bir_lowering=False, debug=False,
                   num_devices=cfg.C)
    c = cfg
    xa = nc.dram_tensor("xa", [c.nap, P], F32, kind="ExternalInput").ap()
    xw = nc.dram_tensor("xw", [c.nwp, P], F32, kind="ExternalInput").ap()
    xo = nc.dram_tensor("xo", [c.nop, P], F32, kind="ExternalInput").ap()
    Win = nc.dram_tensor("Win", [3, P, HID], F32, kind="ExternalInput").ap()
    binp = nc.dram_tensor("bin", [3, P, HID], F32, kind="ExternalInput").ap()
    Wkqv = nc.dram_tensor("Wkqv", [3, P, 3 * HID], F32, kind="ExternalInput").ap()
    bkqv = nc.dram_tensor("bkqv", [3, P, 3 * HID], F32, kind="ExternalInput").ap()
    BDaT = nc.dram_tensor("BDaT", [4, P, HID], F32, kind="ExternalInput").ap()
    BDm = nc.dram_tensor("BDm", [4, P, HID], F32, kind="ExternalInput").ap()
    ident_d = nc.dram_tensor("ident", [P, P], F32, kind="ExternalInput").ap()

    xs_o = nc.dram_tensor("xs", [c.LOCN, P], F32, kind="ExternalOutput").ap()
    kd_o = nc.dram_tensor("kd", [c.LOCN, P], F32, kind="ExternalOutput").ap()
    qp_o = nc.dram_tensor("qp", [c.QTOT, P], F32, kind="ExternalOutput").ap()
    ve_o = nc.dram_tensor("ve", [c.QTOT, P], F32, kind="ExternalOutput").ap()

    xin = (xa, xw, xo)
    with tile.TileContext(nc) as tc:
        with tc.tile_pool(name="consts", bufs=1) as cp, \
             tc.tile_pool(name="work", bufs=4) as wp, \
             tc.tile_pool(name="ppt", bufs=4, space="PSUM") as pp_t, \
             tc.tile_pool(name="ppmm", bufs=4, space="PSUM") as pp_mm:
            consts = {
                "ident": _load_const(nc, cp, ident_d[:, :], (P, P)),
                "Win": [_load_const(nc, cp, Win[t], (P, HID)) for t in range(3)],
                "bin": [_load_const(nc, cp, binp[t], (P, HID)) for t in range(3)],
                "Wkqv": [_load_const(nc, cp, Wkqv[t], (P, 3 * HID)) for t in range(3)],
                "bkqv": [_load_const(nc, cp, bkqv[t], (P, 3 * HID)) for t in range(3)],
                "BDaT": [_load_const(nc, cp, BDaT[e], (P, HID)) for e in range(4)],
                "BDm": [_load_const(nc, cp, BDm[e], (P, HID)) for e in range(4)],
            }
            pools = {"cp": cp, "wp": wp, "pp_t": pp_t, "pp_mm": pp_mm}
            outs = {"kd": kd_o, "qp": qp_o, "ve": ve_o}
            for (t, i_t, r0) in _type_tiles(c):
                x_t = wp.tile([P, P], F32, tag="x_in")
                nc.sync.dma_start(x_t[:], xin[t][i_t * P:(i_t + 1) * P, :])
                xT_ps = pp_t.tile([P, P], F32, tag="tp_ps")
                nc.tensor.transpose(out=xT_ps[:], in_=x_t[:], identity=consts["ident"][:])
                xT = wp.tile([P, P], F32, tag="xT")
                nc.scalar.copy(out=xT[:], in_=xT_ps[:])
                pj_ps = pp_mm.tile([P, HID], F32, tag="mm_ps")
                nc.tensor.matmul(out=pj_ps[:], lhsT=xT[:], rhs=consts["Win"][t][:],
                                 start=True, stop=True)
                pj = wp.tile([P, HID], F32, tag="pj")
                nc.vector.tensor_tensor(out=pj[:], in0=pj_ps[:],
                                        in1=consts["bin"][t][:], op=OP.add)
                xs_t = wp.tile([P, HID], F32, tag="xs_t")
                nc.scalar.activation(out=xs_t[:], in_=pj[:], func=AF.Relu)
                nc.sync.dma_start(xs_o[r0:r0 + P, :], xs_t[:])
                _kqv_chain(nc, pools, c, consts, t, i_t, r0, xs_t, outs)
    nc.compile()
    return nc


def build_l23(cfg, NT, tws, last):
    """Launches 2/3: edge phase + node phase (+ head if last)."""
    nc = bacc.Bacc("TRN2", target_bir_lowering=False, debug=False,
                   num_devices=cfg.C)
    c = cfg
    kg_d = nc.dram_tensor("kg", [P, NT, HID], BF16, kind="ExternalInput").ap()
    vg_d = nc.dram_tensor("vg", [P, NT, HID], BF16, kind="ExternalInput").ap()
    qg_d = nc.dram_tensor("qg", [P, NT, HID], BF16, kind="ExternalInput").ap()
    xs_in = nc.dram_tensor("xsin", [c.LOCN, P], F32, kind="ExternalInput").ap()
    ti_t = nc.dram_tensor("dstlT", [P, NT], I32, kind="ExternalInput").ap()
    iota_d = nc.dram_tensor("iota", [P, P], I32, kind="ExternalInput").ap()
    ident_d = nc.dram_tensor("ident", [P, P], F32, kind="ExternalInput").ap()
    Wo_d = nc.dram_tensor("Wo", [3, P, HID], F32, kind="ExternalInput").ap()
    bo_d = nc.dram_tensor("bo", [3, P, HID], F32, kind="ExternalInput").ap()
    lng_d = nc.dram_tensor("lng", [3, P, HID], F32, kind="ExternalInput").ap()
    lnb_d = nc.dram_tensor("lnb", [3, P, HID], F32, kind="ExternalInput").ap()
    gs_d = nc.dram_tensor("gs", [3], F32, kind="ExternalInput").ap()  # unused on-device; values baked via bo/g mul
    if not last:
        Wkqv = nc.dram_tensor("Wkqv", [3, P, 3 * HID], F32, kind="ExternalInput").ap()
        bkqv = nc.dram_tensor("bkqv", [3, P, 3 * HID], F32, kind="ExternalInput").ap()
        BDaT = nc.dram_tensor("BDaT", [4, P, HID], F32, kind="ExternalInput").ap()
        BDm = nc.dram_tensor("BDm", [4, P, HID], F32, kind="ExternalInput").ap()
    else:
        wh_d = nc.dram_tensor("whead", [P, 1], F32, kind="ExternalInput").ap()
        bh_d = nc.dram_tensor("bh", [P, 1], F32, kind="ExternalInput").ap()

    if not last:
        xs_o = nc.dram_tensor("xs", [c.LOCN, P], F32, kind="ExternalOutput").ap()
        kd_o = nc.dram_tensor("kd", [c.LOCN, P], F32, kind="ExternalOutput").ap()
        qp_o = nc.dram_tensor("qp", [c.QTOT, P], F32, kind="ExternalOutput").ap()
        ve_o = nc.dram_tensor("ve", [c.QTOT, P], F32, kind="ExternalOutput").ap()
    else:
        dl_o = nc.dram_tensor("delta", [c.nap, 1], F32, kind="ExternalOutput").ap()

    # gains folded on host: bo tile already contains g*b_o. g itself baked as consts below.
    g_vals = None  # set in kernel() via attribute hack? no: pass via build arg
    g_list = build_l23.g_list  # [3] floats for this layer

    with tile.TileContext(nc) as tc:
        with tc.tile_pool(name="consts", bufs=1) as cp, \
             tc.tile_pool(name="idx", bufs=4) as idxp, \
             tc.tile_pool(name="gat", bufs=8) as gp, \
             tc.tile_pool(name="ework", bufs=8) as ewp, \
             tc.tile_pool(name="nwork", bufs=4) as wp, \
             tc.tile_pool(name="small", bufs=4) as sp, \
             tc.tile_pool(name="flush", bufs=4) as fp, \
             tc.tile_pool(name="dram", bufs=1, space="DRAM") as dp, \
             tc.tile_pool(name="ppe", bufs=3, space="PSUM") as pp_e, \
             tc.tile_pool(name="ppt", bufs=2, space="PSUM") as pp_t, \
             tc.tile_pool(name="ppmm", bufs=2, space="PSUM") as pp_mm:

            numer = dp.tile([c.LOCN, PAY], F32)
            eps_t = cp.tile([P, 1], F32, tag="lneps")
            nc.vector.memset(eps_t[:], 1e-5)

            consts = {
                "iota": _load_const(nc, cp, iota_d[:, :], (P, P), I32),
                "ident": _load_const(nc, cp, ident_d[:, :], (P, P)),
                "Wo": [_load_const(nc, cp, Wo_d[t], (P, HID)) for t in range(3)],
                "bo": [_load_const(nc, cp, bo_d[t], (P, HID)) for t in range(3)],
                "lng": [_load_const(nc, cp, lng_d[t], (P, HID)) for t in range(3)],
                "lnb": [_load_const(nc, cp, lnb_d[t], (P, HID)) for t in range(3)],
            }
            if not last:
                consts.update({
                    "Wkqv": [_load_const(nc, cp, Wkqv[t], (P, 3 * HID)) for t in range(3)],
                    "bkqv": [_load_const(nc, cp, bkqv[t], (P, 3 * HID)) for t in range(3)],
                    "BDaT": [_load_const(nc, cp, BDaT[e], (P, HID)) for e in range(4)],
                    "BDm": [_load_const(nc, cp, BDm[e], (P, HID)) for e in range(4)],
                })
            else:
                consts["whead"] = _load_const(nc, cp, wh_d[:, :], (P, 1))
                consts["bh"] = _load_const(nc, cp, bh_d[:, :], (P, 1))

            # ---------------- edge phase ----------------
            # streams are host-pre-gathered per edge (bf16); per window load
            # [P, T, HID] slices (partition-major contiguous), batch all DVE
            # ops across the window's T tiles, scatter via bf16 one-hot matmul.
            gtile = 0
            for w in range(c.NWIN):
                T = int(tws[w])
                g0 = gtile
                kgt = gp.tile([P, T, HID], BF16, tag="kgt")
                nc.sync.dma_start(kgt[:], kg_d[:, g0:g0 + T, :])
                vgt = gp.tile([P, T, HID], BF16, tag="vgt")
                nc.sync.dma_start(vgt[:], vg_d[:, g0:g0 + T, :])
                qgt = gp.tile([P, T, HID], BF16, tag="qgt")
                nc.sync.dma_start(qgt[:], qg_d[:, g0:g0 + T, :])
                dstl = idxp.tile([P, T], I32, tag="dstl")
                nc.sync.dma_start(dstl[:], ti_t[:, g0:g0 + T])

                psum_w = pp_e.tile([P, PAY], F32, tag="psw")
                prod = ewp.tile([P, T, HID], BF16, tag="prod")
                nc.vector.tensor_tensor(out=prod[:], in0=kgt[:], in1=qgt[:],
                                        op=OP.mult)
                alpha = ewp.tile([P, T, H], F32, tag="alpha")
                nc.vector.tensor_reduce(
                    out=alpha[:],
                    in_=prod[:].rearrange("p t (h d) -> p t h d", h=H),
                    axis=mybir.AxisListType.X, op=OP.add)
                payload = ewp.tile([P, T, PAY], BF16, tag="payload")
                ex = payload[:, :, HID:HID + H]
                nc.scalar.activation(out=ex, in_=alpha[:], func=AF.Exp)
                nc.vector.tensor_tensor(
                    out=payload[:, :, :HID].rearrange("p t (h d) -> p t h d", h=H),
                    in0=vgt[:].rearrange("p t (h d) -> p t h d", h=H),
                    in1=ex[:, :, :, None].to_broadcast([P, T, H, D]),
                    op=OP.mult)
                onehot = ewp.tile([P, T, P], BF16, tag="onehot")
                nc.vector.tensor_tensor(
                    out=onehot[:],
                    in0=dstl[:, :, None].to_broadcast([P, T, P]),
                    in1=consts["iota"][:, None, :].to_broadcast([P, T, P]),
                    op=OP.is_equal)
                for t in range(T):
                    nc.tensor.matmul(out=psum_w[:], lhsT=onehot[:, t, :],
                                     rhs=payload[:, t, :],
                                     start=(t == 0), stop=(t == T - 1))
                fl = fp.tile([P, PAY], F32, tag="fl")
                nc.vector.tensor_copy(out=fl[:], in_=psum_w[:])
                nc.sync.dma_start(numer[w * P:(w + 1) * P, :], fl[:])
                gtile += T

            # ---------------- node phase ----------------
            pools = {"cp": cp, "wp": wp, "pp_t": pp_t, "pp_mm": pp_mm}
            outs = None if last else {"kd": kd_o, "qp": qp_o, "ve": ve_o}
            for (t, i_t, r0) in _type_tiles(c):
                nm = wp.tile([P, PAY], F32, tag="nm")
                nc.sync.dma_start(nm[:], numer[r0:r0 + P, :])
                den = sp.tile([P, H], F32, tag="den")
                nc.vector.tensor_scalar_add(den[:], nm[:, HID:HID + H], 1e-16)
                rec = sp.tile([P, H], F32, tag="rec")
                nc.vector.reciprocal(rec[:], den[:])
                agg = wp.tile([P, HID], F32, tag="agg")
                nc.vector.tensor_tensor(
                    out=agg[:].rearrange("p (h d) -> p h d", h=H),
                    in0=nm[:, :HID].rearrange("p (h d) -> p h d", h=H),
                    in1=rec[:, :, None].to_broadcast([P, H, D]),
                    op=OP.mult)
                glu = wp.tile([P, HID], F32, tag="glu")
                if os.environ.get("HGT_BACKEND", "hw") == "sim":
                    # CoreSim has no Gelu LUT: tanh approximation (dev only)
                    t1 = wp.tile([P, HID], F32, tag="gelu_t1")
                    nc.vector.tensor_tensor(out=t1[:], in0=agg[:], in1=agg[:], op=OP.mult)
                    nc.vector.tensor_tensor(out=t1[:], in0=t1[:], in1=agg[:], op=OP.mult)
                    nc.vector.tensor_scalar(out=t1[:], in0=t1[:], scalar1=0.044715,
                                            scalar2=None, op0=OP.mult)
                    nc.vector.tensor_tensor(out=t1[:], in0=t1[:], in1=agg[:], op=OP.add)
                    nc.scalar.activation(out=t1[:], in_=t1[:], func=AF.Tanh,
                                         scale=0.7978845608028654)
                    nc.vector.tensor_scalar(out=t1[:], in0=t1[:], scalar1=0.5,
                                            scalar2=0.5, op0=OP.mult, op1=OP.add)
                    nc.vector.tensor_tensor(out=glu[:], in0=t1[:], in1=agg[:], op=OP.mult)
                else:
                    nc.scalar.activation(out=glu[:], in_=agg[:], func=AF.Gelu)
                gluT_ps = pp_t.tile([P, P], F32, tag="tp_ps")
                nc.tensor.transpose(out=gluT_ps[:], in_=glu[:], identity=consts["ident"][:])
                gluT = wp.tile([P, P], F32, tag="gluT")
                nc.scalar.copy(out=gluT[:], in_=gluT_ps[:])
                o_ps = pp_mm.tile([P, HID], F32, tag="mm_ps")
                nc.tensor.matmul(out=o_ps[:], lhsT=gluT[:], rhs=consts["Wo"][t][:],
                                 start=True, stop=True)
                # o3 = g*o + (g*b_o) + (1-g)*xs  (bo const already has g*b_o)
                xs_t = wp.tile([P, HID], F32, tag="xs_ld")
                nc.sync.dma_start(xs_t[:], xs_in[r0:r0 + P, :])
                o1 = wp.tile([P, HID], F32, tag="o1")
                nc.vector.tensor_scalar_mul(o1[:], o_ps[:], float(g_list[t]))
                nc.vector.tensor_tensor(out=o1[:], in0=o1[:], in1=consts["bo"][t][:], op=OP.add)
                xs_s = wp.tile([P, HID], F32, tag="xs_s")
                nc.vector.tensor_scalar_mul(xs_s[:], xs_t[:], float(1.0 - g_list[t]))
                nc.vector.tensor_tensor(out=o1[:], in0=o1[:], in1=xs_s[:], op=OP.add)
                # LayerNorm + relu
                stats = sp.tile([P, nc.vector.BN_STATS_DIM], F32, tag="stats")
                nc.vector.bn_stats(out=stats[:], in_=o1[:])
                mv = sp.tile([P, nc.vector.BN_AGGR_DIM], F32, tag="mv")
                nc.vector.bn_aggr(out=mv[:], in_=stats[:])
                rstd = sp.tile([P, 1], F32, tag="rstd")
                nc.scalar.activation(out=rstd[:], in_=mv[:, 1:2], func=AF.Sqrt,
                                     bias=eps_t[:, 0:1])
                nc.vector.reciprocal(rstd[:], rstd[:])
                xh = wp.tile([P, HID], F32, tag="xh")
                nc.vector.tensor_scalar(
                    out=xh[:], in0=o1[:], scalar1=mv[:, 0:1], scalar2=rstd[:, 0:1],
                    op0=OP.subtract, op1=OP.mult)
                nc.vector.tensor_tensor(out=xh[:], in0=xh[:], in1=consts["lng"][t][:], op=OP.mult)
                nc.vector.tensor_tensor(out=xh[:], in0=xh[:], in1=consts["lnb"][t][:], op=OP.add)
                xs_new = wp.tile([P, HID], F32, tag="xs_new")
                nc.scalar.activation(out=xs_new[:], in_=xh[:], func=AF.Relu)
                if not last:
                    nc.sync.dma_start(xs_o[r0:r0 + P, :], xs_new[:])
                    _kqv_chain(nc, pools, c, consts, t, i_t, r0, xs_new, outs)
                elif t == 0:
                    xnT_ps = pp_t.tile([P, P], F32, tag="tp_ps")
                    nc.tensor.transpose(out=xnT_ps[:], in_=xs_new[:], identity=consts["ident"][:])
                    xnT = wp.tile([P, P], F32, tag="xnT")
                    nc.scalar.copy(out=xnT[:], in_=xnT_ps[:])
                    d_ps = pp_mm.tile([P, 1], F32, tag="mm_ps")
                    nc.tensor.matmul(out=d_ps[:], lhsT=xnT[:], rhs=consts["whead"][:],
                                     start=True, stop=True)
                    dl = sp.tile([P, 1], F32, tag="dl")
                    nc.vector.tensor_tensor(out=dl[:], in0=d_ps[:], in1=consts["bh"][:], op=OP.add)
                    nc.sync.dma_start(dl_o[i_t * P:(i_t + 1) * P, :], dl[:])
    nc.compile()
    return nc


build_l23.g_list = None


# ---------------------------------------------------------------------------
# Runner
# ---------------------------------------------------------------------------

LAUNCH_TIMES_NS = []
TRACE_DIRS = []


def _run(nc, in_maps, cfg):
    backend = os.environ.get("HGT_BACKEND", "hw")
    if backend == "sim":
        from concourse.bass_interp import CoreSim
        results = []
        for m in in_maps:
            sim = CoreSim(nc, trace=False, require_finite=False, require_nnan=False)
            for k, v in m.items():
                sim.tensor(k)[:] = v
            sim.simulate(check_with_hw=False)
            out = {}
            for alloc in nc.m.functions[0].allocations:
                if isinstance(alloc, mybir.MemoryLocationSet) and alloc.kind == "ExternalOutput":
                    name = alloc.memorylocations[0].name
                    out[name] = sim.tensor(name).copy()
            results.append(out)
        return results
    else:
        from concourse.bass_utils import run_bass_kernel_spmd
        trace = os.environ.get("HGT_TRACE", "0") == "1"
        res = run_bass_kernel_spmd(nc, in_maps, core_ids=list(range(cfg.C)),
                                   trace=trace)
        if trace:
            LAUNCH_TIMES_NS.append(res.exec_time_ns)
            it = res.instructions_and_trace
            TRACE_DIRS.append(getattr(it, "trace_path", it))
        return res.results


# ---------------------------------------------------------------------------
# Main entry
# ---------------------------------------------------------------------------

def kernel(**inputs):
    cfg = Cfg()
    return _kernel_impl(cfg, inputs)


def _kernel_impl(cfg, inputs):
    c = cfg
    prm = prep_params(c, inputs)
    g = prep_graph(c, inputs)
    NT, tws = g["NT"], g["tws"]

    # ---- launch 1
    nc1 = build_l1(c)
    in_maps = []
    xa = np.asarray(inputs["x_a"], np.float32)
    xw = np.asarray(inputs["x_w"], np.float32)
    xo = np.asarray(inputs["x_o"], np.float32)

    def padx(x, n, npad):
        out = np.zeros((npad, P), np.float32)
        out[:n, :64] = x
        return out

    for cc in range(c.C):
        in_maps.append({
            "xa": padx(xa[cc * c.nac:(cc + 1) * c.nac], c.nac, c.nap),
            "xw": padx(xw[cc * c.nwc:(cc + 1) * c.nwc], c.nwc, c.nwp),
            "xo": padx(xo[cc * c.noc:(cc + 1) * c.noc], c.noc, c.nop),
            "Win": prm["Win"], "bin": prm["bin"],
            "Wkqv": prm["Wkqv0"], "bkqv": prm["bkqv0"],
            "BDaT": prm["BDaT0"], "BDm": prm["BDm0"],
            "ident": prm["ident"],
        })
    r1 = _run(nc1, in_maps, c)

    def assemble_tables(res):
        """Build global kd table + per-core q' tables + global ve table."""
        kd_tab = np.empty((c.KD_ROWS, HID), np.float32)
        kd_tab[-1] = 1.0
        ve_tab = np.empty((c.VE_ROWS, HID), np.float32)
        ve_tab[-1] = 0.0
        qp_tabs = []
        for cc in range(c.C):
            kd = res[cc]["kd"]
            ve = res[cc]["ve"]
            # kd local [a|w|o] -> global
            kd_tab[c.KOFF[0] + cc * c.nac:c.KOFF[0] + (cc + 1) * c.nac] = kd[:c.nac]
            kd_tab[c.KOFF[1] + cc * c.nwc:c.KOFF[1] + (cc + 1) * c.nwc] = \
                kd[c.base_local[1]:c.base_local[1] + c.nwc]
            kd_tab[c.KOFF[2] + cc * c.noc:c.KOFF[2] + (cc + 1) * c.noc] = \
                kd[c.base_local[2]:c.base_local[2] + c.noc]
            # ve local slots [a-et2, a-et3, w-et0, o-et1] -> global stacked
            ve_tab[c.VOFF[2] + cc * c.nac:c.VOFF[2] + (cc + 1) * c.nac] = \
                ve[c.QB[0]:c.QB[0] + c.nac]
            ve_tab[c.VOFF[3] + cc * c.nac:c.VOFF[3] + (cc + 1) * c.nac] = \
                ve[c.QB[1]:c.QB[1] + c.nac]
            ve_tab[c.VOFF[0] + cc * c.nwc:c.VOFF[0] + (cc + 1) * c.nwc] = \
                ve[c.QB[2]:c.QB[2] + c.nwc]
            ve_tab[c.VOFF[1] + cc * c.noc:c.VOFF[1] + (cc + 1) * c.noc] = \
                ve[c.QB[3]:c.QB[3] + c.noc]
            # pad-edge q rows are -8.0: with pad k rows = 1.0 the pad alpha is
            # 128 * -8 = -1024 (bf16-safe), exp -> 0.
            qp = np.vstack([res[cc]["qp"], np.full((1, HID), -8.0, np.float32)])
            qp_tabs.append(qp)
        return kd_tab, ve_tab, qp_tabs

    import ml_dtypes
    bf16 = ml_dtypes.bfloat16

    # ---- launches 2 and 3
    res = r1
    for l, last in ((1, False), (2, True)):
        kd_tab, ve_tab, qp_tabs = assemble_tables(res)
        lay = l - 1  # layer params index: launch2 -> layer 0, launch3 -> layer 1
        build_l23.g_list = prm[f"g{lay}"]
        nc = build_l23(c, NT, tws, last)
        in_maps = []
        for cc in range(c.C):
            ti = g["tile_idx"][cc]  # [NT, P, 4]
            kg_s = np.ascontiguousarray(
                kd_tab[ti[:, :, 0]].transpose(1, 0, 2)).astype(bf16)
            vg_s = np.ascontiguousarray(
                ve_tab[ti[:, :, 1]].transpose(1, 0, 2)).astype(bf16)
            qg_s = np.ascontiguousarray(
                qp_tabs[cc][ti[:, :, 2]].transpose(1, 0, 2)).astype(bf16)
            dstlT = np.ascontiguousarray(ti[:, :, 3].T)
            m = {
                "kg": kg_s, "vg": vg_s, "qg": qg_s,
                "xsin": res[cc]["xs"],
                "dstlT": dstlT,
                "iota": prm["iota"], "ident": prm["ident"],
                "Wo": prm[f"Wo{lay}"], "bo": prm[f"bo{lay}"],
                "lng": prm[f"lng{lay}"], "lnb": prm[f"lnb{lay}"],
                "gs": prm[f"g{lay}"],
            }
            if not last:
                m.update({"Wkqv": prm[f"Wkqv{lay + 1}"], "bkqv": prm[f"bkqv{lay + 1}"],
                          "BDaT": prm[f"BDaT{lay + 1}"], "BDm": prm[f"BDm{lay + 1}"]})
            else:
                m.update({"whead": prm["whead"], "bh": prm["bh"]})
            in_maps.append(m)
        res = _run(nc, in_maps, c)

    out = np.concatenate([res[cc]["delta"][:c.nac, 0] for cc in range(c.C)])
    return out.astype(np.float32)



# revision 9
# speedup vs baseline: 1.4109x; 1.4109x over previous
"""HGT regressor on 8 Trainium2 NeuronCores (Bass/Tile).

Strategy (graph/data parallel, hint-following):
  - Nodes of each type are partitioned contiguously across the 8 cores
    (a: 12500/core, w: 2500/core, o: 6250/core). Each core owns the edges
    whose *destination* lies in its node shard.
  - Per layer, each core computes K = kqv[:, :128] (raw) and the per-edge-type
    source-side V transform (m_rel folded at source) plus the destination-side
    Q transform (a_rel * p_rel * scale folded into Q) for its own nodes only.
  - The full K / V_et tables are exchanged between layer launches via the host
    (replicated to all cores), i.e. host-mediated all-gather. Q' stays local.
  - Edge phase per core: edges sorted by local destination row, grouped into
    128-node windows; per 128-edge tile: indirect-DMA gathers of K[src],
    V_et[src], Q'_et[dst]; alpha = sum_h(K*Q'); ex = exp(alpha); payload
    [ex*V | ex] is scatter-added into a PSUM window accumulator via a
    one-hot matmul; windows flush densely to a numer/den table in DRAM.
  - Node phase per core: agg = numer/den, gelu, W_o, gated skip, LayerNorm,
    relu, then next-layer projections (or the scalar head in the last layer).
  - Softmax needs no running max: alpha = q'k with these parameter scales is
    O(1); exp cannot overflow, and softmax is shift-invariant anyway.
"""
import os
import sys

sys.path.insert(0, "/opt/trn_rl_repo")

import numpy as np

import concourse.bass as bass
import concourse.mybir as mybir
import concourse.tile as tile
from concourse import bacc

P = 128
H, D, HID = 4, 32, 128
PAY = HID + H  # 132
F32 = mybir.dt.float32
F16 = mybir.dt.float16
BF16 = mybir.dt.bfloat16
I32 = mybir.dt.int32
AF = mybir.ActivationFunctionType
OP = mybir.AluOpType


def _ceil(a, b):
    return (a + b - 1) * b // b if False else -(-a // b) * b


def cdiv(a, b):
    return -(-a // b)


class Cfg:
    """All sizes derived from problem scale; supports mini-scale testing."""

    def __init__(self, NA=100000, NWK=20000, NO=50000, E=150000, C=8):
        self.NA, self.NWK, self.NO, self.E, self.C = NA, NWK, NO, E, C
        assert NA % C == 0 and NWK % C == 0 and NO % C == 0
        self.nac, self.nwc, self.noc = NA // C, NWK // C, NO // C
        self.nap, self.nwp, self.nop = (
            cdiv(self.nac, P) * P,
            cdiv(self.nwc, P) * P,
            cdiv(self.noc, P) * P,
        )
        # local node-row layout (numer/xs/kd rows): [a | w | o], each padded
        self.base_local = (0, self.nap, self.nap + self.nwp)
        self.LOCN = self.nap + self.nwp + self.nop
        self.NWIN = self.LOCN // P
        # per-type tile counts
        self.ntile_a, self.ntile_w, self.ntile_o = (
            self.nap // P,
            self.nwp // P,
            self.nop // P,
        )
        # q' local layout: slots [a-et0, a-et1, w-et2, o-et3]
        self.QB = (0, self.nap, 2 * self.nap, 2 * self.nap + self.nwp)
        self.QTOT = 2 * self.nap + self.nwp + self.nop
        # ve local layout (same bases): slots [a-et2, a-et3, w-et0, o-et1]
        # global kd table layout: [a 0..NA | w | o] + trash
        self.KOFF = (0, NA, NA + NWK)
        self.KD_ROWS = NA + NWK + NO + 1
        # global stacked ve table: [et0 w | et1 o | et2 a | et3 a] + trash
        self.VOFF = (0, NWK, NWK + NO, NWK + NO + NA)
        self.VE_ROWS = NWK + NO + 2 * NA + 1


# edge types: (src_type, dst_type)
ETYPES = ((1, 0), (2, 0), (0, 1), (0, 2))


# ---------------------------------------------------------------------------
# Host-side preprocessing
# ---------------------------------------------------------------------------

def prep_graph(cfg, inputs):
    """Compute per-core edge tile indices. Shared across both layers.

    Returns dict with:
      NT: static tile count (same all cores)
      tws: [NWIN] tiles per window (static across cores)
      tile_idx: [C][NT, P, 4] int32  (kidx, vidx, qidx, dst_local)
    """
    c = cfg
    edges = []  # per et: (src, dst)
    for name_s, name_d in (("src_wa", "dst_wa"), ("src_oa", "dst_oa"),
                           ("src_aw", "dst_aw"), ("src_ao", "dst_ao")):
        edges.append((np.asarray(inputs[name_s]), np.asarray(inputs[name_d])))

    shard_n = (c.nac, c.nwc, c.noc)
    # concat all ets with global indices
    K_TRASH = c.KD_ROWS - 1
    V_TRASH = c.VE_ROWS - 1
    Q_TRASH = c.QTOT  # row appended by host to the q' table

    all_core = []
    koff_by_et = (c.KOFF[1], c.KOFF[2], c.KOFF[0], c.KOFF[0])  # src type offset in kd
    for et, (st, dt) in enumerate(ETYPES):
        src, dst = edges[et]
        kidx = koff_by_et[et] + src
        vidx = c.VOFF[et] + src
        core = dst // shard_n[dt]
        dloc = dst - core * shard_n[dt]  # dst index within its type shard
        # local numer row / q' row
        tb = (c.base_local[0], c.base_local[1], c.base_local[2])[dt]
        row = tb + dloc
        qslot = {0: 0, 1: 1, 2: 2, 3: 3}[et]
        qidx = c.QB[qslot] + dloc
        all_core.append((core, row, kidx, vidx, qidx))

    core_cat = np.concatenate([a[0] for a in all_core])
    row_cat = np.concatenate([a[1] for a in all_core])
    k_cat = np.concatenate([a[2] for a in all_core])
    v_cat = np.concatenate([a[3] for a in all_core])
    q_cat = np.concatenate([a[4] for a in all_core])

    # per-core, per-window edge counts -> static tile structure
    win_cat = row_cat // P
    counts = np.zeros((c.C, c.NWIN), np.int64)
    for cc in range(c.C):
        m = core_cat == cc
        counts[cc] = np.bincount(win_cat[m], minlength=c.NWIN)
    tws = np.maximum(cdiv(counts.max(axis=0), P), 1)  # >=1 tile per window
    NT = int(tws.sum())
    tile_base = np.zeros(c.NWIN, np.int64)
    tile_base[1:] = np.cumsum(tws)[:-1]

    tile_idx = np.zeros((c.C, NT, P, 4), np.int32)
    # fill pads with trash rows -> ex = 0 contributions
    tile_idx[:, :, :, 0] = K_TRASH
    tile_idx[:, :, :, 1] = V_TRASH
    tile_idx[:, :, :, 2] = Q_TRASH
    tile_idx[:, :, :, 3] = 0
    for cc in range(c.C):
        m = core_cat == cc
        rows = row_cat[m]
        order = np.argsort(rows, kind="stable")
        rows = rows[order]
        ks, vs, qs = k_cat[m][order], v_cat[m][order], q_cat[m][order]
        wins = rows // P
        dstl = rows % P
        # position within window
        wstart = np.searchsorted(wins, np.arange(c.NWIN), side="left")
        pos = np.arange(rows.size) - wstart[wins]
        slot_t = pos // P   # tile within window
        slot_p = pos % P    # partition
        gt = tile_base[wins] + slot_t  # global tile id
        tile_idx[cc, gt, slot_p, 0] = ks
        tile_idx[cc, gt, slot_p, 1] = vs
        tile_idx[cc, gt, slot_p, 2] = qs
        tile_idx[cc, gt, slot_p, 3] = dstl
    return {"NT": NT, "tws": tws.astype(np.int64), "tile_idx": tile_idx,
            "tile_base": tile_base}


def blockdiag(M):
    out = np.zeros((HID, HID), np.float32)
    for h in range(H):
        out[h * D:(h + 1) * D, h * D:(h + 1) * D] = M[h]
    return out


def prep_params(cfg, inputs):
    """Fold and lay out all parameters (host, tiny)."""
    scale = np.float32(1.0 / np.sqrt(D))
    a_rel = np.asarray(inputs["a_rel"])
    m_rel = np.asarray(inputs["m_rel"])
    p_rel = np.asarray(inputs["p_rel"])
    prm = {}
    rep = lambda v, w: np.broadcast_to(np.asarray(v, np.float32)[None, :], (P, w)).copy()
    for l in range(2):
        BDaT, BDm = [], []
        for et in range(4):
            a_eff = a_rel[l, et] * (p_rel[l, et] * scale)[:, None, None]
            BDaT.append(blockdiag(a_eff).T.copy())
            BDm.append(blockdiag(m_rel[l, et]))
        prm[f"BDaT{l}"] = np.stack(BDaT)  # [4,128,128]
        prm[f"BDm{l}"] = np.stack(BDm)
        prm[f"Wkqv{l}"] = np.asarray(inputs["W_kqv"])[l]       # [3,128,384]
        prm[f"bkqv{l}"] = np.stack([rep(np.asarray(inputs["b_kqv"])[l, t], 3 * HID) for t in range(3)])
        prm[f"Wo{l}"] = np.asarray(inputs["W_o"])[l]           # [3,128,128]
        g = 1.0 / (1.0 + np.exp(-np.asarray(inputs["skip_p"], np.float64)))  # [2,3]
        prm[f"g{l}"] = g[l].astype(np.float32)
        prm[f"bo{l}"] = np.stack([rep(np.asarray(inputs["b_o"])[l, t] * g[l, t], HID) for t in range(3)])
        prm[f"lng{l}"] = np.stack([rep(np.asarray(inputs["ln_g"])[l, t], HID) for t in range(3)])
        prm[f"lnb{l}"] = np.stack([rep(np.asarray(inputs["ln_b"])[l, t], HID) for t in range(3)])
    # input proj, padded to 128 contraction
    W_in = np.asarray(inputs["W_in"])  # [3,64,128]
    Wp = np.zeros((3, 128, HID), np.float32)
    Wp[:, :64, :] = W_in
    prm["Win"] = Wp
    prm["bin"] = np.stack([rep(np.asarray(inputs["b_in"])[t], HID) for t in range(3)])
    prm["whead"] = np.asarray(inputs["w_head"], np.float32)  # [128,1]
    prm["bh"] = np.full((P, 1), float(np.asarray(inputs["b_head"])[0] + np.asarray(inputs["base"])[0]), np.float32)
    prm["iota"] = np.broadcast_to(np.arange(128, dtype=np.int32)[None, :], (P, 128)).copy()
    prm["ident"] = np.eye(128, dtype=np.float32)
    return prm


# ---------------------------------------------------------------------------
# Builders
# ---------------------------------------------------------------------------

_CONST_N = [0]


def _load_const(nc, cp, ap, shape, dtype=F32):
    _CONST_N[0] += 1
    t = cp.tile(list(shape), dtype, tag=f"cst{_CONST_N[0]}")
    nc.sync.dma_start(t[:], ap)
    return t


def _type_tiles(cfg):
    """Yield (t, i_t, r0) for all node tiles: type, tile-in-type, local row base."""
    out = []
    for t, (ntile, b) in enumerate(
        zip((cfg.ntile_a, cfg.ntile_w, cfg.ntile_o), cfg.base_local)
    ):
        for i in range(ntile):
            out.append((t, i, b + i * P))
    return out


def _kqv_chain(nc, pools, cfg, consts, t, i_t, r0, xs_tile, outs):
    """Emit next-layer projections for one node tile (node-major xs_tile [128,128]).

    Writes kd rows (local), q'_et rows, ve_et rows via DMA to outs dict.
    """
    cp, wp, pp_t, pp_mm = pools["cp"], pools["wp"], pools["pp_t"], pools["pp_mm"]
    kd_o, qp_o, ve_o = outs["kd"], outs["qp"], outs["ve"]
    ident = consts["ident"]

    xsT_ps = pp_t.tile([P, P], F32, tag="tp_ps")
    nc.tensor.transpose(out=xsT_ps[:], in_=xs_tile[:], identity=ident[:])
    xsT = wp.tile([P, P], F32, tag="xsT")
    nc.scalar.copy(out=xsT[:], in_=xsT_ps[:])

    kqv_ps = pp_mm.tile([P, 3 * HID], F32, tag="mm_ps")
    nc.tensor.matmul(out=kqv_ps[:], lhsT=xsT[:], rhs=consts["Wkqv"][t][:],
                     start=True, stop=True)
    kqv = wp.tile([P, 3 * HID], F32, tag="kqv")
    nc.vector.tensor_tensor(out=kqv[:], in0=kqv_ps[:], in1=consts["bkqv"][t][:],
                            op=OP.add)
    nc.sync.dma_start(kd_o[r0:r0 + P, :], kqv[:, :HID])

    qdT_ps = pp_t.tile([P, P], F32, tag="tp_ps")
    nc.tensor.transpose(out=qdT_ps[:], in_=kqv[:, HID:2 * HID], identity=ident[:])
    qdT = wp.tile([P, P], F32, tag="qdT")
    nc.scalar.copy(out=qdT[:], in_=qdT_ps[:])
    vdT_ps = pp_t.tile([P, P], F32, tag="tp_ps")
    nc.tensor.transpose(out=vdT_ps[:], in_=kqv[:, 2 * HID:], identity=ident[:])
    vdT = wp.tile([P, P], F32, tag="vdT")
    nc.scalar.copy(out=vdT[:], in_=vdT_ps[:])

    # per-type (q'-ets, ve-ets, q-slots, v-slots)
    q_ets = ((0, 1), (2,), (3,))[t]
    v_ets = ((2, 3), (0,), (1,))[t]
    q_slots = ((0, 1), (2,), (3,))[t]
    v_slots = ((0, 1), (2,), (3,))[t]
    rt0 = i_t * P
    for et, sl in zip(q_ets, q_slots):
        ps = pp_mm.tile([P, HID], F32, tag="mm_ps")
        nc.tensor.matmul(out=ps[:], lhsT=qdT[:], rhs=consts["BDaT"][et][:],
                         start=True, stop=True)
        sb = wp.tile([P, HID], F32, tag="qp_sb")
        nc.vector.tensor_copy(out=sb[:], in_=ps[:])
        nc.sync.dma_start(qp_o[cfg.QB[sl] + rt0:cfg.QB[sl] + rt0 + P, :], sb[:])
    for et, sl in zip(v_ets, v_slots):
        ps = pp_mm.tile([P, HID], F32, tag="mm_ps")
        nc.tensor.matmul(out=ps[:], lhsT=vdT[:], rhs=consts["BDm"][et][:],
                         start=True, stop=True)
        sb = wp.tile([P, HID], F32, tag="ve_sb")
        nc.vector.tensor_copy(out=sb[:], in_=ps[:])
        nc.sync.dma_start(ve_o[cfg.QB[sl] + rt0:cfg.QB[sl] + rt0 + P, :], sb[:])


def build_l1(cfg):
    """Launch 1: input proj + relu -> xs1; kqv chain -> kd/q'/ve tables."""
    nc = bacc.Bacc("TRN2", target_bir_lowering=False, debug=False,
                   num_devices=cfg.C)
    c = cfg
    xa = nc.dram_tensor("xa", [c.nap, P], F32, kind="ExternalInput").ap()
    xw = nc.dram_tensor("xw", [c.nwp, P], F32, kind="ExternalInput").ap()
    xo = nc.dram_tensor("xo", [c.nop, P], F32, kind="ExternalInput").ap()
    Win = nc.dram_tensor("Win", [3, P, HID], F32, kind="ExternalInput").ap()
    binp = nc.dram_tensor("bin", [3, P, HID], F32, kind="ExternalInput").ap()
    Wkqv = nc.dram_tensor("Wkqv", [3, P, 3 * HID], F32, kind="ExternalInput").ap()
    bkqv = nc.dram_tensor("bkqv", [3, P, 3 * HID], F32, kind="ExternalInput").ap()
    BDaT = nc.dram_tensor("BDaT", [4, P, HID], F32, kind="ExternalInput").ap()
    BDm = nc.dram_tensor("BDm", [4, P, HID], F32, kind="ExternalInput").ap()
    ident_d = nc.dram_tensor("ident", [P, P], F32, kind="ExternalInput").ap()

    xs_o = nc.dram_tensor("xs", [c.LOCN, P], F32, kind="ExternalOutput").ap()
    kd_o = nc.dram_tensor("kd", [c.LOCN, P], F32, kind="ExternalOutput").ap()
    qp_o = nc.dram_tensor("qp", [c.QTOT, P], F32, kind="ExternalOutput").ap()
    ve_o = nc.dram_tensor("ve", [c.QTOT, P], F32, kind="ExternalOutput").ap()

    xin = (xa, xw, xo)
    with tile.TileContext(nc) as tc:
        with tc.tile_pool(name="consts", bufs=1) as cp, \
             tc.tile_pool(name="work", bufs=4) as wp, \
             tc.tile_pool(name="ppt", bufs=4, space="PSUM") as pp_t, \
             tc.tile_pool(name="ppmm", bufs=4, space="PSUM") as pp_mm:
            consts = {
                "ident": _load_const(nc, cp, ident_d[:, :], (P, P)),
                "Win": [_load_const(nc, cp, Win[t], (P, HID)) for t in range(3)],
                "bin": [_load_const(nc, cp, binp[t], (P, HID)) for t in range(3)],
                "Wkqv": [_load_const(nc, cp, Wkqv[t], (P, 3 * HID)) for t in range(3)],
                "bkqv": [_load_const(nc, cp, bkqv[t], (P, 3 * HID)) for t in range(3)],
                "BDaT": [_load_const(nc, cp, BDaT[e], (P, HID)) for e in range(4)],
                "BDm": [_load_const(nc, cp, BDm[e], (P, HID)) for e in range(4)],
            }
            pools = {"cp": cp, "wp": wp, "pp_t": pp_t, "pp_mm": pp_mm}
            outs = {"kd": kd_o, "qp": qp_o, "ve": ve_o}
            for (t, i_t, r0) in _type_tiles(c):
                x_t = wp.tile([P, P], F32, tag="x_in")
                nc.sync.dma_start(x_t[:], xin[t][i_t * P:(i_t + 1) * P, :])
                xT_ps = pp_t.tile([P, P], F32, tag="tp_ps")
                nc.tensor.transpose(out=xT_ps[:], in_=x_t[:], identity=consts["ident"][:])
                xT = wp.tile([P, P], F32, tag="xT")
                nc.scalar.copy(out=xT[:], in_=xT_ps[:])
                pj_ps = pp_mm.tile([P, HID], F32, tag="mm_ps")
                nc.tensor.matmul(out=pj_ps[:], lhsT=xT[:], rhs=consts["Win"][t][:],
                                 start=True, stop=True)
                pj = wp.tile([P, HID], F32, tag="pj")
                nc.vector.tensor_tensor(out=pj[:], in0=pj_ps[:],
                                        in1=consts["bin"][t][:], op=OP.add)
                xs_t = wp.tile([P, HID], F32, tag="xs_t")
                nc.scalar.activation(out=xs_t[:], in_=pj[:], func=AF.Relu)
                nc.sync.dma_start(xs_o[r0:r0 + P, :], xs_t[:])
                _kqv_chain(nc, pools, c, consts, t, i_t, r0, xs_t, outs)
    nc.compile()
    return nc


def build_l23(cfg, NT, tws, last):
    """Launches 2/3: edge phase + node phase (+ head if last)."""
    nc = bacc.Bacc("TRN2", target_bir_lowering=False, debug=False,
                   num_devices=cfg.C)
    c = cfg
    kg_d = nc.dram_tensor("kg", [P, NT, HID], BF16, kind="ExternalInput").ap()
    vg_d = nc.dram_tensor("vg", [P, NT, HID], BF16, kind="ExternalInput").ap()
    qg_d = nc.dram_tensor("qg", [P, NT, HID], BF16, kind="ExternalInput").ap()
    xs_in = nc.dram_tensor("xsin", [c.LOCN, P], F32, kind="ExternalInput").ap()
    ti_t = nc.dram_tensor("dstlT", [P, NT], I32, kind="ExternalInput").ap()
    iota_d = nc.dram_tensor("iota", [P, P], I32, kind="ExternalInput").ap()
    ident_d = nc.dram_tensor("ident", [P, P], F32, kind="ExternalInput").ap()
    Wo_d = nc.dram_tensor("Wo", [3, P, HID], F32, kind="ExternalInput").ap()
    bo_d = nc.dram_tensor("bo", [3, P, HID], F32, kind="ExternalInput").ap()
    lng_d = nc.dram_tensor("lng", [3, P, HID], F32, kind="ExternalInput").ap()
    lnb_d = nc.dram_tensor("lnb", [3, P, HID], F32, kind="ExternalInput").ap()
    gs_d = nc.dram_tensor("gs", [3], F32, kind="ExternalInput").ap()  # unused on-device; values baked via bo/g mul
    if not last:
        Wkqv = nc.dram_tensor("Wkqv", [3, P, 3 * HID], F32, kind="ExternalInput").ap()
        bkqv = nc.dram_tensor("bkqv", [3, P, 3 * HID], F32, kind="ExternalInput").ap()
        BDaT = nc.dram_tensor("BDaT", [4, P, HID], F32, kind="ExternalInput").ap()
        BDm = nc.dram_tensor("BDm", [4, P, HID], F32, kind="ExternalInput").ap()
    else:
        wh_d = nc.dram_tensor("whead", [P, 1], F32, kind="ExternalInput").ap()
        bh_d = nc.dram_tensor("bh", [P, 1], F32, kind="ExternalInput").ap()

    if not last:
        xs_o = nc.dram_tensor("xs", [c.LOCN, P], F32, kind="ExternalOutput").ap()
        kd_o = nc.dram_tensor("kd", [c.LOCN, P], F32, kind="ExternalOutput").ap()
        qp_o = nc.dram_tensor("qp", [c.QTOT, P], F32, kind="ExternalOutput").ap()
        ve_o = nc.dram_tensor("ve", [c.QTOT, P], F32, kind="ExternalOutput").ap()
    else:
        dl_o = nc.dram_tensor("delta", [c.nap, 1], F32, kind="ExternalOutput").ap()

    # gains folded on host: bo tile already contains g*b_o. g itself baked as consts below.
    g_vals = None  # set in kernel() via attribute hack? no: pass via build arg
    g_list = build_l23.g_list  # [3] floats for this layer

    with tile.TileContext(nc) as tc:
        with tc.tile_pool(name="consts", bufs=1) as cp, \
             tc.tile_pool(name="idx", bufs=4) as idxp, \
             tc.tile_pool(name="gat", bufs=6) as gp, \
             tc.tile_pool(name="ework", bufs=6) as ewp, \
             tc.tile_pool(name="nwork", bufs=4) as wp, \
             tc.tile_pool(name="small", bufs=4) as sp, \
             tc.tile_pool(name="flush", bufs=4) as fp, \
             tc.tile_pool(name="dram", bufs=1, space="DRAM") as dp, \
             tc.tile_pool(name="ppe", bufs=3, space="PSUM") as pp_e, \
             tc.tile_pool(name="ppt", bufs=2, space="PSUM") as pp_t, \
             tc.tile_pool(name="ppmm", bufs=2, space="PSUM") as pp_mm:

            numer = dp.tile([c.LOCN, PAY], F32)
            eps_t = cp.tile([P, 1], F32, tag="lneps")
            nc.vector.memset(eps_t[:], 1e-5)

            consts = {
                "iota": _load_const(nc, cp, iota_d[:, :], (P, P), I32),
                "ident": _load_const(nc, cp, ident_d[:, :], (P, P)),
                "Wo": [_load_const(nc, cp, Wo_d[t], (P, HID)) for t in range(3)],
                "bo": [_load_const(nc, cp, bo_d[t], (P, HID)) for t in range(3)],
                "lng": [_load_const(nc, cp, lng_d[t], (P, HID)) for t in range(3)],
                "lnb": [_load_const(nc, cp, lnb_d[t], (P, HID)) for t in range(3)],
            }
            if not last:
                consts.update({
                    "Wkqv": [_load_const(nc, cp, Wkqv[t], (P, 3 * HID)) for t in range(3)],
                    "bkqv": [_load_const(nc, cp, bkqv[t], (P, 3 * HID)) for t in range(3)],
                    "BDaT": [_load_const(nc, cp, BDaT[e], (P, HID)) for e in range(4)],
                    "BDm": [_load_const(nc, cp, BDm[e], (P, HID)) for e in range(4)],
                })
            else:
                consts["whead"] = _load_const(nc, cp, wh_d[:, :], (P, 1))
                consts["bh"] = _load_const(nc, cp, bh_d[:, :], (P, 1))

            # ---------------- edge phase ----------------
            # streams are host-pre-gathered per edge (bf16); per window load
            # [P, T, HID] slices (partition-major contiguous), batch all DVE
            # ops across the window's T tiles, scatter via bf16 one-hot matmul.
            gtile = 0
            for w in range(c.NWIN):
                T = int(tws[w])
                g0 = gtile
                kgt = gp.tile([P, T, HID], BF16, tag="kgt")
                nc.sync.dma_start(kgt[:], kg_d[:, g0:g0 + T, :])
                vgt = gp.tile([P, T, HID], BF16, tag="vgt")
                nc.sync.dma_start(vgt[:], vg_d[:, g0:g0 + T, :])
                qgt = gp.tile([P, T, HID], BF16, tag="qgt")
                nc.sync.dma_start(qgt[:], qg_d[:, g0:g0 + T, :])
                dstl = idxp.tile([P, T], I32, tag="dstl")
                nc.sync.dma_start(dstl[:], ti_t[:, g0:g0 + T])

                psum_w = pp_e.tile([P, PAY], F32, tag="psw")
                prod = ewp.tile([P, T, HID], BF16, tag="prod")
                nc.vector.tensor_tensor(out=prod[:], in0=kgt[:], in1=qgt[:],
                                        op=OP.mult)
                alpha = ewp.tile([P, T, H], F32, tag="alpha")
                nc.vector.tensor_reduce(
                    out=alpha[:],
                    in_=prod[:].rearrange("p t (h d) -> p t h d", h=H),
                    axis=mybir.AxisListType.X, op=OP.add)
                payload = ewp.tile([P, T, PAY], BF16, tag="payload")
                ex = payload[:, :, HID:HID + H]
                nc.scalar.activation(out=ex, in_=alpha[:], func=AF.Exp)
                nc.vector.tensor_tensor(
                    out=payload[:, :, :HID].rearrange("p t (h d) -> p t h d", h=H),
                    in0=vgt[:].rearrange("p t (h d) -> p t h d", h=H),
                    in1=ex[:, :, :, None].to_broadcast([P, T, H, D]),
                    op=OP.mult)
                onehot = ewp.tile([P, T, P], BF16, tag="onehot")
                nc.vector.tensor_tensor(
                    out=onehot[:],
                    in0=dstl[:, :, None].to_broadcast([P, T, P]),
                    in1=consts["iota"][:, None, :].to_broadcast([P, T, P]),
                    op=OP.is_equal)
                for t in range(T):
                    nc.tensor.matmul(out=psum_w[:], lhsT=onehot[:, t, :],
                                     rhs=payload[:, t, :],
                                     start=(t == 0), stop=(t == T - 1))
                fl = fp.tile([P, PAY], F32, tag="fl")
                nc.vector.tensor_copy(out=fl[:], in_=psum_w[:])
                nc.sync.dma_start(numer[w * P:(w + 1) * P, :], fl[:])
                gtile += T

            # ---------------- node phase ----------------
            pools = {"cp": cp, "wp": wp, "pp_t": pp_t, "pp_mm": pp_mm}
            outs = None if last else {"kd": kd_o, "qp": qp_o, "ve": ve_o}
            tiles = _type_tiles(c)
            NTL = len(tiles)
            # pass 1: all tiles up through the skip-add + LN stats; gelu is the
            # only table-based ACT function here so the scalar engine loads
            # the gelu table once instead of thrashing gelu<->sqrt per tile.
            o1_all = cp.tile([P, NTL, HID], F16, tag="o1_all")
            mv_all = cp.tile([P, NTL, 2], F32, tag="mv_all")
            for i, (t, i_t, r0) in enumerate(tiles):
                nm = wp.tile([P, PAY], F32, tag="nm")
                nc.sync.dma_start(nm[:], numer[r0:r0 + P, :])
                den = sp.tile([P, H], F32, tag="den")
                nc.vector.tensor_scalar_add(den[:], nm[:, HID:HID + H], 1e-16)
                rec = sp.tile([P, H], F32, tag="rec")
                nc.vector.reciprocal(rec[:], den[:])
                agg = wp.tile([P, HID], F32, tag="agg")
                nc.vector.tensor_tensor(
                    out=agg[:].rearrange("p (h d) -> p h d", h=H),
                    in0=nm[:, :HID].rearrange("p (h d) -> p h d", h=H),
                    in1=rec[:, :, None].to_broadcast([P, H, D]),
                    op=OP.mult)
                glu = wp.tile([P, HID], F32, tag="glu")
                if os.environ.get("HGT_BACKEND", "hw") == "sim":
                    # CoreSim has no Gelu LUT: tanh approximation (dev only)
                    t1 = wp.tile([P, HID], F32, tag="gelu_t1")
                    nc.vector.tensor_tensor(out=t1[:], in0=agg[:], in1=agg[:], op=OP.mult)
                    nc.vector.tensor_tensor(out=t1[:], in0=t1[:], in1=agg[:], op=OP.mult)
                    nc.vector.tensor_scalar(out=t1[:], in0=t1[:], scalar1=0.044715,
                                            scalar2=None, op0=OP.mult)
                    nc.vector.tensor_tensor(out=t1[:], in0=t1[:], in1=agg[:], op=OP.add)
                    nc.scalar.activation(out=t1[:], in_=t1[:], func=AF.Tanh,
                                         scale=0.7978845608028654)
                    nc.vector.tensor_scalar(out=t1[:], in0=t1[:], scalar1=0.5,
                                            scalar2=0.5, op0=OP.mult, op1=OP.add)
                    nc.vector.tensor_tensor(out=glu[:], in0=t1[:], in1=agg[:], op=OP.mult)
                else:
                    nc.scalar.activation(out=glu[:], in_=agg[:], func=AF.Gelu)
                gluT_ps = pp_t.tile([P, P], F32, tag="tp_ps")
                nc.tensor.transpose(out=gluT_ps[:], in_=glu[:], identity=consts["ident"][:])
                gluT = wp.tile([P, P], F32, tag="gluT")
                nc.scalar.copy(out=gluT[:], in_=gluT_ps[:])
                o_ps = pp_mm.tile([P, HID], F32, tag="mm_ps")
                nc.tensor.matmul(out=o_ps[:], lhsT=gluT[:], rhs=consts["Wo"][t][:],
                                 start=True, stop=True)
                # o3 = g*o + (g*b_o) + (1-g)*xs  (bo const already has g*b_o)
                xs_t = wp.tile([P, HID], F32, tag="xs_ld")
                nc.sync.dma_start(xs_t[:], xs_in[r0:r0 + P, :])
                o1 = o1_all[:, i, :]
                nc.vector.tensor_scalar_mul(o1, o_ps[:], float(g_list[t]))
                nc.vector.tensor_tensor(out=o1, in0=o1, in1=consts["bo"][t][:], op=OP.add)
                xs_s = wp.tile([P, HID], F32, tag="xs_s")
                nc.vector.tensor_scalar_mul(xs_s[:], xs_t[:], float(1.0 - g_list[t]))
                nc.vector.tensor_tensor(out=o1, in0=o1, in1=xs_s[:], op=OP.add)
                stats = sp.tile([P, nc.vector.BN_STATS_DIM], F32, tag="stats")
                nc.vector.bn_stats(out=stats[:], in_=o1)
                nc.vector.bn_aggr(out=mv_all[:, i, :], in_=stats[:])
            # one batched sqrt for all tiles' variances (single table load)
            rstd_all = cp.tile([P, NTL], F32, tag="rstd_all")
            nc.scalar.activation(out=rstd_all[:], in_=mv_all[:, :, 1],
                                 func=AF.Sqrt,
                                 bias=eps_t[:, 0:1])
            nc.vector.reciprocal(rstd_all[:], rstd_all[:])
            # pass 2: normalize + relu + next-layer projections (relu/copy are
            # in every ACT table set, so no further table switches).
            for i, (t, i_t, r0) in enumerate(tiles):
                xh = wp.tile([P, HID], F32, tag="xh")
                nc.vector.tensor_scalar(
                    out=xh[:], in0=o1_all[:, i, :], scalar1=mv_all[:, i, 0:1],
                    scalar2=rstd_all[:, i:i + 1],
                    op0=OP.subtract, op1=OP.mult)
                nc.vector.tensor_tensor(out=xh[:], in0=xh[:], in1=consts["lng"][t][:], op=OP.mult)
                nc.vector.tensor_tensor(out=xh[:], in0=xh[:], in1=consts["lnb"][t][:], op=OP.add)
                xs_new = wp.tile([P, HID], F32, tag="xs_new")
                nc.scalar.activation(out=xs_new[:], in_=xh[:], func=AF.Relu)
                if not last:
                    nc.sync.dma_start(xs_o[r0:r0 + P, :], xs_new[:])
                    _kqv_chain(nc, pools, c, consts, t, i_t, r0, xs_new, outs)
                elif t == 0:
                    xnT_ps = pp_t.tile([P, P], F32, tag="tp_ps")
                    nc.tensor.transpose(out=xnT_ps[:], in_=xs_new[:], identity=consts["ident"][:])
                    xnT = wp.tile([P, P], F32, tag="xnT")
                    nc.scalar.copy(out=xnT[:], in_=xnT_ps[:])
                    d_ps = pp_mm.tile([P, 1], F32, tag="mm_ps")
                    nc.tensor.matmul(out=d_ps[:], lhsT=xnT[:], rhs=consts["whead"][:],
                                     start=True, stop=True)
                    dl = sp.tile([P, 1], F32, tag="dl")
                    nc.vector.tensor_tensor(out=dl[:], in0=d_ps[:], in1=consts["bh"][:], op=OP.add)
                    nc.sync.dma_start(dl_o[i_t * P:(i_t + 1) * P, :], dl[:])
    nc.compile()
    return nc


build_l23.g_list = None


# ---------------------------------------------------------------------------
# Runner
# ---------------------------------------------------------------------------

LAUNCH_TIMES_NS = []
TRACE_DIRS = []


def _run(nc, in_maps, cfg):
    backend = os.environ.get("HGT_BACKEND", "hw")
    if backend == "sim":
        from concourse.bass_interp import CoreSim
        results = []
        for m in in_maps:
            sim = CoreSim(nc, trace=False, require_finite=False, require_nnan=False)
            for k, v in m.items():
                sim.tensor(k)[:] = v
            sim.simulate(check_with_hw=False)
            out = {}
            for alloc in nc.m.functions[0].allocations:
                if isinstance(alloc, mybir.MemoryLocationSet) and alloc.kind == "ExternalOutput":
                    name = alloc.memorylocations[0].name
                    out[name] = sim.tensor(name).copy()
            results.append(out)
        return results
    else:
        from concourse.bass_utils import run_bass_kernel_spmd
        trace = os.environ.get("HGT_TRACE", "0") == "1"
        res = run_bass_kernel_spmd(nc, in_maps, core_ids=list(range(cfg.C)),
                                   trace=trace)
        if trace:
            LAUNCH_TIMES_NS.append(res.exec_time_ns)
            it = res.instructions_and_trace
            TRACE_DIRS.append(getattr(it, "trace_path", it))
        return res.results


# ---------------------------------------------------------------------------
# Main entry
# ---------------------------------------------------------------------------

def kernel(**inputs):
    cfg = Cfg()
    return _kernel_impl(cfg, inputs)


def _kernel_impl(cfg, inputs):
    c = cfg
    prm = prep_params(c, inputs)
    g = prep_graph(c, inputs)
    NT, tws = g["NT"], g["tws"]

    # ---- launch 1
    nc1 = build_l1(c)
    in_maps = []
    xa = np.asarray(inputs["x_a"], np.float32)
    xw = np.asarray(inputs["x_w"], np.float32)
    xo = np.asarray(inputs["x_o"], np.float32)

    def padx(x, n, npad):
        out = np.zeros((npad, P), np.float32)
        out[:n, :64] = x
        return out

    for cc in range(c.C):
        in_maps.append({
            "xa": padx(xa[cc * c.nac:(cc + 1) * c.nac], c.nac, c.nap),
            "xw": padx(xw[cc * c.nwc:(cc + 1) * c.nwc], c.nwc, c.nwp),
            "xo": padx(xo[cc * c.noc:(cc + 1) * c.noc], c.noc, c.nop),
            "Win": prm["Win"], "bin": prm["bin"],
            "Wkqv": prm["Wkqv0"], "bkqv": prm["bkqv0"],
            "BDaT": prm["BDaT0"], "BDm": prm["BDm0"],
            "ident": prm["ident"],
        })
    r1 = _run(nc1, in_maps, c)

    def assemble_tables(res):
        """Build global kd table + per-core q' tables + global ve table."""
        kd_tab = np.empty((c.KD_ROWS, HID), np.float32)
        kd_tab[-1] = 1.0
        ve_tab = np.empty((c.VE_ROWS, HID), np.float32)
        ve_tab[-1] = 0.0
        qp_tabs = []
        for cc in range(c.C):
            kd = res[cc]["kd"]
            ve = res[cc]["ve"]
            # kd local [a|w|o] -> global
            kd_tab[c.KOFF[0] + cc * c.nac:c.KOFF[0] + (cc + 1) * c.nac] = kd[:c.nac]
            kd_tab[c.KOFF[1] + cc * c.nwc:c.KOFF[1] + (cc + 1) * c.nwc] = \
                kd[c.base_local[1]:c.base_local[1] + c.nwc]
            kd_tab[c.KOFF[2] + cc * c.noc:c.KOFF[2] + (cc + 1) * c.noc] = \
                kd[c.base_local[2]:c.base_local[2] + c.noc]
            # ve local slots [a-et2, a-et3, w-et0, o-et1] -> global stacked
            ve_tab[c.VOFF[2] + cc * c.nac:c.VOFF[2] + (cc + 1) * c.nac] = \
                ve[c.QB[0]:c.QB[0] + c.nac]
            ve_tab[c.VOFF[3] + cc * c.nac:c.VOFF[3] + (cc + 1) * c.nac] = \
                ve[c.QB[1]:c.QB[1] + c.nac]
            ve_tab[c.VOFF[0] + cc * c.nwc:c.VOFF[0] + (cc + 1) * c.nwc] = \
                ve[c.QB[2]:c.QB[2] + c.nwc]
            ve_tab[c.VOFF[1] + cc * c.noc:c.VOFF[1] + (cc + 1) * c.noc] = \
                ve[c.QB[3]:c.QB[3] + c.noc]
            # pad-edge q rows are -8.0: with pad k rows = 1.0 the pad alpha is
            # 128 * -8 = -1024 (bf16-safe), exp -> 0.
            qp = np.vstack([res[cc]["qp"], np.full((1, HID), -8.0, np.float32)])
            qp_tabs.append(qp)
        return kd_tab, ve_tab, qp_tabs

    import ml_dtypes
    bf16 = ml_dtypes.bfloat16

    # ---- launches 2 and 3
    res = r1
    for l, last in ((1, False), (2, True)):
        kd_tab, ve_tab, qp_tabs = assemble_tables(res)
        lay = l - 1  # layer params index: launch2 -> layer 0, launch3 -> layer 1
        build_l23.g_list = prm[f"g{lay}"]
        nc = build_l23(c, NT, tws, last)
        in_maps = []
        for cc in range(c.C):
            ti = g["tile_idx"][cc]  # [NT, P, 4]
            kg_s = np.ascontiguousarray(
                kd_tab[ti[:, :, 0]].transpose(1, 0, 2)).astype(bf16)
            vg_s = np.ascontiguousarray(
                ve_tab[ti[:, :, 1]].transpose(1, 0, 2)).astype(bf16)
            qg_s = np.ascontiguousarray(
                qp_tabs[cc][ti[:, :, 2]].transpose(1, 0, 2)).astype(bf16)
            dstlT = np.ascontiguousarray(ti[:, :, 3].T)
            m = {
                "kg": kg_s, "vg": vg_s, "qg": qg_s,
                "xsin": res[cc]["xs"],
                "dstlT": dstlT,
                "iota": prm["iota"], "ident": prm["ident"],
                "Wo": prm[f"Wo{lay}"], "bo": prm[f"bo{lay}"],
                "lng": prm[f"lng{lay}"], "lnb": prm[f"lnb{lay}"],
                "gs": prm[f"g{lay}"],
            }
            if not last:
                m.update({"Wkqv": prm[f"Wkqv{lay + 1}"], "bkqv": prm[f"bkqv{lay + 1}"],
                          "BDaT": prm[f"BDaT{lay + 1}"], "BDm": prm[f"BDm{lay + 1}"]})
            else:
                m.update({"whead": prm["whead"], "bh": prm["bh"]})
            in_maps.append(m)
        res = _run(nc, in_maps, c)

    out = np.concatenate([res[cc]["delta"][:c.nac, 0] for cc in range(c.C)])
    return out.astype(np.float32)



# revision 19
# speedup vs baseline: 2.5767x; 1.8263x over previous
"""HGT regressor on 8 Trainium2 NeuronCores (Bass/Tile).

Strategy (graph/data parallel, hint-following):
  - Nodes of each type are partitioned contiguously across the 8 cores
    (a: 12500/core, w: 2500/core, o: 6250/core). Each core owns the edges
    whose *destination* lies in its node shard.
  - Per layer, each core computes K = kqv[:, :128] (raw) and the per-edge-type
    source-side V transform (m_rel folded at source) plus the destination-side
    Q transform (a_rel * p_rel * scale folded into Q) for its own nodes only.
  - The full K / V_et tables are exchanged between layer launches via the host
    (replicated to all cores), i.e. host-mediated all-gather. Q' stays local.
  - Edge phase per core: edges sorted by local destination row, grouped into
    128-node windows; per 128-edge tile: indirect-DMA gathers of K[src],
    V_et[src], Q'_et[dst]; alpha = sum_h(K*Q'); ex = exp(alpha); payload
    [ex*V | ex] is scatter-added into a PSUM window accumulator via a
    one-hot matmul; windows flush densely to a numer/den table in DRAM.
  - Node phase per core: agg = numer/den, gelu, W_o, gated skip, LayerNorm,
    relu, then next-layer projections (or the scalar head in the last layer).
  - Softmax needs no running max: alpha = q'k with these parameter scales is
    O(1); exp cannot overflow, and softmax is shift-invariant anyway.
"""
import os
import sys

sys.path.insert(0, "/opt/trn_rl_repo")

import numpy as np

import concourse.bass as bass
import concourse.mybir as mybir
import concourse.tile as tile
from concourse import bacc

P = 128
H, D, HID = 4, 32, 128
PAY = HID + H  # 132
F32 = mybir.dt.float32
F16 = mybir.dt.float16
BF16 = mybir.dt.bfloat16
I32 = mybir.dt.int32
AF = mybir.ActivationFunctionType
OP = mybir.AluOpType


def _ceil(a, b):
    return (a + b - 1) * b // b if False else -(-a // b) * b


def cdiv(a, b):
    return -(-a // b)


class Cfg:
    """All sizes derived from problem scale; supports mini-scale testing."""

    def __init__(self, NA=100000, NWK=20000, NO=50000, E=150000, C=8):
        self.NA, self.NWK, self.NO, self.E, self.C = NA, NWK, NO, E, C
        assert NA % C == 0 and NWK % C == 0 and NO % C == 0
        self.nac, self.nwc, self.noc = NA // C, NWK // C, NO // C
        self.nap, self.nwp, self.nop = (
            cdiv(self.nac, P) * P,
            cdiv(self.nwc, P) * P,
            cdiv(self.noc, P) * P,
        )
        # local node-row layout (numer/xs/kd rows): [a | w | o], each padded
        self.base_local = (0, self.nap, self.nap + self.nwp)
        self.LOCN = self.nap + self.nwp + self.nop
        self.NWIN = self.LOCN // P
        # per-type tile counts
        self.ntile_a, self.ntile_w, self.ntile_o = (
            self.nap // P,
            self.nwp // P,
            self.nop // P,
        )
        # q' local layout: slots [a-et0, a-et1, w-et2, o-et3]
        self.QB = (0, self.nap, 2 * self.nap, 2 * self.nap + self.nwp)
        self.QTOT = 2 * self.nap + self.nwp + self.nop
        # ve local layout (same bases): slots [a-et2, a-et3, w-et0, o-et1]
        # global kd table layout: [a 0..NA | w | o] + trash
        self.KOFF = (0, NA, NA + NWK)
        self.KD_ROWS = NA + NWK + NO + 1
        # global stacked ve table: [et0 w | et1 o | et2 a | et3 a] + trash
        self.VOFF = (0, NWK, NWK + NO, NWK + NO + NA)
        self.VE_ROWS = NWK + NO + 2 * NA + 1


# edge types: (src_type, dst_type)
ETYPES = ((1, 0), (2, 0), (0, 1), (0, 2))


# ---------------------------------------------------------------------------
# Host-side preprocessing
# ---------------------------------------------------------------------------

def prep_graph(cfg, inputs):
    """Compute per-core edge tile indices. Shared across both layers.

    Returns dict with:
      NT: static tile count (same all cores)
      tws: [NWIN] tiles per window (static across cores)
      tile_idx: [C][NT, P, 4] int32  (kidx, vidx, qidx, dst_local)
    """
    c = cfg
    edges = []  # per et: (src, dst)
    for name_s, name_d in (("src_wa", "dst_wa"), ("src_oa", "dst_oa"),
                           ("src_aw", "dst_aw"), ("src_ao", "dst_ao")):
        edges.append((np.asarray(inputs[name_s]), np.asarray(inputs[name_d])))

    shard_n = (c.nac, c.nwc, c.noc)
    # concat all ets with global indices
    K_TRASH = c.KD_ROWS - 1
    V_TRASH = c.VE_ROWS - 1
    Q_TRASH = c.QTOT  # row appended by host to the q' table

    all_core = []
    koff_by_et = (c.KOFF[1], c.KOFF[2], c.KOFF[0], c.KOFF[0])  # src type offset in kd
    for et, (st, dt) in enumerate(ETYPES):
        src, dst = edges[et]
        kidx = koff_by_et[et] + src
        vidx = c.VOFF[et] + src
        core = dst // shard_n[dt]
        dloc = dst - core * shard_n[dt]  # dst index within its type shard
        # local numer row / q' row
        tb = (c.base_local[0], c.base_local[1], c.base_local[2])[dt]
        row = tb + dloc
        qslot = {0: 0, 1: 1, 2: 2, 3: 3}[et]
        qidx = c.QB[qslot] + dloc
        all_core.append((core, row, kidx, vidx, qidx))

    core_cat = np.concatenate([a[0] for a in all_core])
    row_cat = np.concatenate([a[1] for a in all_core])
    k_cat = np.concatenate([a[2] for a in all_core])
    v_cat = np.concatenate([a[3] for a in all_core])
    q_cat = np.concatenate([a[4] for a in all_core])

    # per-core, per-window edge counts -> static tile structure
    win_cat = row_cat // P
    counts = np.zeros((c.C, c.NWIN), np.int64)
    for cc in range(c.C):
        m = core_cat == cc
        counts[cc] = np.bincount(win_cat[m], minlength=c.NWIN)
    tws = np.maximum(cdiv(counts.max(axis=0), P), 1)  # >=1 tile per window
    NT = int(tws.sum())
    tile_base = np.zeros(c.NWIN, np.int64)
    tile_base[1:] = np.cumsum(tws)[:-1]

    tile_idx = np.zeros((c.C, NT, P, 4), np.int32)
    # fill pads with trash rows -> ex = 0 contributions
    tile_idx[:, :, :, 0] = K_TRASH
    tile_idx[:, :, :, 1] = V_TRASH
    tile_idx[:, :, :, 2] = Q_TRASH
    tile_idx[:, :, :, 3] = 0
    for cc in range(c.C):
        m = core_cat == cc
        rows = row_cat[m]
        order = np.argsort(rows, kind="stable")
        rows = rows[order]
        ks, vs, qs = k_cat[m][order], v_cat[m][order], q_cat[m][order]
        wins = rows // P
        dstl = rows % P
        # position within window
        wstart = np.searchsorted(wins, np.arange(c.NWIN), side="left")
        pos = np.arange(rows.size) - wstart[wins]
        slot_t = pos // P   # tile within window
        slot_p = pos % P    # partition
        gt = tile_base[wins] + slot_t  # global tile id
        tile_idx[cc, gt, slot_p, 0] = ks
        tile_idx[cc, gt, slot_p, 1] = vs
        tile_idx[cc, gt, slot_p, 2] = qs
        tile_idx[cc, gt, slot_p, 3] = dstl
    return {"NT": NT, "tws": tws.astype(np.int64), "tile_idx": tile_idx,
            "tile_base": tile_base}


def blockdiag(M):
    out = np.zeros((HID, HID), np.float32)
    for h in range(H):
        out[h * D:(h + 1) * D, h * D:(h + 1) * D] = M[h]
    return out


def prep_params(cfg, inputs):
    """Fold and lay out all parameters (host, tiny)."""
    scale = np.float32(1.0 / np.sqrt(D))
    a_rel = np.asarray(inputs["a_rel"])
    m_rel = np.asarray(inputs["m_rel"])
    p_rel = np.asarray(inputs["p_rel"])
    prm = {}
    rep = lambda v, w: np.broadcast_to(np.asarray(v, np.float32)[None, :], (P, w)).copy()
    for l in range(2):
        BDaT, BDm = [], []
        for et in range(4):
            a_eff = a_rel[l, et] * (p_rel[l, et] * scale)[:, None, None]
            BDaT.append(blockdiag(a_eff).T.copy())
            BDm.append(blockdiag(m_rel[l, et]))
        prm[f"BDaT{l}"] = np.stack(BDaT)  # [4,128,128]
        prm[f"BDm{l}"] = np.stack(BDm)
        prm[f"Wkqv{l}"] = np.asarray(inputs["W_kqv"])[l]       # [3,128,384]
        prm[f"bkqv{l}"] = np.stack([rep(np.asarray(inputs["b_kqv"])[l, t], 3 * HID) for t in range(3)])
        # folded projection weights: per type, slots [kd | q'_ets | v'_ets]
        # q' = xs @ (Wq @ BDaT_et), v' = xs @ (Wv @ BDm_et); biases likewise.
        import ml_dtypes
        q_ets = ((0, 1), (2,), (3,))
        v_ets = ((2, 3), (0,), (1,))
        Wcat = np.zeros((3, HID, 5 * HID), np.float32)
        bcat = np.zeros((3, P, 5 * HID), np.float32)
        for t in range(3):
            Wk = np.asarray(inputs["W_kqv"])[l, t]
            bk = np.asarray(inputs["b_kqv"])[l, t]
            cols = [Wk[:, :HID]]
            bs = [bk[:HID]]
            for et in q_ets[t]:
                cols.append(Wk[:, HID:2 * HID] @ prm[f"BDaT{l}"][et])
                bs.append(bk[HID:2 * HID] @ prm[f"BDaT{l}"][et])
            for et in v_ets[t]:
                cols.append(Wk[:, 2 * HID:] @ prm[f"BDm{l}"][et])
                bs.append(bk[2 * HID:] @ prm[f"BDm{l}"][et])
            cat = np.concatenate(cols, axis=1)
            Wcat[t, :, :cat.shape[1]] = cat
            bcat[t, :, :cat.shape[1]] = np.concatenate(bs)[None, :]
        prm[f"Wcat{l}"] = Wcat.astype(ml_dtypes.bfloat16)
        prm[f"bcat{l}"] = bcat
        prm[f"Wo{l}"] = np.asarray(inputs["W_o"])[l]           # [3,128,128]
        g = 1.0 / (1.0 + np.exp(-np.asarray(inputs["skip_p"], np.float64)))  # [2,3]
        prm[f"g{l}"] = g[l].astype(np.float32)
        prm[f"bo{l}"] = np.stack([rep(np.asarray(inputs["b_o"])[l, t] * g[l, t], HID) for t in range(3)])
        prm[f"lng{l}"] = np.stack([rep(np.asarray(inputs["ln_g"])[l, t], HID) for t in range(3)])
        prm[f"lnb{l}"] = np.stack([rep(np.asarray(inputs["ln_b"])[l, t], HID) for t in range(3)])
    # input proj, padded to 128 contraction
    W_in = np.asarray(inputs["W_in"])  # [3,64,128]
    Wp = np.zeros((3, 128, HID), np.float32)
    Wp[:, :64, :] = W_in
    prm["Win"] = Wp
    prm["bin"] = np.stack([rep(np.asarray(inputs["b_in"])[t], HID) for t in range(3)])
    prm["whead"] = np.asarray(inputs["w_head"], np.float32)  # [128,1]
    prm["bh"] = np.full((P, 1), float(np.asarray(inputs["b_head"])[0] + np.asarray(inputs["base"])[0]), np.float32)
    prm["iota"] = np.broadcast_to(np.arange(128, dtype=np.int32)[None, :], (P, 128)).copy()
    prm["ident"] = np.eye(128, dtype=np.float32)
    return prm


# ---------------------------------------------------------------------------
# Builders
# ---------------------------------------------------------------------------

_CONST_N = [0]


def _load_const(nc, cp, ap, shape, dtype=F32):
    _CONST_N[0] += 1
    t = cp.tile(list(shape), dtype, tag=f"cst{_CONST_N[0]}")
    nc.sync.dma_start(t[:], ap)
    return t


def _type_tiles(cfg):
    """Yield (t, i_t, r0) for all node tiles: type, tile-in-type, local row base."""
    out = []
    for t, (ntile, b) in enumerate(
        zip((cfg.ntile_a, cfg.ntile_w, cfg.ntile_o), cfg.base_local)
    ):
        for i in range(ntile):
            out.append((t, i, b + i * P))
    return out


def _kqv_width(t):
    """Staging width in slots for type t: [xs | kd | q'_ets | v'_ets]."""
    return 6 if t == 0 else 4


def _kqv_tile(nc, pools, cfg, consts, t, xs_tile, stg, gi):
    """Next-layer projections for one tile via folded weights.

    Writes kd/q'/v' (slots 1..) of stg[:, gi, :]; slot 0 (xs) is written by
    the caller. One transpose + 1-2 bf16 matmuls + 1-2 bias-adds.
    """
    wp, pp_t, pp_mm = pools["wp"], pools["pp_t"], pools["pp_mm"]
    xsT_ps = pp_t.tile([P, P], F32, tag="tp_ps")
    nc.tensor.transpose(out=xsT_ps[:], in_=xs_tile[:], identity=consts["ident"][:])
    xsT = wp.tile([P, P], BF16, tag="xsT")
    nc.scalar.copy(out=xsT[:], in_=xsT_ps[:])
    W = consts["Wcat"][t]
    B = consts["bcat"][t]
    nw = (_kqv_width(t) - 1) * HID  # matmul output width (kd + q + v slots)
    n1 = min(nw, 4 * HID)
    ps1 = pp_mm.tile([P, 4 * HID], F32, tag="mmk_ps")
    nc.tensor.matmul(out=ps1[:, :n1], lhsT=xsT[:], rhs=W[:, :n1],
                     start=True, stop=True)
    nc.vector.tensor_tensor(out=stg[:, gi, HID:HID + n1], in0=ps1[:, :n1],
                            in1=B[:, :n1], op=OP.add)
    if nw > n1:
        ps2 = pp_mm.tile([P, HID], F32, tag="mm_ps")
        nc.tensor.matmul(out=ps2[:], lhsT=xsT[:], rhs=W[:, n1:nw],
                         start=True, stop=True)
        nc.vector.tensor_tensor(out=stg[:, gi, HID + n1:HID + nw], in0=ps2[:],
                                in1=B[:, n1:nw], op=OP.add)


def _kqv_flush(nc, cfg, outs, t, i0, G, stg, r0base, write_xs=True):
    """DMA a group of G tiles' staged [xs|kd|q|v] slots to their tables."""
    kd_o, qp_o, ve_o = outs["kd"], outs["qp"], outs["ve"]
    xs_o = outs.get("xs")
    rt0 = i0 * P

    def wr(dst, lo):
        nc.gpsimd.dma_start(dst.rearrange("(g p) f -> p g f", p=P),
                            stg[:, :, lo * HID:(lo + 1) * HID])

    if write_xs and xs_o is not None:
        wr(xs_o[r0base:r0base + G * P, :], 0)
    wr(kd_o[r0base:r0base + G * P, :], 1)
    q_slots = ((0, 1), (2,), (3,))[t]
    v_slots = ((0, 1), (2,), (3,))[t]
    for j, sl in enumerate(q_slots):
        wr(qp_o[cfg.QB[sl] + rt0:cfg.QB[sl] + rt0 + G * P, :], 2 + j)
    for j, sl in enumerate(v_slots):
        wr(ve_o[cfg.QB[sl] + rt0:cfg.QB[sl] + rt0 + G * P, :],
           2 + len(q_slots) + j)


def _type_groups(cfg, tiles, G=4):
    """Split the ordered tile list into same-type groups of <= G."""
    groups = []
    cur = []
    for tt in tiles:
        if cur and (tt[0] != cur[0][0] or len(cur) == G):
            groups.append(cur)
            cur = []
        cur.append(tt)
    if cur:
        groups.append(cur)
    return groups


def build_l1(cfg):
    """Launch 1: input proj + relu -> xs1; kqv chain -> kd/q'/ve tables."""
    nc = bacc.Bacc("TRN2", target_bir_lowering=False, debug=False,
                   num_devices=cfg.C)
    c = cfg
    xa = nc.dram_tensor("xa", [c.nap, P], F32, kind="ExternalInput").ap()
    xw = nc.dram_tensor("xw", [c.nwp, P], F32, kind="ExternalInput").ap()
    xo = nc.dram_tensor("xo", [c.nop, P], F32, kind="ExternalInput").ap()
    Win = nc.dram_tensor("Win", [3, P, HID], F32, kind="ExternalInput").ap()
    binp = nc.dram_tensor("bin", [3, P, HID], F32, kind="ExternalInput").ap()
    Wcat_d = nc.dram_tensor("Wcat", [3, P, 5 * HID], BF16, kind="ExternalInput").ap()
    bcat_d = nc.dram_tensor("bcat", [3, P, 5 * HID], F32, kind="ExternalInput").ap()
    ident_d = nc.dram_tensor("ident", [P, P], F32, kind="ExternalInput").ap()

    xs_o = nc.dram_tensor("xs", [c.LOCN, P], F32, kind="ExternalOutput").ap()
    kd_o = nc.dram_tensor("kd", [c.LOCN, P], F32, kind="ExternalOutput").ap()
    qp_o = nc.dram_tensor("qp", [c.QTOT, P], F32, kind="ExternalOutput").ap()
    ve_o = nc.dram_tensor("ve", [c.QTOT, P], F32, kind="ExternalOutput").ap()

    xin = (xa, xw, xo)
    with tile.TileContext(nc) as tc:
        with tc.tile_pool(name="consts", bufs=1) as cp, \
             tc.tile_pool(name="work", bufs=4) as wp, \
             tc.tile_pool(name="stgp", bufs=2) as sgp, \
             tc.tile_pool(name="ppt", bufs=2, space="PSUM") as pp_t, \
             tc.tile_pool(name="ppmm", bufs=2, space="PSUM") as pp_mm:
            consts = {
                "ident": _load_const(nc, cp, ident_d[:, :], (P, P)),
                "Win": [_load_const(nc, cp, Win[t], (P, HID)) for t in range(3)],
                "bin": [_load_const(nc, cp, binp[t], (P, HID)) for t in range(3)],
                "Wcat": [_load_const(nc, cp, Wcat_d[t], (P, 5 * HID), BF16) for t in range(3)],
                "bcat": [_load_const(nc, cp, bcat_d[t], (P, 5 * HID)) for t in range(3)],
            }
            pools = {"cp": cp, "wp": wp, "pp_t": pp_t, "pp_mm": pp_mm}
            outs = {"kd": kd_o, "qp": qp_o, "ve": ve_o, "xs": xs_o}
            for grp in _type_groups(c, _type_tiles(c)):
                t = grp[0][0]
                G = len(grp)
                stg = sgp.tile([P, G, _kqv_width(t) * HID], F32, tag="stg")
                for gi, (_, i_t, r0) in enumerate(grp):
                    x_t = wp.tile([P, P], F32, tag="x_in")
                    nc.sync.dma_start(x_t[:], xin[t][i_t * P:(i_t + 1) * P, :])
                    xT_ps = pp_t.tile([P, P], F32, tag="tp_ps")
                    nc.tensor.transpose(out=xT_ps[:], in_=x_t[:], identity=consts["ident"][:])
                    xT = wp.tile([P, P], F32, tag="xT")
                    nc.scalar.copy(out=xT[:], in_=xT_ps[:])
                    pj_ps = pp_mm.tile([P, HID], F32, tag="mm_ps")
                    nc.tensor.matmul(out=pj_ps[:], lhsT=xT[:], rhs=consts["Win"][t][:],
                                     start=True, stop=True)
                    pj = wp.tile([P, HID], F32, tag="pj")
                    nc.vector.tensor_tensor(out=pj[:], in0=pj_ps[:],
                                            in1=consts["bin"][t][:], op=OP.add)
                    nc.scalar.activation(out=stg[:, gi, :HID], in_=pj[:], func=AF.Relu)
                    _kqv_tile(nc, pools, c, consts, t, stg[:, gi, :HID], stg, gi)
                _kqv_flush(nc, c, outs, t, grp[0][1], G, stg, grp[0][2])
    nc.compile()
    return nc


def build_l23(cfg, NT, tws, last):
    """Launches 2/3: edge phase + node phase (+ head if last)."""
    nc = bacc.Bacc("TRN2", target_bir_lowering=False, debug=False,
                   num_devices=cfg.C)
    c = cfg
    kg_d = nc.dram_tensor("kg", [P, NT, HID], BF16, kind="ExternalInput").ap()
    vg_d = nc.dram_tensor("vg", [P, NT, HID], BF16, kind="ExternalInput").ap()
    qg_d = nc.dram_tensor("qg", [P, NT, HID], BF16, kind="ExternalInput").ap()
    NWIN_E = c.ntile_a if last else c.NWIN   # only a-dst windows feed the head
    LOCN_E = c.nap if last else c.LOCN
    xs_in = nc.dram_tensor("xsin", [LOCN_E, P], F32, kind="ExternalInput").ap()
    ti_t = nc.dram_tensor("dstlT", [P, NT], I32, kind="ExternalInput").ap()
    iota_d = nc.dram_tensor("iota", [P, P], I32, kind="ExternalInput").ap()
    ident_d = nc.dram_tensor("ident", [P, P], F32, kind="ExternalInput").ap()
    Wo_d = nc.dram_tensor("Wo", [3, P, HID], F32, kind="ExternalInput").ap()
    bo_d = nc.dram_tensor("bo", [3, P, HID], F32, kind="ExternalInput").ap()
    lng_d = nc.dram_tensor("lng", [3, P, HID], F32, kind="ExternalInput").ap()
    lnb_d = nc.dram_tensor("lnb", [3, P, HID], F32, kind="ExternalInput").ap()
    gs_d = nc.dram_tensor("gs", [3], F32, kind="ExternalInput").ap()  # unused on-device; values baked via bo/g mul
    if not last:
        Wcat_d = nc.dram_tensor("Wcat", [3, P, 5 * HID], BF16, kind="ExternalInput").ap()
        bcat_d = nc.dram_tensor("bcat", [3, P, 5 * HID], F32, kind="ExternalInput").ap()
    else:
        wh_d = nc.dram_tensor("whead", [P, 1], F32, kind="ExternalInput").ap()
        bh_d = nc.dram_tensor("bh", [P, 1], F32, kind="ExternalInput").ap()

    if not last:
        xs_o = nc.dram_tensor("xs", [c.LOCN, P], F32, kind="ExternalOutput").ap()
        kd_o = nc.dram_tensor("kd", [c.LOCN, P], F32, kind="ExternalOutput").ap()
        qp_o = nc.dram_tensor("qp", [c.QTOT, P], F32, kind="ExternalOutput").ap()
        ve_o = nc.dram_tensor("ve", [c.QTOT, P], F32, kind="ExternalOutput").ap()
    else:
        dl_o = nc.dram_tensor("delta", [c.nap, 1], F32, kind="ExternalOutput").ap()

    # gains folded on host: bo tile already contains g*b_o. g itself baked as consts below.
    g_vals = None  # set in kernel() via attribute hack? no: pass via build arg
    g_list = build_l23.g_list  # [3] floats for this layer

    with tile.TileContext(nc) as tc:
        with tc.tile_pool(name="consts", bufs=1) as cp, \
             tc.tile_pool(name="idx", bufs=2) as idxp, \
             tc.tile_pool(name="gat", bufs=2) as gp, \
             tc.tile_pool(name="ework", bufs=2) as ewp, \
             tc.tile_pool(name="nwork", bufs=3) as wp, \
             tc.tile_pool(name="stgp", bufs=2) as sgp, \
             tc.tile_pool(name="small", bufs=4) as sp, \
             tc.tile_pool(name="flush", bufs=2) as fp, \
             tc.tile_pool(name="dram", bufs=1, space="DRAM") as dp, \
             tc.tile_pool(name="ppe", bufs=2, space="PSUM") as pp_e, \
             tc.tile_pool(name="ppt", bufs=2, space="PSUM") as pp_t, \
             tc.tile_pool(name="ppmm", bufs=2, space="PSUM") as pp_mm:

            numer = dp.tile([LOCN_E, PAY], F32)
            eps_t = cp.tile([P, 1], F32, tag="lneps")
            nc.vector.memset(eps_t[:], 1e-5)

            consts = {
                "iota": _load_const(nc, cp, iota_d[:, :], (P, P), I32),
                "ident": _load_const(nc, cp, ident_d[:, :], (P, P)),
                "Wo": [_load_const(nc, cp, Wo_d[t], (P, HID)) for t in range(3)],
                "bo": [_load_const(nc, cp, bo_d[t], (P, HID)) for t in range(3)],
                "lng": [_load_const(nc, cp, lng_d[t], (P, HID)) for t in range(3)],
                "lnb": [_load_const(nc, cp, lnb_d[t], (P, HID)) for t in range(3)],
            }
            if not last:
                consts.update({
                    "Wcat": [_load_const(nc, cp, Wcat_d[t], (P, 5 * HID), BF16) for t in range(3)],
                    "bcat": [_load_const(nc, cp, bcat_d[t], (P, 5 * HID)) for t in range(3)],
                })
            else:
                consts["whead"] = _load_const(nc, cp, wh_d[:, :], (P, 1))
                consts["bh"] = _load_const(nc, cp, bh_d[:, :], (P, 1))

            # ---------------- edge phase ----------------
            # streams are host-pre-gathered per edge (bf16). Windows are
            # grouped greedily into supers (<= SUPER_T tiles); per super one
            # DMA per stream + one batched op per DVE stage; scatter stays a
            # per-tile bf16 one-hot matmul into the window's PSUM accumulator.
            SUPER_T = 20
            supers = []  # (g0, [T_w...], w0)
            gtile = 0
            w = 0
            while w < NWIN_E:
                g0 = gtile
                ts = []
                w0 = w
                while w < NWIN_E and len(ts) < 4 and \
                        sum(ts) + int(tws[w]) <= SUPER_T:
                    ts.append(int(tws[w]))
                    gtile += int(tws[w])
                    w += 1
                supers.append((g0, ts, w0))
            for g0, ts, w0 in supers:
                TS = sum(ts)
                kgt = gp.tile([P, TS, HID], BF16, tag="kgt")
                nc.sync.dma_start(kgt[:], kg_d[:, g0:g0 + TS, :])
                vgt = gp.tile([P, TS, HID], BF16, tag="vgt")
                nc.sync.dma_start(vgt[:], vg_d[:, g0:g0 + TS, :])
                qgt = gp.tile([P, TS, HID], BF16, tag="qgt")
                nc.scalar.dma_start(qgt[:], qg_d[:, g0:g0 + TS, :])
                dstl = idxp.tile([P, TS], I32, tag="dstl")
                nc.scalar.dma_start(dstl[:], ti_t[:, g0:g0 + TS])

                prod = ewp.tile([P, TS, HID], BF16, tag="prod")
                nc.vector.tensor_tensor(out=prod[:], in0=kgt[:], in1=qgt[:],
                                        op=OP.mult)
                alpha = ewp.tile([P, TS, H], F32, tag="alpha")
                nc.vector.tensor_reduce(
                    out=alpha[:],
                    in_=prod[:].rearrange("p t (h d) -> p t h d", h=H),
                    axis=mybir.AxisListType.X, op=OP.add)
                payload = ewp.tile([P, TS, PAY], BF16, tag="payload")
                ex = payload[:, :, HID:HID + H]
                nc.scalar.activation(out=ex, in_=alpha[:], func=AF.Exp)
                nc.vector.tensor_tensor(
                    out=payload[:, :, :HID].rearrange("p t (h d) -> p t h d", h=H),
                    in0=vgt[:].rearrange("p t (h d) -> p t h d", h=H),
                    in1=ex[:, :, :, None].to_broadcast([P, TS, H, D]),
                    op=OP.mult)
                onehot = ewp.tile([P, TS, P], BF16, tag="onehot")
                nc.vector.tensor_tensor(
                    out=onehot[:],
                    in0=dstl[:, :, None].to_broadcast([P, TS, P]),
                    in1=consts["iota"][:, None, :].to_broadcast([P, TS, P]),
                    op=OP.is_equal)
                fl = fp.tile([P, len(ts), PAY], F32, tag="fl")
                toff = 0
                for wi, T in enumerate(ts):
                    psum_w = pp_e.tile([P, PAY], F32, tag="psw")
                    for t in range(toff, toff + T):
                        nc.tensor.matmul(out=psum_w[:], lhsT=onehot[:, t, :],
                                         rhs=payload[:, t, :],
                                         start=(t == toff),
                                         stop=(t == toff + T - 1))
                    nc.vector.tensor_copy(out=fl[:, wi, :], in_=psum_w[:])
                    toff += T
                nc.sync.dma_start(
                    numer[w0 * P:(w0 + len(ts)) * P, :].rearrange(
                        "(t p) f -> p t f", p=P),
                    fl[:])

            # ---------------- node phase ----------------
            pools = {"cp": cp, "wp": wp, "pp_t": pp_t, "pp_mm": pp_mm}
            outs = None if last else {"kd": kd_o, "qp": qp_o, "ve": ve_o, "xs": xs_o}
            tiles = [x for x in _type_tiles(c) if (not last) or x[0] == 0]
            NTL = len(tiles)
            # pass 1: all tiles up through the skip-add + LN stats; gelu is the
            # only table-based ACT function here so the scalar engine loads
            # the gelu table once instead of thrashing gelu<->sqrt per tile.
            o1_all = cp.tile([P, NTL, HID], F16, tag="o1_all")
            mv_all = cp.tile([P, NTL, 2], F32, tag="mv_all")
            for i, (t, i_t, r0) in enumerate(tiles):
                nm = wp.tile([P, PAY], F32, tag="nm")
                nc.sync.dma_start(nm[:], numer[r0:r0 + P, :])
                den = sp.tile([P, H], F32, tag="den")
                nc.vector.tensor_scalar_add(den[:], nm[:, HID:HID + H], 1e-16)
                rec = sp.tile([P, H], F32, tag="rec")
                nc.vector.reciprocal(rec[:], den[:])
                agg = wp.tile([P, HID], F32, tag="agg")
                nc.vector.tensor_tensor(
                    out=agg[:].rearrange("p (h d) -> p h d", h=H),
                    in0=nm[:, :HID].rearrange("p (h d) -> p h d", h=H),
                    in1=rec[:, :, None].to_broadcast([P, H, D]),
                    op=OP.mult)
                glu = wp.tile([P, HID], F32, tag="glu")
                if os.environ.get("HGT_BACKEND", "hw") == "sim":
                    # CoreSim has no Gelu LUT: tanh approximation (dev only)
                    t1 = wp.tile([P, HID], F32, tag="gelu_t1")
                    nc.vector.tensor_tensor(out=t1[:], in0=agg[:], in1=agg[:], op=OP.mult)
                    nc.vector.tensor_tensor(out=t1[:], in0=t1[:], in1=agg[:], op=OP.mult)
                    nc.vector.tensor_scalar(out=t1[:], in0=t1[:], scalar1=0.044715,
                                            scalar2=None, op0=OP.mult)
                    nc.vector.tensor_tensor(out=t1[:], in0=t1[:], in1=agg[:], op=OP.add)
                    nc.scalar.activation(out=t1[:], in_=t1[:], func=AF.Tanh,
                                         scale=0.7978845608028654)
                    nc.vector.tensor_scalar(out=t1[:], in0=t1[:], scalar1=0.5,
                                            scalar2=0.5, op0=OP.mult, op1=OP.add)
                    nc.vector.tensor_tensor(out=glu[:], in0=t1[:], in1=agg[:], op=OP.mult)
                else:
                    nc.scalar.activation(out=glu[:], in_=agg[:], func=AF.Gelu)
                gluT_ps = pp_t.tile([P, P], F32, tag="tp_ps")
                nc.tensor.transpose(out=gluT_ps[:], in_=glu[:], identity=consts["ident"][:])
                gluT = wp.tile([P, P], F32, tag="gluT")
                nc.scalar.copy(out=gluT[:], in_=gluT_ps[:])
                o_ps = pp_mm.tile([P, HID], F32, tag="mm_ps")
                nc.tensor.matmul(out=o_ps[:], lhsT=gluT[:], rhs=consts["Wo"][t][:],
                                 start=True, stop=True)
                # o3 = g*o + (g*b_o) + (1-g)*xs  (bo const already has g*b_o)
                xs_t = wp.tile([P, HID], F32, tag="xs_ld")
                nc.sync.dma_start(xs_t[:], xs_in[r0:r0 + P, :])
                o1 = o1_all[:, i, :]
                nc.vector.tensor_scalar_mul(o1, o_ps[:], float(g_list[t]))
                nc.vector.tensor_tensor(out=o1, in0=o1, in1=consts["bo"][t][:], op=OP.add)
                xs_s = wp.tile([P, HID], F32, tag="xs_s")
                nc.vector.tensor_scalar_mul(xs_s[:], xs_t[:], float(1.0 - g_list[t]))
                nc.vector.tensor_tensor(out=o1, in0=o1, in1=xs_s[:], op=OP.add)
                stats = sp.tile([P, nc.vector.BN_STATS_DIM], F32, tag="stats")
                nc.vector.bn_stats(out=stats[:], in_=o1)
                nc.vector.bn_aggr(out=mv_all[:, i, :], in_=stats[:])
            # one batched sqrt for all tiles' variances (single table load)
            rstd_all = cp.tile([P, NTL], F32, tag="rstd_all")
            nc.scalar.activation(out=rstd_all[:], in_=mv_all[:, :, 1],
                                 func=AF.Sqrt,
                                 bias=eps_t[:, 0:1])
            nc.vector.reciprocal(rstd_all[:], rstd_all[:])
            # pass 2: normalize + relu + next-layer projections (relu/copy are
            # in every ACT table set, so no further table switches).
            def _xh_relu(i, t, dst_ap):
                xh = wp.tile([P, HID], F32, tag="xh")
                nc.vector.tensor_scalar(
                    out=xh[:], in0=o1_all[:, i, :], scalar1=mv_all[:, i, 0:1],
                    scalar2=rstd_all[:, i:i + 1],
                    op0=OP.subtract, op1=OP.mult)
                nc.vector.tensor_tensor(out=xh[:], in0=xh[:], in1=consts["lng"][t][:], op=OP.mult)
                nc.vector.tensor_tensor(out=xh[:], in0=xh[:], in1=consts["lnb"][t][:], op=OP.add)
                nc.scalar.activation(out=dst_ap, in_=xh[:], func=AF.Relu)

            if not last:
                gi0 = 0
                for grp in _type_groups(c, tiles):
                    t = grp[0][0]
                    G = len(grp)
                    stg = sgp.tile([P, G, _kqv_width(t) * HID], F32, tag="stg")
                    for gi, (_, i_t, r0) in enumerate(grp):
                        _xh_relu(gi0 + gi, t, stg[:, gi, :HID])
                        _kqv_tile(nc, pools, c, consts, t, stg[:, gi, :HID], stg, gi)
                    _kqv_flush(nc, c, outs, t, grp[0][1], G, stg, grp[0][2])
                    gi0 += G
            else:
                for g0i in range(0, len(tiles), 4):
                    grp = tiles[g0i:g0i + 4]
                    G = len(grp)
                    dlst = sp.tile([P, 4, 1], F32, tag="dlst")
                    for gi, (t, i_t, r0) in enumerate(grp):
                        xs_new = wp.tile([P, HID], F32, tag="xs_new")
                        _xh_relu(g0i + gi, t, xs_new[:])
                        xnT_ps = pp_t.tile([P, P], F32, tag="tp_ps")
                        nc.tensor.transpose(out=xnT_ps[:], in_=xs_new[:], identity=consts["ident"][:])
                        xnT = wp.tile([P, P], F32, tag="xnT")
                        nc.scalar.copy(out=xnT[:], in_=xnT_ps[:])
                        d_ps = pp_mm.tile([P, 1], F32, tag="mm_ps")
                        nc.tensor.matmul(out=d_ps[:], lhsT=xnT[:], rhs=consts["whead"][:],
                                         start=True, stop=True)
                        nc.vector.tensor_tensor(out=dlst[:, gi, :], in0=d_ps[:],
                                                in1=consts["bh"][:], op=OP.add)
                    nc.gpsimd.dma_start(
                        dl_o[g0i * P:(g0i + G) * P, :].rearrange(
                            "(g p) f -> p g f", p=P),
                        dlst[:, :G, :])
    nc.compile()
    return nc


build_l23.g_list = None


# ---------------------------------------------------------------------------
# Runner
# ---------------------------------------------------------------------------

LAUNCH_TIMES_NS = []
TRACE_DIRS = []


def _run(nc, in_maps, cfg):
    backend = os.environ.get("HGT_BACKEND", "hw")
    if backend == "sim":
        from concourse.bass_interp import CoreSim
        results = []
        for m in in_maps:
            sim = CoreSim(nc, trace=False, require_finite=False, require_nnan=False)
            for k, v in m.items():
                sim.tensor(k)[:] = v
            sim.simulate(check_with_hw=False)
            out = {}
            for alloc in nc.m.functions[0].allocations:
                if isinstance(alloc, mybir.MemoryLocationSet) and alloc.kind == "ExternalOutput":
                    name = alloc.memorylocations[0].name
                    out[name] = sim.tensor(name).copy()
            results.append(out)
        return results
    else:
        from concourse.bass_utils import run_bass_kernel_spmd
        trace = os.environ.get("HGT_TRACE", "0") == "1"
        res = run_bass_kernel_spmd(nc, in_maps, core_ids=list(range(cfg.C)),
                                   trace=trace)
        if trace:
            LAUNCH_TIMES_NS.append(res.exec_time_ns)
            it = res.instructions_and_trace
            TRACE_DIRS.append(getattr(it, "trace_path", it))
        return res.results


# ---------------------------------------------------------------------------
# Main entry
# ---------------------------------------------------------------------------

def kernel(**inputs):
    cfg = Cfg()
    return _kernel_impl(cfg, inputs)


def _kernel_impl(cfg, inputs):
    c = cfg
    prm = prep_params(c, inputs)
    g = prep_graph(c, inputs)
    NT, tws = g["NT"], g["tws"]

    # ---- launch 1
    nc1 = build_l1(c)
    in_maps = []
    xa = np.asarray(inputs["x_a"], np.float32)
    xw = np.asarray(inputs["x_w"], np.float32)
    xo = np.asarray(inputs["x_o"], np.float32)

    def padx(x, n, npad):
        out = np.zeros((npad, P), np.float32)
        out[:n, :64] = x
        return out

    for cc in range(c.C):
        in_maps.append({
            "xa": padx(xa[cc * c.nac:(cc + 1) * c.nac], c.nac, c.nap),
            "xw": padx(xw[cc * c.nwc:(cc + 1) * c.nwc], c.nwc, c.nwp),
            "xo": padx(xo[cc * c.noc:(cc + 1) * c.noc], c.noc, c.nop),
            "Win": prm["Win"], "bin": prm["bin"],
            "Wcat": prm["Wcat0"], "bcat": prm["bcat0"],
            "ident": prm["ident"],
        })
    r1 = _run(nc1, in_maps, c)

    def assemble_tables(res):
        """Build global kd table + per-core q' tables + global ve table."""
        kd_tab = np.empty((c.KD_ROWS, HID), np.float32)
        kd_tab[-1] = 1.0
        ve_tab = np.empty((c.VE_ROWS, HID), np.float32)
        ve_tab[-1] = 0.0
        qp_tabs = []
        for cc in range(c.C):
            kd = res[cc]["kd"]
            ve = res[cc]["ve"]
            # kd local [a|w|o] -> global
            kd_tab[c.KOFF[0] + cc * c.nac:c.KOFF[0] + (cc + 1) * c.nac] = kd[:c.nac]
            kd_tab[c.KOFF[1] + cc * c.nwc:c.KOFF[1] + (cc + 1) * c.nwc] = \
                kd[c.base_local[1]:c.base_local[1] + c.nwc]
            kd_tab[c.KOFF[2] + cc * c.noc:c.KOFF[2] + (cc + 1) * c.noc] = \
                kd[c.base_local[2]:c.base_local[2] + c.noc]
            # ve local slots [a-et2, a-et3, w-et0, o-et1] -> global stacked
            ve_tab[c.VOFF[2] + cc * c.nac:c.VOFF[2] + (cc + 1) * c.nac] = \
                ve[c.QB[0]:c.QB[0] + c.nac]
            ve_tab[c.VOFF[3] + cc * c.nac:c.VOFF[3] + (cc + 1) * c.nac] = \
                ve[c.QB[1]:c.QB[1] + c.nac]
            ve_tab[c.VOFF[0] + cc * c.nwc:c.VOFF[0] + (cc + 1) * c.nwc] = \
                ve[c.QB[2]:c.QB[2] + c.nwc]
            ve_tab[c.VOFF[1] + cc * c.noc:c.VOFF[1] + (cc + 1) * c.noc] = \
                ve[c.QB[3]:c.QB[3] + c.noc]
            # pad-edge q rows are -8.0: with pad k rows = 1.0 the pad alpha is
            # 128 * -8 = -1024 (bf16-safe), exp -> 0.
            qp = np.vstack([res[cc]["qp"], np.full((1, HID), -8.0, np.float32)])
            qp_tabs.append(qp)
        return kd_tab, ve_tab, qp_tabs

    import ml_dtypes
    bf16 = ml_dtypes.bfloat16

    # ---- launches 2 and 3
    # last launch only needs a-dst windows (head reads only a-type nodes)
    NT_a = int(tws[:c.ntile_a].sum())
    res = r1
    for l, last in ((1, False), (2, True)):
        kd_tab, ve_tab, qp_tabs = assemble_tables(res)
        lay = l - 1  # layer params index: launch2 -> layer 0, launch3 -> layer 1
        build_l23.g_list = prm[f"g{lay}"]
        NT_l = NT_a if last else NT
        nc = build_l23(c, NT_l, tws, last)
        in_maps = []
        for cc in range(c.C):
            ti = g["tile_idx"][cc][:NT_l]  # [NT_l, P, 4]
            kg_s = np.ascontiguousarray(
                kd_tab[ti[:, :, 0]].transpose(1, 0, 2)).astype(bf16)
            vg_s = np.ascontiguousarray(
                ve_tab[ti[:, :, 1]].transpose(1, 0, 2)).astype(bf16)
            qg_s = np.ascontiguousarray(
                qp_tabs[cc][ti[:, :, 2]].transpose(1, 0, 2)).astype(bf16)
            dstlT = np.ascontiguousarray(ti[:, :, 3].T)
            m = {
                "kg": kg_s, "vg": vg_s, "qg": qg_s,
                "xsin": res[cc]["xs"][:c.nap] if last else res[cc]["xs"],
                "dstlT": dstlT,
                "iota": prm["iota"], "ident": prm["ident"],
                "Wo": prm[f"Wo{lay}"], "bo": prm[f"bo{lay}"],
                "lng": prm[f"lng{lay}"], "lnb": prm[f"lnb{lay}"],
                "gs": prm[f"g{lay}"],
            }
            if not last:
                m.update({"Wcat": prm[f"Wcat{lay + 1}"], "bcat": prm[f"bcat{lay + 1}"]})
            else:
                m.update({"whead": prm["whead"], "bh": prm["bh"]})
            in_maps.append(m)
        res = _run(nc, in_maps, c)

    out = np.concatenate([res[cc]["delta"][:c.nac, 0] for cc in range(c.C)])
    return out.astype(np.float32)



# revision 21
# speedup vs baseline: 2.9837x; 1.1579x over previous
"""HGT regressor on 8 Trainium2 NeuronCores (Bass/Tile).

Strategy (graph/data parallel, hint-following):
  - Nodes of each type are partitioned contiguously across the 8 cores
    (a: 12500/core, w: 2500/core, o: 6250/core). Each core owns the edges
    whose *destination* lies in its node shard.
  - Per layer, each core computes K = kqv[:, :128] (raw) and the per-edge-type
    source-side V transform (m_rel folded at source) plus the destination-side
    Q transform (a_rel * p_rel * scale folded into Q) for its own nodes only.
  - The full K / V_et tables are exchanged between layer launches via the host
    (replicated to all cores), i.e. host-mediated all-gather. Q' stays local.
  - Edge phase per core: edges sorted by local destination row, grouped into
    128-node windows; per 128-edge tile: indirect-DMA gathers of K[src],
    V_et[src], Q'_et[dst]; alpha = sum_h(K*Q'); ex = exp(alpha); payload
    [ex*V | ex] is scatter-added into a PSUM window accumulator via a
    one-hot matmul; windows flush densely to a numer/den table in DRAM.
  - Node phase per core: agg = numer/den, gelu, W_o, gated skip, LayerNorm,
    relu, then next-layer projections (or the scalar head in the last layer).
  - Softmax needs no running max: alpha = q'k with these parameter scales is
    O(1); exp cannot overflow, and softmax is shift-invariant anyway.
"""
import os
import sys

sys.path.insert(0, "/opt/trn_rl_repo")

import numpy as np

import concourse.bass as bass
import concourse.mybir as mybir
import concourse.tile as tile
from concourse import bacc

P = 128
H, D, HID = 4, 32, 128
PAY = HID + H  # 132
F32 = mybir.dt.float32
F16 = mybir.dt.float16
BF16 = mybir.dt.bfloat16
I32 = mybir.dt.int32
AF = mybir.ActivationFunctionType
OP = mybir.AluOpType


def _ceil(a, b):
    return (a + b - 1) * b // b if False else -(-a // b) * b


def cdiv(a, b):
    return -(-a // b)


class Cfg:
    """All sizes derived from problem scale; supports mini-scale testing."""

    def __init__(self, NA=100000, NWK=20000, NO=50000, E=150000, C=8):
        self.NA, self.NWK, self.NO, self.E, self.C = NA, NWK, NO, E, C
        assert NA % C == 0 and NWK % C == 0 and NO % C == 0
        self.nac, self.nwc, self.noc = NA // C, NWK // C, NO // C
        self.nap, self.nwp, self.nop = (
            cdiv(self.nac, P) * P,
            cdiv(self.nwc, P) * P,
            cdiv(self.noc, P) * P,
        )
        # local node-row layout (numer/xs/kd rows): [a | w | o], each padded
        self.base_local = (0, self.nap, self.nap + self.nwp)
        self.LOCN = self.nap + self.nwp + self.nop
        self.NWIN = self.LOCN // P
        # per-type tile counts
        self.ntile_a, self.ntile_w, self.ntile_o = (
            self.nap // P,
            self.nwp // P,
            self.nop // P,
        )
        # q' local layout: slots [a-et0, a-et1, w-et2, o-et3]
        self.QB = (0, self.nap, 2 * self.nap, 2 * self.nap + self.nwp)
        self.QTOT = 2 * self.nap + self.nwp + self.nop
        # ve local layout (same bases): slots [a-et2, a-et3, w-et0, o-et1]
        # global kd table layout: [a 0..NA | w | o] + trash
        self.KOFF = (0, NA, NA + NWK)
        self.KD_ROWS = NA + NWK + NO + 1
        # global stacked ve table: [et0 w | et1 o | et2 a | et3 a] + trash
        self.VOFF = (0, NWK, NWK + NO, NWK + NO + NA)
        self.VE_ROWS = NWK + NO + 2 * NA + 1


# edge types: (src_type, dst_type)
ETYPES = ((1, 0), (2, 0), (0, 1), (0, 2))


# ---------------------------------------------------------------------------
# Host-side preprocessing
# ---------------------------------------------------------------------------

def prep_graph(cfg, inputs):
    """Compute per-core edge tile indices. Shared across both layers.

    Returns dict with:
      NT: static tile count (same all cores)
      tws: [NWIN] tiles per window (static across cores)
      tile_idx: [C][NT, P, 4] int32  (kidx, vidx, qidx, dst_local)
    """
    c = cfg
    edges = []  # per et: (src, dst)
    for name_s, name_d in (("src_wa", "dst_wa"), ("src_oa", "dst_oa"),
                           ("src_aw", "dst_aw"), ("src_ao", "dst_ao")):
        edges.append((np.asarray(inputs[name_s]), np.asarray(inputs[name_d])))

    shard_n = (c.nac, c.nwc, c.noc)
    # concat all ets with global indices
    K_TRASH = c.KD_ROWS - 1
    V_TRASH = c.VE_ROWS - 1
    Q_TRASH = c.QTOT  # row appended by host to the q' table

    all_core = []
    koff_by_et = (c.KOFF[1], c.KOFF[2], c.KOFF[0], c.KOFF[0])  # src type offset in kd
    for et, (st, dt) in enumerate(ETYPES):
        src, dst = edges[et]
        kidx = koff_by_et[et] + src
        vidx = c.VOFF[et] + src
        core = dst // shard_n[dt]
        dloc = dst - core * shard_n[dt]  # dst index within its type shard
        # local numer row / q' row
        tb = (c.base_local[0], c.base_local[1], c.base_local[2])[dt]
        row = tb + dloc
        qslot = {0: 0, 1: 1, 2: 2, 3: 3}[et]
        qidx = c.QB[qslot] + dloc
        all_core.append((core, row, kidx, vidx, qidx))

    core_cat = np.concatenate([a[0] for a in all_core])
    row_cat = np.concatenate([a[1] for a in all_core])
    k_cat = np.concatenate([a[2] for a in all_core])
    v_cat = np.concatenate([a[3] for a in all_core])
    q_cat = np.concatenate([a[4] for a in all_core])

    # per-core, per-window edge counts -> static tile structure
    win_cat = row_cat // P
    counts = np.zeros((c.C, c.NWIN), np.int64)
    for cc in range(c.C):
        m = core_cat == cc
        counts[cc] = np.bincount(win_cat[m], minlength=c.NWIN)
    tws = np.maximum(cdiv(counts.max(axis=0), P), 1)  # >=1 tile per window
    NT = int(tws.sum())
    tile_base = np.zeros(c.NWIN, np.int64)
    tile_base[1:] = np.cumsum(tws)[:-1]

    tile_idx = np.zeros((c.C, NT, P, 4), np.int32)
    # fill pads with trash rows -> ex = 0 contributions
    tile_idx[:, :, :, 0] = K_TRASH
    tile_idx[:, :, :, 1] = V_TRASH
    tile_idx[:, :, :, 2] = Q_TRASH
    tile_idx[:, :, :, 3] = 0
    for cc in range(c.C):
        m = core_cat == cc
        rows = row_cat[m]
        order = np.argsort(rows, kind="stable")
        rows = rows[order]
        ks, vs, qs = k_cat[m][order], v_cat[m][order], q_cat[m][order]
        wins = rows // P
        dstl = rows % P
        # position within window
        wstart = np.searchsorted(wins, np.arange(c.NWIN), side="left")
        pos = np.arange(rows.size) - wstart[wins]
        slot_t = pos // P   # tile within window
        slot_p = pos % P    # partition
        gt = tile_base[wins] + slot_t  # global tile id
        tile_idx[cc, gt, slot_p, 0] = ks
        tile_idx[cc, gt, slot_p, 1] = vs
        tile_idx[cc, gt, slot_p, 2] = qs
        tile_idx[cc, gt, slot_p, 3] = dstl
    return {"NT": NT, "tws": tws.astype(np.int64), "tile_idx": tile_idx,
            "tile_base": tile_base}


def blockdiag(M):
    out = np.zeros((HID, HID), np.float32)
    for h in range(H):
        out[h * D:(h + 1) * D, h * D:(h + 1) * D] = M[h]
    return out


def prep_params(cfg, inputs):
    """Fold and lay out all parameters (host, tiny)."""
    scale = np.float32(1.0 / np.sqrt(D))
    a_rel = np.asarray(inputs["a_rel"])
    m_rel = np.asarray(inputs["m_rel"])
    p_rel = np.asarray(inputs["p_rel"])
    prm = {}
    rep = lambda v, w: np.broadcast_to(np.asarray(v, np.float32)[None, :], (P, w)).copy()
    for l in range(2):
        BDaT, BDm = [], []
        for et in range(4):
            a_eff = a_rel[l, et] * (p_rel[l, et] * scale)[:, None, None]
            BDaT.append(blockdiag(a_eff).T.copy())
            BDm.append(blockdiag(m_rel[l, et]))
        prm[f"BDaT{l}"] = np.stack(BDaT)  # [4,128,128]
        prm[f"BDm{l}"] = np.stack(BDm)
        prm[f"Wkqv{l}"] = np.asarray(inputs["W_kqv"])[l]       # [3,128,384]
        prm[f"bkqv{l}"] = np.stack([rep(np.asarray(inputs["b_kqv"])[l, t], 3 * HID) for t in range(3)])
        # folded projection weights: per type, slots [kd | q'_ets | v'_ets]
        # q' = xs @ (Wq @ BDaT_et), v' = xs @ (Wv @ BDm_et); biases likewise.
        import ml_dtypes
        q_ets = ((0, 1), (2,), (3,))
        v_ets = ((2, 3), (0,), (1,))
        Wcat = np.zeros((3, HID, 5 * HID), np.float32)
        bcat = np.zeros((3, P, 5 * HID), np.float32)
        for t in range(3):
            Wk = np.asarray(inputs["W_kqv"])[l, t]
            bk = np.asarray(inputs["b_kqv"])[l, t]
            cols = [Wk[:, :HID]]
            bs = [bk[:HID]]
            for et in q_ets[t]:
                cols.append(Wk[:, HID:2 * HID] @ prm[f"BDaT{l}"][et])
                bs.append(bk[HID:2 * HID] @ prm[f"BDaT{l}"][et])
            for et in v_ets[t]:
                cols.append(Wk[:, 2 * HID:] @ prm[f"BDm{l}"][et])
                bs.append(bk[2 * HID:] @ prm[f"BDm{l}"][et])
            cat = np.concatenate(cols, axis=1)
            Wcat[t, :, :cat.shape[1]] = cat
            bcat[t, :, :cat.shape[1]] = np.concatenate(bs)[None, :]
        prm[f"Wcat{l}"] = Wcat.astype(ml_dtypes.bfloat16)
        prm[f"bcat{l}"] = bcat
        prm[f"Wo{l}"] = np.asarray(inputs["W_o"])[l]           # [3,128,128]
        g = 1.0 / (1.0 + np.exp(-np.asarray(inputs["skip_p"], np.float64)))  # [2,3]
        prm[f"g{l}"] = g[l].astype(np.float32)
        prm[f"bo{l}"] = np.stack([rep(np.asarray(inputs["b_o"])[l, t] * g[l, t], HID) for t in range(3)])
        prm[f"lng{l}"] = np.stack([rep(np.asarray(inputs["ln_g"])[l, t], HID) for t in range(3)])
        prm[f"lnb{l}"] = np.stack([rep(np.asarray(inputs["ln_b"])[l, t], HID) for t in range(3)])
    # input proj, padded to 128 contraction
    W_in = np.asarray(inputs["W_in"])  # [3,64,128]
    Wp = np.zeros((3, 128, HID), np.float32)
    Wp[:, :64, :] = W_in
    prm["Win"] = Wp
    prm["bin"] = np.stack([rep(np.asarray(inputs["b_in"])[t], HID) for t in range(3)])
    prm["whead"] = np.asarray(inputs["w_head"], np.float32)  # [128,1]
    prm["bh"] = np.full((P, 1), float(np.asarray(inputs["b_head"])[0] + np.asarray(inputs["base"])[0]), np.float32)
    prm["iota"] = np.broadcast_to(np.arange(128, dtype=np.int32)[None, :], (P, 128)).copy()
    prm["ident"] = np.eye(128, dtype=np.float32)
    return prm


# ---------------------------------------------------------------------------
# Builders
# ---------------------------------------------------------------------------

_CONST_N = [0]


def _load_const(nc, cp, ap, shape, dtype=F32):
    _CONST_N[0] += 1
    t = cp.tile(list(shape), dtype, tag=f"cst{_CONST_N[0]}")
    nc.sync.dma_start(t[:], ap)
    return t


def _type_tiles(cfg):
    """Yield (t, i_t, r0) for all node tiles: type, tile-in-type, local row base."""
    out = []
    for t, (ntile, b) in enumerate(
        zip((cfg.ntile_a, cfg.ntile_w, cfg.ntile_o), cfg.base_local)
    ):
        for i in range(ntile):
            out.append((t, i, b + i * P))
    return out


def _kqv_width(t):
    """Staging width in slots for type t: [xs | kd | q'_ets | v'_ets]."""
    return 6 if t == 0 else 4


def _kqv_tile(nc, pools, cfg, consts, t, xs_tile, stg, gi):
    """Next-layer projections for one tile via folded weights.

    Writes kd/q'/v' (slots 1..) of stg[:, gi, :]; slot 0 (xs) is written by
    the caller. One transpose + 1-2 bf16 matmuls + 1-2 bias-adds.
    """
    wp, pp_t, pp_mm = pools["wp"], pools["pp_t"], pools["pp_mm"]
    xsT_ps = pp_t.tile([P, P], F32, tag="tp_ps")
    nc.tensor.transpose(out=xsT_ps[:], in_=xs_tile[:], identity=consts["ident"][:])
    xsT = wp.tile([P, P], BF16, tag="xsT")
    nc.scalar.copy(out=xsT[:], in_=xsT_ps[:])
    W = consts["Wcat"][t]
    B = consts["bcat"][t]
    nw = (_kqv_width(t) - 1) * HID  # matmul output width (kd + q + v slots)
    n1 = min(nw, 4 * HID)
    ps1 = pp_mm.tile([P, 4 * HID], F32, tag="mmk_ps")
    nc.tensor.matmul(out=ps1[:, :n1], lhsT=xsT[:], rhs=W[:, :n1],
                     start=True, stop=True)
    if _kqv_tile.bias_zero:
        # biases are all zero: plain copy, on the (less busy) scalar engine
        nc.scalar.copy(out=stg[:, gi, HID:HID + n1], in_=ps1[:, :n1])
    else:
        nc.vector.tensor_tensor(out=stg[:, gi, HID:HID + n1], in0=ps1[:, :n1],
                                in1=B[:, :n1], op=OP.add)
    if nw > n1:
        ps2 = pp_mm.tile([P, HID], F32, tag="mm_ps")
        nc.tensor.matmul(out=ps2[:], lhsT=xsT[:], rhs=W[:, n1:nw],
                         start=True, stop=True)
        if _kqv_tile.bias_zero:
            nc.vector.tensor_copy(out=stg[:, gi, HID + n1:HID + nw], in_=ps2[:])
        else:
            nc.vector.tensor_tensor(out=stg[:, gi, HID + n1:HID + nw],
                                    in0=ps2[:], in1=B[:, n1:nw], op=OP.add)


_kqv_tile.bias_zero = False


def _kqv_flush(nc, cfg, outs, t, i0, G, stg, r0base, write_xs=True):
    """DMA a group of G tiles' staged [xs|kd|q|v] slots to their tables."""
    kd_o, qp_o, ve_o = outs["kd"], outs["qp"], outs["ve"]
    xs_o = outs.get("xs")
    rt0 = i0 * P

    def wr(dst, lo):
        nc.gpsimd.dma_start(dst.rearrange("(g p) f -> p g f", p=P),
                            stg[:, :, lo * HID:(lo + 1) * HID])

    if write_xs and xs_o is not None:
        wr(xs_o[r0base:r0base + G * P, :], 0)
    wr(kd_o[r0base:r0base + G * P, :], 1)
    q_slots = ((0, 1), (2,), (3,))[t]
    v_slots = ((0, 1), (2,), (3,))[t]
    for j, sl in enumerate(q_slots):
        wr(qp_o[cfg.QB[sl] + rt0:cfg.QB[sl] + rt0 + G * P, :], 2 + j)
    for j, sl in enumerate(v_slots):
        wr(ve_o[cfg.QB[sl] + rt0:cfg.QB[sl] + rt0 + G * P, :],
           2 + len(q_slots) + j)


def _type_groups(cfg, tiles, G=4):
    """Split the ordered tile list into same-type groups of <= G."""
    groups = []
    cur = []
    for tt in tiles:
        if cur and (tt[0] != cur[0][0] or len(cur) == G):
            groups.append(cur)
            cur = []
        cur.append(tt)
    if cur:
        groups.append(cur)
    return groups


def build_l1(cfg):
    """Launch 1: input proj + relu -> xs1; kqv chain -> kd/q'/ve tables."""
    nc = bacc.Bacc("TRN2", target_bir_lowering=False, debug=False,
                   num_devices=cfg.C)
    c = cfg
    xa = nc.dram_tensor("xa", [c.nap, P], F32, kind="ExternalInput").ap()
    xw = nc.dram_tensor("xw", [c.nwp, P], F32, kind="ExternalInput").ap()
    xo = nc.dram_tensor("xo", [c.nop, P], F32, kind="ExternalInput").ap()
    Win = nc.dram_tensor("Win", [3, P, HID], F32, kind="ExternalInput").ap()
    binp = nc.dram_tensor("bin", [3, P, HID], F32, kind="ExternalInput").ap()
    Wcat_d = nc.dram_tensor("Wcat", [3, P, 5 * HID], BF16, kind="ExternalInput").ap()
    bcat_d = nc.dram_tensor("bcat", [3, P, 5 * HID], F32, kind="ExternalInput").ap()
    ident_d = nc.dram_tensor("ident", [P, P], F32, kind="ExternalInput").ap()

    xs_o = nc.dram_tensor("xs", [c.LOCN, P], F32, kind="ExternalOutput").ap()
    kd_o = nc.dram_tensor("kd", [c.LOCN, P], F32, kind="ExternalOutput").ap()
    qp_o = nc.dram_tensor("qp", [c.QTOT, P], F32, kind="ExternalOutput").ap()
    ve_o = nc.dram_tensor("ve", [c.QTOT, P], F32, kind="ExternalOutput").ap()

    xin = (xa, xw, xo)
    with tile.TileContext(nc) as tc:
        with tc.tile_pool(name="consts", bufs=1) as cp, \
             tc.tile_pool(name="work", bufs=4) as wp, \
             tc.tile_pool(name="stgp", bufs=2) as sgp, \
             tc.tile_pool(name="ppt", bufs=2, space="PSUM") as pp_t, \
             tc.tile_pool(name="ppmm", bufs=2, space="PSUM") as pp_mm:
            consts = {
                "ident": _load_const(nc, cp, ident_d[:, :], (P, P)),
                "Win": [_load_const(nc, cp, Win[t], (P, HID)) for t in range(3)],
                "bin": [_load_const(nc, cp, binp[t], (P, HID)) for t in range(3)],
                "Wcat": [_load_const(nc, cp, Wcat_d[t], (P, 5 * HID), BF16) for t in range(3)],
                "bcat": [_load_const(nc, cp, bcat_d[t], (P, 5 * HID)) for t in range(3)],
            }
            pools = {"cp": cp, "wp": wp, "pp_t": pp_t, "pp_mm": pp_mm}
            outs = {"kd": kd_o, "qp": qp_o, "ve": ve_o, "xs": xs_o}
            for grp in _type_groups(c, _type_tiles(c)):
                t = grp[0][0]
                G = len(grp)
                stg = sgp.tile([P, G, _kqv_width(t) * HID], F32, tag="stg")
                for gi, (_, i_t, r0) in enumerate(grp):
                    x_t = wp.tile([P, P], F32, tag="x_in")
                    nc.sync.dma_start(x_t[:], xin[t][i_t * P:(i_t + 1) * P, :])
                    xT_ps = pp_t.tile([P, P], F32, tag="tp_ps")
                    nc.tensor.transpose(out=xT_ps[:], in_=x_t[:], identity=consts["ident"][:])
                    xT = wp.tile([P, P], F32, tag="xT")
                    nc.scalar.copy(out=xT[:], in_=xT_ps[:])
                    pj_ps = pp_mm.tile([P, HID], F32, tag="mm_ps")
                    nc.tensor.matmul(out=pj_ps[:], lhsT=xT[:], rhs=consts["Win"][t][:],
                                     start=True, stop=True)
                    pj = wp.tile([P, HID], F32, tag="pj")
                    nc.vector.tensor_tensor(out=pj[:], in0=pj_ps[:],
                                            in1=consts["bin"][t][:], op=OP.add)
                    nc.scalar.activation(out=stg[:, gi, :HID], in_=pj[:], func=AF.Relu)
                    _kqv_tile(nc, pools, c, consts, t, stg[:, gi, :HID], stg, gi)
                _kqv_flush(nc, c, outs, t, grp[0][1], G, stg, grp[0][2])
    nc.compile()
    return nc


def build_l23(cfg, NT, tws, last):
    """Launches 2/3: edge phase + node phase (+ head if last)."""
    nc = bacc.Bacc("TRN2", target_bir_lowering=False, debug=False,
                   num_devices=cfg.C)
    c = cfg
    kg_d = nc.dram_tensor("kg", [P, NT, HID], BF16, kind="ExternalInput").ap()
    vg_d = nc.dram_tensor("vg", [P, NT, HID], BF16, kind="ExternalInput").ap()
    qg_d = nc.dram_tensor("qg", [P, NT, HID], BF16, kind="ExternalInput").ap()
    NWIN_E = c.ntile_a if last else c.NWIN   # only a-dst windows feed the head
    LOCN_E = c.nap if last else c.LOCN
    xs_in = nc.dram_tensor("xsin", [LOCN_E, P], F32, kind="ExternalInput").ap()
    ti_t = nc.dram_tensor("dstlT", [P, NT], I32, kind="ExternalInput").ap()
    iota_d = nc.dram_tensor("iota", [P, P], I32, kind="ExternalInput").ap()
    ident_d = nc.dram_tensor("ident", [P, P], F32, kind="ExternalInput").ap()
    Wo_d = nc.dram_tensor("Wo", [3, P, HID], F32, kind="ExternalInput").ap()
    bo_d = nc.dram_tensor("bo", [3, P, HID], F32, kind="ExternalInput").ap()
    lng_d = nc.dram_tensor("lng", [3, P, HID], F32, kind="ExternalInput").ap()
    lnb_d = nc.dram_tensor("lnb", [3, P, HID], F32, kind="ExternalInput").ap()
    gs_d = nc.dram_tensor("gs", [3], F32, kind="ExternalInput").ap()  # unused on-device; values baked via bo/g mul
    if not last:
        Wcat_d = nc.dram_tensor("Wcat", [3, P, 5 * HID], BF16, kind="ExternalInput").ap()
        bcat_d = nc.dram_tensor("bcat", [3, P, 5 * HID], F32, kind="ExternalInput").ap()
    else:
        wh_d = nc.dram_tensor("whead", [P, 1], F32, kind="ExternalInput").ap()
        bh_d = nc.dram_tensor("bh", [P, 1], F32, kind="ExternalInput").ap()

    if not last:
        xs_o = nc.dram_tensor("xs", [c.LOCN, P], F32, kind="ExternalOutput").ap()
        kd_o = nc.dram_tensor("kd", [c.LOCN, P], F32, kind="ExternalOutput").ap()
        qp_o = nc.dram_tensor("qp", [c.QTOT, P], F32, kind="ExternalOutput").ap()
        ve_o = nc.dram_tensor("ve", [c.QTOT, P], F32, kind="ExternalOutput").ap()
    else:
        dl_o = nc.dram_tensor("delta", [c.nap, 1], F32, kind="ExternalOutput").ap()

    # gains folded on host: bo tile already contains g*b_o. g itself baked as consts below.
    g_vals = None  # set in kernel() via attribute hack? no: pass via build arg
    g_list = build_l23.g_list  # [3] floats for this layer

    with tile.TileContext(nc) as tc:
        with tc.tile_pool(name="consts", bufs=1) as cp, \
             tc.tile_pool(name="idx", bufs=2) as idxp, \
             tc.tile_pool(name="gat", bufs=2) as gp, \
             tc.tile_pool(name="ework", bufs=2) as ewp, \
             tc.tile_pool(name="nwork", bufs=3) as wp, \
             tc.tile_pool(name="stgp", bufs=2) as sgp, \
             tc.tile_pool(name="small", bufs=4) as sp, \
             tc.tile_pool(name="flush", bufs=2) as fp, \
             tc.tile_pool(name="dram", bufs=1, space="DRAM") as dp, \
             tc.tile_pool(name="ppe", bufs=2, space="PSUM") as pp_e, \
             tc.tile_pool(name="ppt", bufs=2, space="PSUM") as pp_t, \
             tc.tile_pool(name="ppmm", bufs=2, space="PSUM") as pp_mm:

            numer = dp.tile([LOCN_E, PAY], F32)
            eps_t = cp.tile([P, 1], F32, tag="lneps")
            nc.vector.memset(eps_t[:], 1e-5)

            consts = {
                "iota": _load_const(nc, cp, iota_d[:, :], (P, P), I32),
                "ident": _load_const(nc, cp, ident_d[:, :], (P, P)),
                "Wo": [_load_const(nc, cp, Wo_d[t], (P, HID)) for t in range(3)],
                "bo": [_load_const(nc, cp, bo_d[t], (P, HID)) for t in range(3)],
                "lng": [_load_const(nc, cp, lng_d[t], (P, HID)) for t in range(3)],
                "lnb": [_load_const(nc, cp, lnb_d[t], (P, HID)) for t in range(3)],
            }
            if not last:
                consts.update({
                    "Wcat": [_load_const(nc, cp, Wcat_d[t], (P, 5 * HID), BF16) for t in range(3)],
                    "bcat": [_load_const(nc, cp, bcat_d[t], (P, 5 * HID)) for t in range(3)],
                })
            else:
                consts["whead"] = _load_const(nc, cp, wh_d[:, :], (P, 1))
                consts["bh"] = _load_const(nc, cp, bh_d[:, :], (P, 1))

            # ---------------- edge phase ----------------
            # streams are host-pre-gathered per edge (bf16). Windows are
            # grouped greedily into supers (<= SUPER_T tiles); per super one
            # DMA per stream + one batched op per DVE stage; scatter stays a
            # per-tile bf16 one-hot matmul into the window's PSUM accumulator.
            SUPER_T = 20
            supers = []  # (g0, [T_w...], w0)
            gtile = 0
            w = 0
            while w < NWIN_E:
                g0 = gtile
                ts = []
                w0 = w
                while w < NWIN_E and len(ts) < 4 and \
                        sum(ts) + int(tws[w]) <= SUPER_T:
                    ts.append(int(tws[w]))
                    gtile += int(tws[w])
                    w += 1
                supers.append((g0, ts, w0))
            for g0, ts, w0 in supers:
                TS = sum(ts)
                kgt = gp.tile([P, TS, HID], BF16, tag="kgt")
                nc.sync.dma_start(kgt[:], kg_d[:, g0:g0 + TS, :])
                vgt = gp.tile([P, TS, HID], BF16, tag="vgt")
                nc.sync.dma_start(vgt[:], vg_d[:, g0:g0 + TS, :])
                qgt = gp.tile([P, TS, HID], BF16, tag="qgt")
                nc.scalar.dma_start(qgt[:], qg_d[:, g0:g0 + TS, :])
                dstl = idxp.tile([P, TS], I32, tag="dstl")
                nc.scalar.dma_start(dstl[:], ti_t[:, g0:g0 + TS])

                prod = ewp.tile([P, TS, HID], BF16, tag="prod")
                nc.vector.tensor_tensor(out=prod[:], in0=kgt[:], in1=qgt[:],
                                        op=OP.mult)
                alpha = ewp.tile([P, TS, H], F32, tag="alpha")
                nc.vector.tensor_reduce(
                    out=alpha[:],
                    in_=prod[:].rearrange("p t (h d) -> p t h d", h=H),
                    axis=mybir.AxisListType.X, op=OP.add)
                payload = ewp.tile([P, TS, PAY], BF16, tag="payload")
                ex = payload[:, :, HID:HID + H]
                nc.scalar.activation(out=ex, in_=alpha[:], func=AF.Exp)
                nc.vector.tensor_tensor(
                    out=payload[:, :, :HID].rearrange("p t (h d) -> p t h d", h=H),
                    in0=vgt[:].rearrange("p t (h d) -> p t h d", h=H),
                    in1=ex[:, :, :, None].to_broadcast([P, TS, H, D]),
                    op=OP.mult)
                onehot = ewp.tile([P, TS, P], BF16, tag="onehot")
                nc.vector.tensor_tensor(
                    out=onehot[:],
                    in0=dstl[:, :, None].to_broadcast([P, TS, P]),
                    in1=consts["iota"][:, None, :].to_broadcast([P, TS, P]),
                    op=OP.is_equal)
                fl = fp.tile([P, len(ts), PAY], F32, tag="fl")
                toff = 0
                for wi, T in enumerate(ts):
                    psum_w = pp_e.tile([P, PAY], F32, tag="psw")
                    for t in range(toff, toff + T):
                        nc.tensor.matmul(out=psum_w[:], lhsT=onehot[:, t, :],
                                         rhs=payload[:, t, :],
                                         start=(t == toff),
                                         stop=(t == toff + T - 1))
                    nc.vector.tensor_copy(out=fl[:, wi, :], in_=psum_w[:])
                    toff += T
                nc.sync.dma_start(
                    numer[w0 * P:(w0 + len(ts)) * P, :].rearrange(
                        "(t p) f -> p t f", p=P),
                    fl[:])

            # ---------------- node phase ----------------
            pools = {"cp": cp, "wp": wp, "pp_t": pp_t, "pp_mm": pp_mm}
            outs = None if last else {"kd": kd_o, "qp": qp_o, "ve": ve_o, "xs": xs_o}
            tiles = [x for x in _type_tiles(c) if (not last) or x[0] == 0]
            NTL = len(tiles)
            # pass 1: all tiles up through the skip-add + LN stats; gelu is the
            # only table-based ACT function here so the scalar engine loads
            # the gelu table once instead of thrashing gelu<->sqrt per tile.
            o1_all = cp.tile([P, NTL, HID], F16, tag="o1_all")
            mv_all = cp.tile([P, NTL, 2], F32, tag="mv_all")
            gi0 = 0
            for grp in _type_groups(c, tiles):
                t = grp[0][0]
                G = len(grp)
                r00 = grp[0][2]
                nm4 = wp.tile([P, G, PAY], F32, tag="nm")
                nc.scalar.dma_start(
                    nm4[:],
                    numer[r00:r00 + G * P, :].rearrange("(g p) f -> p g f", p=P))
                xs4 = wp.tile([P, G, HID], F32, tag="xs_ld")
                nc.scalar.dma_start(
                    xs4[:],
                    xs_in[r00:r00 + G * P, :].rearrange("(g p) f -> p g f", p=P))
                den = sp.tile([P, G, H], F32, tag="den")
                nc.vector.tensor_scalar_add(den[:], nm4[:, :, HID:HID + H], 1e-16)
                rec = sp.tile([P, G, H], F32, tag="rec")
                nc.vector.reciprocal(rec[:], den[:])
                agg = wp.tile([P, G, HID], F32, tag="agg")
                nc.vector.tensor_tensor(
                    out=agg[:].rearrange("p g (h d) -> p g h d", h=H),
                    in0=nm4[:, :, :HID].rearrange("p g (h d) -> p g h d", h=H),
                    in1=rec[:, :, :, None].to_broadcast([P, G, H, D]),
                    op=OP.mult)
                glu = wp.tile([P, G, HID], F32, tag="glu")
                if os.environ.get("HGT_BACKEND", "hw") == "sim":
                    # CoreSim has no Gelu LUT: tanh approximation (dev only)
                    t1 = wp.tile([P, G, HID], F32, tag="gelu_t1")
                    nc.vector.tensor_tensor(out=t1[:], in0=agg[:], in1=agg[:], op=OP.mult)
                    nc.vector.tensor_tensor(out=t1[:], in0=t1[:], in1=agg[:], op=OP.mult)
                    nc.vector.tensor_scalar(out=t1[:], in0=t1[:], scalar1=0.044715,
                                            scalar2=None, op0=OP.mult)
                    nc.vector.tensor_tensor(out=t1[:], in0=t1[:], in1=agg[:], op=OP.add)
                    nc.scalar.activation(out=t1[:], in_=t1[:], func=AF.Tanh,
                                         scale=0.7978845608028654)
                    nc.vector.tensor_scalar(out=t1[:], in0=t1[:], scalar1=0.5,
                                            scalar2=0.5, op0=OP.mult, op1=OP.add)
                    nc.vector.tensor_tensor(out=glu[:], in0=t1[:], in1=agg[:], op=OP.mult)
                else:
                    nc.scalar.activation(out=glu[:], in_=agg[:], func=AF.Gelu)
                ops4 = pp_mm.tile([P, G, HID], F32, tag="mmk_ps")
                for gi in range(G):
                    gluT_ps = pp_t.tile([P, P], F32, tag="tp_ps")
                    nc.tensor.transpose(out=gluT_ps[:], in_=glu[:, gi, :],
                                        identity=consts["ident"][:])
                    gluT = wp.tile([P, P], F32, tag="gluT")
                    nc.scalar.copy(out=gluT[:], in_=gluT_ps[:])
                    nc.tensor.matmul(out=ops4[:, gi, :], lhsT=gluT[:],
                                     rhs=consts["Wo"][t][:],
                                     start=True, stop=True)
                # o3 = g*o + (g*b_o) + (1-g)*xs  (bo const already has g*b_o)
                o1g = o1_all[:, gi0:gi0 + G, :]
                nc.vector.tensor_scalar_mul(o1g, ops4[:], float(g_list[t]))
                nc.vector.tensor_tensor(
                    out=o1g, in0=o1g,
                    in1=consts["bo"][t][:, None, :].to_broadcast([P, G, HID]),
                    op=OP.add)
                xs_s = wp.tile([P, G, HID], F32, tag="xs_s")
                nc.vector.tensor_scalar_mul(xs_s[:], xs4[:], float(1.0 - g_list[t]))
                nc.vector.tensor_tensor(out=o1g, in0=o1g, in1=xs_s[:], op=OP.add)
                for gi in range(G):
                    stats = sp.tile([P, nc.vector.BN_STATS_DIM], F32, tag="stats")
                    nc.vector.bn_stats(out=stats[:], in_=o1_all[:, gi0 + gi, :])
                    nc.vector.bn_aggr(out=mv_all[:, gi0 + gi, :], in_=stats[:])
                gi0 += G
            # one batched sqrt for all tiles' variances (single table load)
            rstd_all = cp.tile([P, NTL], F32, tag="rstd_all")
            nc.scalar.activation(out=rstd_all[:], in_=mv_all[:, :, 1],
                                 func=AF.Sqrt,
                                 bias=eps_t[:, 0:1])
            nc.vector.reciprocal(rstd_all[:], rstd_all[:])
            # pass 2: normalize + relu + next-layer projections (relu/copy are
            # in every ACT table set, so no further table switches).
            def _xh_relu(i, t, dst_ap):
                xh = wp.tile([P, HID], F32, tag="xh")
                nc.vector.tensor_scalar(
                    out=xh[:], in0=o1_all[:, i, :], scalar1=mv_all[:, i, 0:1],
                    scalar2=rstd_all[:, i:i + 1],
                    op0=OP.subtract, op1=OP.mult)
                if not build_l23.ln_trivial:
                    nc.vector.tensor_tensor(out=xh[:], in0=xh[:], in1=consts["lng"][t][:], op=OP.mult)
                    nc.vector.tensor_tensor(out=xh[:], in0=xh[:], in1=consts["lnb"][t][:], op=OP.add)
                nc.scalar.activation(out=dst_ap, in_=xh[:], func=AF.Relu)

            if not last:
                gi0 = 0
                for grp in _type_groups(c, tiles):
                    t = grp[0][0]
                    G = len(grp)
                    stg = sgp.tile([P, G, _kqv_width(t) * HID], F32, tag="stg")
                    for gi, (_, i_t, r0) in enumerate(grp):
                        _xh_relu(gi0 + gi, t, stg[:, gi, :HID])
                        _kqv_tile(nc, pools, c, consts, t, stg[:, gi, :HID], stg, gi)
                    _kqv_flush(nc, c, outs, t, grp[0][1], G, stg, grp[0][2])
                    gi0 += G
            else:
                for g0i in range(0, len(tiles), 4):
                    grp = tiles[g0i:g0i + 4]
                    G = len(grp)
                    dlst = sp.tile([P, 4, 1], F32, tag="dlst")
                    for gi, (t, i_t, r0) in enumerate(grp):
                        xs_new = wp.tile([P, HID], F32, tag="xs_new")
                        _xh_relu(g0i + gi, t, xs_new[:])
                        xnT_ps = pp_t.tile([P, P], F32, tag="tp_ps")
                        nc.tensor.transpose(out=xnT_ps[:], in_=xs_new[:], identity=consts["ident"][:])
                        xnT = wp.tile([P, P], F32, tag="xnT")
                        nc.scalar.copy(out=xnT[:], in_=xnT_ps[:])
                        d_ps = pp_mm.tile([P, 1], F32, tag="mm_ps")
                        nc.tensor.matmul(out=d_ps[:], lhsT=xnT[:], rhs=consts["whead"][:],
                                         start=True, stop=True)
                        nc.vector.tensor_tensor(out=dlst[:, gi, :], in0=d_ps[:],
                                                in1=consts["bh"][:], op=OP.add)
                    nc.gpsimd.dma_start(
                        dl_o[g0i * P:(g0i + G) * P, :].rearrange(
                            "(g p) f -> p g f", p=P),
                        dlst[:, :G, :])
    nc.compile()
    return nc


build_l23.g_list = None
build_l23.ln_trivial = False


# ---------------------------------------------------------------------------
# Runner
# ---------------------------------------------------------------------------

LAUNCH_TIMES_NS = []
TRACE_DIRS = []


def _run(nc, in_maps, cfg):
    backend = os.environ.get("HGT_BACKEND", "hw")
    if backend == "sim":
        from concourse.bass_interp import CoreSim
        results = []
        for m in in_maps:
            sim = CoreSim(nc, trace=False, require_finite=False, require_nnan=False)
            for k, v in m.items():
                sim.tensor(k)[:] = v
            sim.simulate(check_with_hw=False)
            out = {}
            for alloc in nc.m.functions[0].allocations:
                if isinstance(alloc, mybir.MemoryLocationSet) and alloc.kind == "ExternalOutput":
                    name = alloc.memorylocations[0].name
                    out[name] = sim.tensor(name).copy()
            results.append(out)
        return results
    else:
        from concourse.bass_utils import run_bass_kernel_spmd
        trace = os.environ.get("HGT_TRACE", "0") == "1"
        res = run_bass_kernel_spmd(nc, in_maps, core_ids=list(range(cfg.C)),
                                   trace=trace)
        if trace:
            LAUNCH_TIMES_NS.append(res.exec_time_ns)
            it = res.instructions_and_trace
            TRACE_DIRS.append(getattr(it, "trace_path", it))
        return res.results


# ---------------------------------------------------------------------------
# Main entry
# ---------------------------------------------------------------------------

def kernel(**inputs):
    cfg = Cfg()
    return _kernel_impl(cfg, inputs)


def _kernel_impl(cfg, inputs):
    c = cfg
    prm = prep_params(c, inputs)
    g = prep_graph(c, inputs)
    NT, tws = g["NT"], g["tws"]

    # ---- launch 1
    _kqv_tile.bias_zero = not np.asarray(inputs["b_kqv"])[0].any()
    nc1 = build_l1(c)
    in_maps = []
    xa = np.asarray(inputs["x_a"], np.float32)
    xw = np.asarray(inputs["x_w"], np.float32)
    xo = np.asarray(inputs["x_o"], np.float32)

    def padx(x, n, npad):
        out = np.zeros((npad, P), np.float32)
        out[:n, :64] = x
        return out

    for cc in range(c.C):
        in_maps.append({
            "xa": padx(xa[cc * c.nac:(cc + 1) * c.nac], c.nac, c.nap),
            "xw": padx(xw[cc * c.nwc:(cc + 1) * c.nwc], c.nwc, c.nwp),
            "xo": padx(xo[cc * c.noc:(cc + 1) * c.noc], c.noc, c.nop),
            "Win": prm["Win"], "bin": prm["bin"],
            "Wcat": prm["Wcat0"], "bcat": prm["bcat0"],
            "ident": prm["ident"],
        })
    r1 = _run(nc1, in_maps, c)

    def assemble_tables(res):
        """Build global kd table + per-core q' tables + global ve table."""
        kd_tab = np.empty((c.KD_ROWS, HID), np.float32)
        kd_tab[-1] = 1.0
        ve_tab = np.empty((c.VE_ROWS, HID), np.float32)
        ve_tab[-1] = 0.0
        qp_tabs = []
        for cc in range(c.C):
            kd = res[cc]["kd"]
            ve = res[cc]["ve"]
            # kd local [a|w|o] -> global
            kd_tab[c.KOFF[0] + cc * c.nac:c.KOFF[0] + (cc + 1) * c.nac] = kd[:c.nac]
            kd_tab[c.KOFF[1] + cc * c.nwc:c.KOFF[1] + (cc + 1) * c.nwc] = \
                kd[c.base_local[1]:c.base_local[1] + c.nwc]
            kd_tab[c.KOFF[2] + cc * c.noc:c.KOFF[2] + (cc + 1) * c.noc] = \
                kd[c.base_local[2]:c.base_local[2] + c.noc]
            # ve local slots [a-et2, a-et3, w-et0, o-et1] -> global stacked
            ve_tab[c.VOFF[2] + cc * c.nac:c.VOFF[2] + (cc + 1) * c.nac] = \
                ve[c.QB[0]:c.QB[0] + c.nac]
            ve_tab[c.VOFF[3] + cc * c.nac:c.VOFF[3] + (cc + 1) * c.nac] = \
                ve[c.QB[1]:c.QB[1] + c.nac]
            ve_tab[c.VOFF[0] + cc * c.nwc:c.VOFF[0] + (cc + 1) * c.nwc] = \
                ve[c.QB[2]:c.QB[2] + c.nwc]
            ve_tab[c.VOFF[1] + cc * c.noc:c.VOFF[1] + (cc + 1) * c.noc] = \
                ve[c.QB[3]:c.QB[3] + c.noc]
            # pad-edge q rows are -8.0: with pad k rows = 1.0 the pad alpha is
            # 128 * -8 = -1024 (bf16-safe), exp -> 0.
            qp = np.vstack([res[cc]["qp"], np.full((1, HID), -8.0, np.float32)])
            qp_tabs.append(qp)
        return kd_tab, ve_tab, qp_tabs

    import ml_dtypes
    bf16 = ml_dtypes.bfloat16

    # ---- launches 2 and 3
    # last launch only needs a-dst windows (head reads only a-type nodes)
    NT_a = int(tws[:c.ntile_a].sum())
    res = r1
    for l, last in ((1, False), (2, True)):
        kd_tab, ve_tab, qp_tabs = assemble_tables(res)
        lay = l - 1  # layer params index: launch2 -> layer 0, launch3 -> layer 1
        build_l23.g_list = prm[f"g{lay}"]
        build_l23.ln_trivial = bool(
            (np.asarray(inputs["ln_g"])[lay] == 1).all()
            and not np.asarray(inputs["ln_b"])[lay].any())
        _kqv_tile.bias_zero = (not last) and \
            not np.asarray(inputs["b_kqv"])[lay + 1].any()
        NT_l = NT_a if last else NT
        nc = build_l23(c, NT_l, tws, last)
        in_maps = []
        for cc in range(c.C):
            ti = g["tile_idx"][cc][:NT_l]  # [NT_l, P, 4]
            kg_s = np.ascontiguousarray(
                kd_tab[ti[:, :, 0]].transpose(1, 0, 2)).astype(bf16)
            vg_s = np.ascontiguousarray(
                ve_tab[ti[:, :, 1]].transpose(1, 0, 2)).astype(bf16)
            qg_s = np.ascontiguousarray(
                qp_tabs[cc][ti[:, :, 2]].transpose(1, 0, 2)).astype(bf16)
            dstlT = np.ascontiguousarray(ti[:, :, 3].T)
            m = {
                "kg": kg_s, "vg": vg_s, "qg": qg_s,
                "xsin": res[cc]["xs"][:c.nap] if last else res[cc]["xs"],
                "dstlT": dstlT,
                "iota": prm["iota"], "ident": prm["ident"],
                "Wo": prm[f"Wo{lay}"], "bo": prm[f"bo{lay}"],
                "lng": prm[f"lng{lay}"], "lnb": prm[f"lnb{lay}"],
                "gs": prm[f"g{lay}"],
            }
            if not last:
                m.update({"Wcat": prm[f"Wcat{lay + 1}"], "bcat": prm[f"bcat{lay + 1}"]})
            else:
                m.update({"whead": prm["whead"], "bh": prm["bh"]})
            in_maps.append(m)
        res = _run(nc, in_maps, c)

    out = np.concatenate([res[cc]["delta"][:c.nac, 0] for cc in range(c.C)])
    return out.astype(np.float32)



# revision 26
# speedup vs baseline: 3.6014x; 1.2070x over previous
"""HGT regressor on 8 Trainium2 NeuronCores (Bass/Tile).

Strategy (graph/data parallel, hint-following):
  - Nodes of each type are partitioned contiguously across the 8 cores
    (a: 12500/core, w: 2500/core, o: 6250/core). Each core owns the edges
    whose *destination* lies in its node shard.
  - Per layer, each core computes K = kqv[:, :128] (raw) and the per-edge-type
    source-side V transform (m_rel folded at source) plus the destination-side
    Q transform (a_rel * p_rel * scale folded into Q) for its own nodes only.
  - The full K / V_et tables are exchanged between layer launches via the host
    (replicated to all cores), i.e. host-mediated all-gather. Q' stays local.
  - Edge phase per core: edges sorted by local destination row, grouped into
    128-node windows; per 128-edge tile: indirect-DMA gathers of K[src],
    V_et[src], Q'_et[dst]; alpha = sum_h(K*Q'); ex = exp(alpha); payload
    [ex*V | ex] is scatter-added into a PSUM window accumulator via a
    one-hot matmul; windows flush densely to a numer/den table in DRAM.
  - Node phase per core: agg = numer/den, gelu, W_o, gated skip, LayerNorm,
    relu, then next-layer projections (or the scalar head in the last layer).
  - Softmax needs no running max: alpha = q'k with these parameter scales is
    O(1); exp cannot overflow, and softmax is shift-invariant anyway.
"""
import os
import sys

sys.path.insert(0, "/opt/trn_rl_repo")

import numpy as np

import concourse.bass as bass
import concourse.mybir as mybir
import concourse.tile as tile
from concourse import bacc

P = 128
H, D, HID = 4, 32, 128
PAY = HID + H  # 132
F32 = mybir.dt.float32
F16 = mybir.dt.float16
BF16 = mybir.dt.bfloat16
I32 = mybir.dt.int32
AF = mybir.ActivationFunctionType
OP = mybir.AluOpType


def _ceil(a, b):
    return (a + b - 1) * b // b if False else -(-a // b) * b


def cdiv(a, b):
    return -(-a // b)


class Cfg:
    """All sizes derived from problem scale; supports mini-scale testing."""

    def __init__(self, NA=100000, NWK=20000, NO=50000, E=150000, C=8):
        self.NA, self.NWK, self.NO, self.E, self.C = NA, NWK, NO, E, C
        assert NA % C == 0 and NWK % C == 0 and NO % C == 0
        self.nac, self.nwc, self.noc = NA // C, NWK // C, NO // C
        self.nap, self.nwp, self.nop = (
            cdiv(self.nac, P) * P,
            cdiv(self.nwc, P) * P,
            cdiv(self.noc, P) * P,
        )
        # local node-row layout (numer/xs/kd rows): [a | w | o], each padded
        self.base_local = (0, self.nap, self.nap + self.nwp)
        self.LOCN = self.nap + self.nwp + self.nop
        self.NWIN = self.LOCN // P
        # per-type tile counts
        self.ntile_a, self.ntile_w, self.ntile_o = (
            self.nap // P,
            self.nwp // P,
            self.nop // P,
        )
        # q' local layout: slots [a-et0, a-et1, w-et2, o-et3]
        self.QB = (0, self.nap, 2 * self.nap, 2 * self.nap + self.nwp)
        self.QTOT = 2 * self.nap + self.nwp + self.nop
        # ve local layout (same bases): slots [a-et2, a-et3, w-et0, o-et1]
        # global kd table layout: [a 0..NA | w | o] + trash
        self.KOFF = (0, NA, NA + NWK)
        self.KD_ROWS = NA + NWK + NO + 1
        # global stacked ve table: [et0 w | et1 o | et2 a | et3 a] + trash
        self.VOFF = (0, NWK, NWK + NO, NWK + NO + NA)
        self.VE_ROWS = NWK + NO + 2 * NA + 1


# edge types: (src_type, dst_type)
ETYPES = ((1, 0), (2, 0), (0, 1), (0, 2))


# ---------------------------------------------------------------------------
# Host-side preprocessing
# ---------------------------------------------------------------------------

def prep_graph(cfg, inputs):
    """Compute per-core edge tile indices. Shared across both layers.

    Returns dict with:
      NT: static tile count (same all cores)
      tws: [NWIN] tiles per window (static across cores)
      tile_idx: [C][NT, P, 4] int32  (kidx, vidx, qidx, dst_local)
    """
    c = cfg
    edges = []  # per et: (src, dst)
    for name_s, name_d in (("src_wa", "dst_wa"), ("src_oa", "dst_oa"),
                           ("src_aw", "dst_aw"), ("src_ao", "dst_ao")):
        edges.append((np.asarray(inputs[name_s]), np.asarray(inputs[name_d])))

    shard_n = (c.nac, c.nwc, c.noc)
    # concat all ets with global indices
    K_TRASH = c.KD_ROWS - 1
    V_TRASH = c.VE_ROWS - 1
    Q_TRASH = c.QTOT  # row appended by host to the q' table

    all_core = []
    koff_by_et = (c.KOFF[1], c.KOFF[2], c.KOFF[0], c.KOFF[0])  # src type offset in kd
    for et, (st, dt) in enumerate(ETYPES):
        src, dst = edges[et]
        kidx = koff_by_et[et] + src
        vidx = c.VOFF[et] + src
        core = dst // shard_n[dt]
        dloc = dst - core * shard_n[dt]  # dst index within its type shard
        # local numer row / q' row
        tb = (c.base_local[0], c.base_local[1], c.base_local[2])[dt]
        row = tb + dloc
        qslot = {0: 0, 1: 1, 2: 2, 3: 3}[et]
        qidx = c.QB[qslot] + dloc
        all_core.append((core, row, kidx, vidx, qidx))

    core_cat = np.concatenate([a[0] for a in all_core])
    row_cat = np.concatenate([a[1] for a in all_core])
    k_cat = np.concatenate([a[2] for a in all_core])
    v_cat = np.concatenate([a[3] for a in all_core])
    q_cat = np.concatenate([a[4] for a in all_core])

    # per-core, per-window edge counts -> static tile structure
    win_cat = row_cat // P
    counts = np.zeros((c.C, c.NWIN), np.int64)
    for cc in range(c.C):
        m = core_cat == cc
        counts[cc] = np.bincount(win_cat[m], minlength=c.NWIN)
    tws = np.maximum(cdiv(counts.max(axis=0), P), 1)  # >=1 tile per window
    NT = int(tws.sum())
    tile_base = np.zeros(c.NWIN, np.int64)
    tile_base[1:] = np.cumsum(tws)[:-1]

    tile_idx = np.zeros((c.C, NT, P, 4), np.int32)
    # fill pads with trash rows -> ex = 0 contributions
    tile_idx[:, :, :, 0] = K_TRASH
    tile_idx[:, :, :, 1] = V_TRASH
    tile_idx[:, :, :, 2] = Q_TRASH
    tile_idx[:, :, :, 3] = 0
    for cc in range(c.C):
        m = core_cat == cc
        rows = row_cat[m]
        order = np.argsort(rows, kind="stable")
        rows = rows[order]
        ks, vs, qs = k_cat[m][order], v_cat[m][order], q_cat[m][order]
        wins = rows // P
        dstl = rows % P
        # position within window
        wstart = np.searchsorted(wins, np.arange(c.NWIN), side="left")
        pos = np.arange(rows.size) - wstart[wins]
        slot_t = pos // P   # tile within window
        slot_p = pos % P    # partition
        gt = tile_base[wins] + slot_t  # global tile id
        tile_idx[cc, gt, slot_p, 0] = ks
        tile_idx[cc, gt, slot_p, 1] = vs
        tile_idx[cc, gt, slot_p, 2] = qs
        tile_idx[cc, gt, slot_p, 3] = dstl
    return {"NT": NT, "tws": tws.astype(np.int64), "tile_idx": tile_idx,
            "tile_base": tile_base}


def blockdiag(M):
    out = np.zeros((HID, HID), np.float32)
    for h in range(H):
        out[h * D:(h + 1) * D, h * D:(h + 1) * D] = M[h]
    return out


def prep_params(cfg, inputs):
    """Fold and lay out all parameters (host, tiny)."""
    scale = np.float32(1.0 / np.sqrt(D))
    a_rel = np.asarray(inputs["a_rel"])
    m_rel = np.asarray(inputs["m_rel"])
    p_rel = np.asarray(inputs["p_rel"])
    import ml_dtypes as _mld
    prm = {}
    rep = lambda v, w: np.broadcast_to(np.asarray(v, np.float32)[None, :], (P, w)).copy()
    for l in range(2):
        BDaT, BDm = [], []
        for et in range(4):
            a_eff = a_rel[l, et] * (p_rel[l, et] * scale)[:, None, None]
            BDaT.append(blockdiag(a_eff).T.copy())
            BDm.append(blockdiag(m_rel[l, et]))
        prm[f"BDaT{l}"] = np.stack(BDaT)  # [4,128,128]
        prm[f"BDm{l}"] = np.stack(BDm)
        prm[f"Wkqv{l}"] = np.asarray(inputs["W_kqv"])[l]       # [3,128,384]
        prm[f"bkqv{l}"] = np.stack([rep(np.asarray(inputs["b_kqv"])[l, t], 3 * HID) for t in range(3)])
        # folded projection weights: per type, slots [kd | q'_ets | v'_ets]
        # q' = xs @ (Wq @ BDaT_et), v' = xs @ (Wv @ BDm_et); biases likewise.
        import ml_dtypes
        q_ets = ((0, 1), (2,), (3,))
        v_ets = ((2, 3), (0,), (1,))
        Wcat = np.zeros((3, HID, 5 * HID), np.float32)
        bcat = np.zeros((3, P, 5 * HID), np.float32)
        for t in range(3):
            Wk = np.asarray(inputs["W_kqv"])[l, t]
            bk = np.asarray(inputs["b_kqv"])[l, t]
            cols = [Wk[:, :HID]]
            bs = [bk[:HID]]
            for et in q_ets[t]:
                cols.append(Wk[:, HID:2 * HID] @ prm[f"BDaT{l}"][et])
                bs.append(bk[HID:2 * HID] @ prm[f"BDaT{l}"][et])
            for et in v_ets[t]:
                cols.append(Wk[:, 2 * HID:] @ prm[f"BDm{l}"][et])
                bs.append(bk[2 * HID:] @ prm[f"BDm{l}"][et])
            cat = np.concatenate(cols, axis=1)
            Wcat[t, :, :cat.shape[1]] = cat
            bcat[t, :, :cat.shape[1]] = np.concatenate(bs)[None, :]
        prm[f"Wcat{l}"] = Wcat.astype(ml_dtypes.bfloat16)
        prm[f"bcat{l}"] = bcat
        prm[f"Wo{l}"] = np.asarray(inputs["W_o"])[l]           # [3,128,128]
        g = 1.0 / (1.0 + np.exp(-np.asarray(inputs["skip_p"], np.float64)))  # [2,3]
        prm[f"g{l}"] = g[l].astype(np.float32)
        prm[f"bo{l}"] = np.stack([rep(np.asarray(inputs["b_o"])[l, t] * g[l, t], HID) for t in range(3)])
        prm[f"lng{l}"] = np.stack([rep(np.asarray(inputs["ln_g"])[l, t], HID) for t in range(3)])
        prm[f"lnb{l}"] = np.stack([rep(np.asarray(inputs["ln_b"])[l, t], HID) for t in range(3)])
    # input proj, padded to 128 contraction
    W_in = np.asarray(inputs["W_in"])  # [3,64,128]
    Wp = np.zeros((3, 128, HID), np.float32)
    Wp[:, :64, :] = W_in
    prm["Win"] = Wp
    prm["bin"] = np.stack([rep(np.asarray(inputs["b_in"])[t], HID) for t in range(3)])
    prm["WinB"] = W_in.astype(np.float16)  # [3,64,128]
    prm["binT"] = np.ascontiguousarray(
        np.asarray(inputs["b_in"], np.float32)[:, :, None])  # [3,128,1]
    prm["whead"] = np.asarray(inputs["w_head"], np.float32)  # [128,1]
    prm["bh"] = np.full((P, 1), float(np.asarray(inputs["b_head"])[0] + np.asarray(inputs["base"])[0]), np.float32)
    prm["iota"] = np.broadcast_to(
        np.arange(128, dtype=np.float32).astype(_mld.bfloat16)[None, :],
        (P, 128)).copy()
    prm["ident"] = np.eye(128, dtype=np.float32)
    prm["identb"] = np.eye(128, dtype=np.float16)
    return prm


# ---------------------------------------------------------------------------
# Builders
# ---------------------------------------------------------------------------

_CONST_N = [0]


def _load_const(nc, cp, ap, shape, dtype=F32):
    _CONST_N[0] += 1
    t = cp.tile(list(shape), dtype, tag=f"cst{_CONST_N[0]}")
    nc.sync.dma_start(t[:], ap)
    return t


def _type_tiles(cfg):
    """Yield (t, i_t, r0) for all node tiles: type, tile-in-type, local row base."""
    out = []
    for t, (ntile, b) in enumerate(
        zip((cfg.ntile_a, cfg.ntile_w, cfg.ntile_o), cfg.base_local)
    ):
        for i in range(ntile):
            out.append((t, i, b + i * P))
    return out


def _kqv_width(t):
    """Staging width in slots for type t: [xs | kd | q'_ets | v'_ets]."""
    return 6 if t == 0 else 4


def _kqv_tile(nc, pools, cfg, consts, t, xs_tile, stg, gi):
    """Next-layer projections for one tile via folded weights.

    Writes kd/q'/v' (slots 1..) of stg[:, gi, :]; slot 0 (xs) is written by
    the caller. One transpose + 1-2 bf16 matmuls + 1-2 bias-adds.
    """
    wp, pp_t = pools["wp"], pools["pp_t"]
    xsT_ps = pp_t.tile([P, P], F32, tag="tp_ps")
    nc.tensor.transpose(out=xsT_ps[:], in_=xs_tile[:], identity=consts["ident"][:])
    xsT = wp.tile([P, P], BF16, tag="xsT")
    nc.scalar.copy(out=xsT[:], in_=xsT_ps[:])
    _kqv_mms(nc, pools, cfg, consts, t, xsT[:], stg, gi)


def _kqv_mms(nc, pools, cfg, consts, t, xsT_ap, stg, gi):
    pp_mm = pools["pp_mm"]
    xsT = xsT_ap
    W = consts["Wcat"][t]
    B = consts["bcat"][t]
    nw = (_kqv_width(t) - 1) * HID  # matmul output width (kd + q + v slots)
    n1 = min(nw, 4 * HID)
    ps1 = pp_mm.tile([P, 4 * HID], F32, tag="mmk_ps")
    nc.tensor.matmul(out=ps1[:, :n1], lhsT=xsT[:], rhs=W[:, :n1],
                     start=True, stop=True)
    if _kqv_tile.bias_zero:
        # biases are all zero: plain copy, on the (less busy) scalar engine
        nc.scalar.copy(out=stg[:, gi, HID:HID + n1], in_=ps1[:, :n1])
    else:
        nc.vector.tensor_tensor(out=stg[:, gi, HID:HID + n1], in0=ps1[:, :n1],
                                in1=B[:, :n1], op=OP.add)
    if nw > n1:
        ps2 = pp_mm.tile([P, HID], F32, tag="mm_ps")
        nc.tensor.matmul(out=ps2[:], lhsT=xsT[:], rhs=W[:, n1:nw],
                         start=True, stop=True)
        if _kqv_tile.bias_zero:
            nc.vector.tensor_copy(out=stg[:, gi, HID + n1:HID + nw], in_=ps2[:])
        else:
            nc.vector.tensor_tensor(out=stg[:, gi, HID + n1:HID + nw],
                                    in0=ps2[:], in1=B[:, n1:nw], op=OP.add)


_kqv_tile.bias_zero = False


def _kqv_flush(nc, cfg, outs, t, i0, G, stg, r0base, write_xs=True):
    """DMA a group of G tiles' staged [xs|kd|q|v] slots to their tables."""
    kd_o, qp_o, ve_o = outs["kd"], outs["qp"], outs["ve"]
    xs_o = outs.get("xs")
    rt0 = i0 * P

    def wr(dst, lo):
        nc.gpsimd.dma_start(dst.rearrange("(g p) f -> p g f", p=P),
                            stg[:, :, lo * HID:(lo + 1) * HID])

    if write_xs and xs_o is not None:
        wr(xs_o[r0base:r0base + G * P, :], 0)
    wr(kd_o[r0base:r0base + G * P, :], 1)
    q_slots = ((0, 1), (2,), (3,))[t]
    v_slots = ((0, 1), (2,), (3,))[t]
    for j, sl in enumerate(q_slots):
        wr(qp_o[cfg.QB[sl] + rt0:cfg.QB[sl] + rt0 + G * P, :], 2 + j)
    for j, sl in enumerate(v_slots):
        wr(ve_o[cfg.QB[sl] + rt0:cfg.QB[sl] + rt0 + G * P, :],
           2 + len(q_slots) + j)


def _type_groups(cfg, tiles, G=4):
    """Split the ordered tile list into same-type groups of <= G."""
    groups = []
    cur = []
    for tt in tiles:
        if cur and (tt[0] != cur[0][0] or len(cur) == G):
            groups.append(cur)
            cur = []
        cur.append(tt)
    if cur:
        groups.append(cur)
    return groups


def build_l1(cfg):
    """Launch 1: input proj + relu -> xs1; kqv chain -> kd/q'/ve tables."""
    nc = bacc.Bacc("TRN2", target_bir_lowering=False, debug=False,
                   num_devices=cfg.C)
    c = cfg
    xa = nc.dram_tensor("xta", [64, c.nap], F16, kind="ExternalInput").ap()
    xw = nc.dram_tensor("xtw", [64, c.nwp], F16, kind="ExternalInput").ap()
    xo = nc.dram_tensor("xto", [64, c.nop], F16, kind="ExternalInput").ap()
    Win = nc.dram_tensor("Win", [3, 64, HID], F16, kind="ExternalInput").ap()
    binp = nc.dram_tensor("binT", [3, P, 1], F32, kind="ExternalInput").ap()
    Wcat_d = nc.dram_tensor("Wcat", [3, P, 5 * HID], BF16, kind="ExternalInput").ap()
    bcat_d = nc.dram_tensor("bcat", [3, P, 5 * HID], F32, kind="ExternalInput").ap()
    ident_d = nc.dram_tensor("ident", [P, P], F32, kind="ExternalInput").ap()
    identb_d = nc.dram_tensor("identb", [P, P], F16, kind="ExternalInput").ap()

    xs_o = nc.dram_tensor("xs", [c.LOCN, P], F32, kind="ExternalOutput").ap()
    kd_o = nc.dram_tensor("kd", [c.LOCN, P], F32, kind="ExternalOutput").ap()
    qp_o = nc.dram_tensor("qp", [c.QTOT, P], F32, kind="ExternalOutput").ap()
    ve_o = nc.dram_tensor("ve", [c.QTOT, P], F32, kind="ExternalOutput").ap()

    xin = (xa, xw, xo)
    with tile.TileContext(nc) as tc:
        with tc.tile_pool(name="consts", bufs=1) as cp, \
             tc.tile_pool(name="work", bufs=4) as wp, \
             tc.tile_pool(name="stgp", bufs=2) as sgp, \
             tc.tile_pool(name="ppt", bufs=2, space="PSUM") as pp_t, \
             tc.tile_pool(name="ppmm", bufs=2, space="PSUM") as pp_mm:
            consts = {
                "ident": _load_const(nc, cp, ident_d[:, :], (P, P)),
                "identb": _load_const(nc, cp, identb_d[:, :], (P, P), F16),
                "Win": [_load_const(nc, cp, Win[t], (64, HID), F16) for t in range(3)],
                "bin": [_load_const(nc, cp, binp[t], (P, 1)) for t in range(3)],
                "Wcat": [_load_const(nc, cp, Wcat_d[t], (P, 5 * HID), BF16) for t in range(3)],
                "bcat": [_load_const(nc, cp, bcat_d[t], (P, 5 * HID)) for t in range(3)],
            }
            pools = {"cp": cp, "wp": wp, "pp_t": pp_t, "pp_mm": pp_mm}
            outs = {"kd": kd_o, "qp": qp_o, "ve": ve_o, "xs": xs_o}
            for grp in _type_groups(c, _type_tiles(c)):
                t = grp[0][0]
                G = len(grp)
                i0 = grp[0][1]
                stg = sgp.tile([P, G, _kqv_width(t) * HID], F32, tag="stg")
                # one projection matmul + one relu for the whole group:
                # xsT = relu(Win.T @ xT) directly feature-major (no input
                # transpose; xT comes pre-transposed from the host).
                xt = wp.tile([64, G * P], F16, tag="x_in")
                nc.sync.dma_start(xt[:], xin[t][:, i0 * P:(i0 + G) * P])
                pj_ps = pp_mm.tile([P, G, P], F32, tag="mmk_ps")
                nc.tensor.matmul(
                    out=pj_ps[:].rearrange("p g n -> p (g n)"),
                    lhsT=consts["Win"][t][:], rhs=xt[:],
                    start=True, stop=True)
                xsT_all = wp.tile([P, G, P], F16, tag="xsT_all")
                nc.scalar.activation(out=xsT_all[:], in_=pj_ps[:], func=AF.Relu,
                                     bias=consts["bin"][t][:, 0:1])
                for gi, (_, i_t, r0) in enumerate(grp):
                    xs_ps = pp_t.tile([P, P], F16, tag="tpb_ps")
                    nc.tensor.transpose(out=xs_ps[:], in_=xsT_all[:, gi, :],
                                        identity=consts["identb"][:])
                    nc.scalar.copy(out=stg[:, gi, :HID], in_=xs_ps[:])
                    _kqv_mms(nc, pools, c, consts, t, xsT_all[:, gi, :], stg, gi)
                _kqv_flush(nc, c, outs, t, grp[0][1], G, stg, grp[0][2])
    nc.compile()
    return nc


def build_l23(cfg, NT, tws, last):
    """Launches 2/3: edge phase + node phase (+ head if last)."""
    nc = bacc.Bacc("TRN2", target_bir_lowering=False, debug=False,
                   num_devices=cfg.C)
    c = cfg
    kg_d = nc.dram_tensor("kg", [P, NT, HID], BF16, kind="ExternalInput").ap()
    vg_d = nc.dram_tensor("vg", [P, NT, HID], BF16, kind="ExternalInput").ap()
    qg_d = nc.dram_tensor("qg", [P, NT, HID], BF16, kind="ExternalInput").ap()
    NWIN_E = c.ntile_a if last else c.NWIN   # only a-dst windows feed the head
    LOCN_E = c.nap if last else c.LOCN
    xs_in = nc.dram_tensor("xsin", [LOCN_E, P], F32, kind="ExternalInput").ap()
    ti_t = nc.dram_tensor("dstlT", [P, NT], BF16, kind="ExternalInput").ap()
    iota_d = nc.dram_tensor("iota", [P, P], BF16, kind="ExternalInput").ap()
    ident_d = nc.dram_tensor("ident", [P, P], F32, kind="ExternalInput").ap()
    Wo_d = nc.dram_tensor("Wo", [3, P, HID], F32, kind="ExternalInput").ap()
    bo_d = nc.dram_tensor("bo", [3, P, HID], F32, kind="ExternalInput").ap()
    lng_d = nc.dram_tensor("lng", [3, P, HID], F32, kind="ExternalInput").ap()
    lnb_d = nc.dram_tensor("lnb", [3, P, HID], F32, kind="ExternalInput").ap()
    gs_d = nc.dram_tensor("gs", [3], F32, kind="ExternalInput").ap()  # unused on-device; values baked via bo/g mul
    if not last:
        Wcat_d = nc.dram_tensor("Wcat", [3, P, 5 * HID], BF16, kind="ExternalInput").ap()
        bcat_d = nc.dram_tensor("bcat", [3, P, 5 * HID], F32, kind="ExternalInput").ap()
    else:
        wh_d = nc.dram_tensor("whead", [P, 1], F32, kind="ExternalInput").ap()
        bh_d = nc.dram_tensor("bh", [P, 1], F32, kind="ExternalInput").ap()

    if not last:
        xs_o = nc.dram_tensor("xs", [c.LOCN, P], F32, kind="ExternalOutput").ap()
        kd_o = nc.dram_tensor("kd", [c.LOCN, P], F32, kind="ExternalOutput").ap()
        qp_o = nc.dram_tensor("qp", [c.QTOT, P], F32, kind="ExternalOutput").ap()
        ve_o = nc.dram_tensor("ve", [c.QTOT, P], F32, kind="ExternalOutput").ap()
    else:
        dl_o = nc.dram_tensor("delta", [c.nap, 1], F32, kind="ExternalOutput").ap()

    # gains folded on host: bo tile already contains g*b_o. g itself baked as consts below.
    g_vals = None  # set in kernel() via attribute hack? no: pass via build arg
    g_list = build_l23.g_list  # [3] floats for this layer

    with tile.TileContext(nc) as tc:
        with tc.tile_pool(name="consts", bufs=1) as cp, \
             tc.tile_pool(name="idx", bufs=2) as idxp, \
             tc.tile_pool(name="gat", bufs=2) as gp, \
             tc.tile_pool(name="ework", bufs=2) as ewp, \
             tc.tile_pool(name="nwork", bufs=3) as wp, \
             tc.tile_pool(name="stgp", bufs=2) as sgp, \
             tc.tile_pool(name="small", bufs=4) as sp, \
             tc.tile_pool(name="flush", bufs=2) as fp, \
             tc.tile_pool(name="dram", bufs=1, space="DRAM") as dp, \
             tc.tile_pool(name="ppe", bufs=2, space="PSUM") as pp_e, \
             tc.tile_pool(name="ppt", bufs=2, space="PSUM") as pp_t, \
             tc.tile_pool(name="ppmm", bufs=2, space="PSUM") as pp_mm:

            numer = dp.tile([LOCN_E, PAY], F32)
            eps_t = cp.tile([P, 1], F32, tag="lneps")
            nc.vector.memset(eps_t[:], 1e-5)

            consts = {
                "iota": _load_const(nc, cp, iota_d[:, :], (P, P), BF16),
                "ident": _load_const(nc, cp, ident_d[:, :], (P, P)),
                "Wo": [_load_const(nc, cp, Wo_d[t], (P, HID)) for t in range(3)],
                "bo": [_load_const(nc, cp, bo_d[t], (P, HID)) for t in range(3)],
                "lng": [_load_const(nc, cp, lng_d[t], (P, HID)) for t in range(3)],
                "lnb": [_load_const(nc, cp, lnb_d[t], (P, HID)) for t in range(3)],
            }
            if not last:
                consts.update({
                    "Wcat": [_load_const(nc, cp, Wcat_d[t], (P, 5 * HID), BF16) for t in range(3)],
                    "bcat": [_load_const(nc, cp, bcat_d[t], (P, 5 * HID)) for t in range(3)],
                })
            else:
                consts["whead"] = _load_const(nc, cp, wh_d[:, :], (P, 1))
                consts["bh"] = _load_const(nc, cp, bh_d[:, :], (P, 1))

            # ---------------- edge phase ----------------
            # streams are host-pre-gathered per edge (bf16). Windows are
            # grouped greedily into supers (<= SUPER_T tiles); per super one
            # DMA per stream + one batched op per DVE stage; scatter stays a
            # per-tile bf16 one-hot matmul into the window's PSUM accumulator.
            SUPER_T = 20
            supers = []  # (g0, [T_w...], w0)
            gtile = 0
            w = 0
            while w < NWIN_E:
                g0 = gtile
                ts = []
                w0 = w
                while w < NWIN_E and len(ts) < 4 and \
                        sum(ts) + int(tws[w]) <= SUPER_T:
                    ts.append(int(tws[w]))
                    gtile += int(tws[w])
                    w += 1
                supers.append((g0, ts, w0))
            for g0, ts, w0 in supers:
                TS = sum(ts)
                kgt = gp.tile([P, TS, HID], BF16, tag="kgt")
                nc.sync.dma_start(kgt[:], kg_d[:, g0:g0 + TS, :])
                vgt = gp.tile([P, TS, HID], BF16, tag="vgt")
                nc.sync.dma_start(vgt[:], vg_d[:, g0:g0 + TS, :])
                qgt = gp.tile([P, TS, HID], BF16, tag="qgt")
                nc.scalar.dma_start(qgt[:], qg_d[:, g0:g0 + TS, :])
                dstl = idxp.tile([P, TS], BF16, tag="dstl")
                nc.scalar.dma_start(dstl[:], ti_t[:, g0:g0 + TS])

                prod = ewp.tile([P, TS, HID], BF16, tag="prod")
                nc.vector.tensor_tensor(out=prod[:], in0=kgt[:], in1=qgt[:],
                                        op=OP.mult)
                alpha = ewp.tile([P, TS, H], F32, tag="alpha")
                nc.vector.tensor_reduce(
                    out=alpha[:],
                    in_=prod[:].rearrange("p t (h d) -> p t h d", h=H),
                    axis=mybir.AxisListType.X, op=OP.add)
                payload = ewp.tile([P, TS, PAY], BF16, tag="payload")
                ex = payload[:, :, HID:HID + H]
                nc.scalar.activation(out=ex, in_=alpha[:], func=AF.Exp)
                nc.vector.tensor_tensor(
                    out=payload[:, :, :HID].rearrange("p t (h d) -> p t h d", h=H),
                    in0=vgt[:].rearrange("p t (h d) -> p t h d", h=H),
                    in1=ex[:, :, :, None].to_broadcast([P, TS, H, D]),
                    op=OP.mult)
                onehot = ewp.tile([P, TS, P], BF16, tag="onehot")
                nc.vector.tensor_tensor(
                    out=onehot[:],
                    in0=dstl[:, :, None].to_broadcast([P, TS, P]),
                    in1=consts["iota"][:, None, :].to_broadcast([P, TS, P]),
                    op=OP.is_equal)
                fl = fp.tile([P, len(ts), PAY], F32, tag="fl")
                toff = 0
                for wi, T in enumerate(ts):
                    psum_w = pp_e.tile([P, PAY], F32, tag="psw")
                    for t in range(toff, toff + T):
                        nc.tensor.matmul(out=psum_w[:], lhsT=onehot[:, t, :],
                                         rhs=payload[:, t, :],
                                         start=(t == toff),
                                         stop=(t == toff + T - 1))
                    nc.vector.tensor_copy(out=fl[:, wi, :], in_=psum_w[:])
                    toff += T
                nc.sync.dma_start(
                    numer[w0 * P:(w0 + len(ts)) * P, :].rearrange(
                        "(t p) f -> p t f", p=P),
                    fl[:])

            # ---------------- node phase ----------------
            pools = {"cp": cp, "wp": wp, "pp_t": pp_t, "pp_mm": pp_mm}
            outs = None if last else {"kd": kd_o, "qp": qp_o, "ve": ve_o, "xs": xs_o}
            tiles = [x for x in _type_tiles(c) if (not last) or x[0] == 0]
            NTL = len(tiles)
            # pass 1: all tiles up through the skip-add + LN stats; gelu is the
            # only table-based ACT function here so the scalar engine loads
            # the gelu table once instead of thrashing gelu<->sqrt per tile.
            o1_all = cp.tile([P, NTL, HID], F16, tag="o1_all")
            mv_all = cp.tile([P, NTL, 2], F32, tag="mv_all")
            gi0 = 0
            for grp in _type_groups(c, tiles):
                t = grp[0][0]
                G = len(grp)
                r00 = grp[0][2]
                nm4 = wp.tile([P, G, PAY], F32, tag="nm")
                nc.scalar.dma_start(
                    nm4[:],
                    numer[r00:r00 + G * P, :].rearrange("(g p) f -> p g f", p=P))
                xs4 = wp.tile([P, G, HID], F32, tag="xs_ld")
                nc.scalar.dma_start(
                    xs4[:],
                    xs_in[r00:r00 + G * P, :].rearrange("(g p) f -> p g f", p=P))
                den = sp.tile([P, G, H], F32, tag="den")
                nc.vector.tensor_scalar_add(den[:], nm4[:, :, HID:HID + H], 1e-16)
                rec = sp.tile([P, G, H], F32, tag="rec")
                nc.vector.reciprocal(rec[:], den[:])
                agg = wp.tile([P, G, HID], F32, tag="agg")
                nc.vector.tensor_tensor(
                    out=agg[:].rearrange("p g (h d) -> p g h d", h=H),
                    in0=nm4[:, :, :HID].rearrange("p g (h d) -> p g h d", h=H),
                    in1=rec[:, :, :, None].to_broadcast([P, G, H, D]),
                    op=OP.mult)
                glu = wp.tile([P, G, HID], F32, tag="glu")
                if os.environ.get("HGT_BACKEND", "hw") == "sim":
                    # CoreSim has no Gelu LUT: tanh approximation (dev only)
                    t1 = wp.tile([P, G, HID], F32, tag="gelu_t1")
                    nc.vector.tensor_tensor(out=t1[:], in0=agg[:], in1=agg[:], op=OP.mult)
                    nc.vector.tensor_tensor(out=t1[:], in0=t1[:], in1=agg[:], op=OP.mult)
                    nc.vector.tensor_scalar(out=t1[:], in0=t1[:], scalar1=0.044715,
                                            scalar2=None, op0=OP.mult)
                    nc.vector.tensor_tensor(out=t1[:], in0=t1[:], in1=agg[:], op=OP.add)
                    nc.scalar.activation(out=t1[:], in_=t1[:], func=AF.Tanh,
                                         scale=0.7978845608028654)
                    nc.vector.tensor_scalar(out=t1[:], in0=t1[:], scalar1=0.5,
                                            scalar2=0.5, op0=OP.mult, op1=OP.add)
                    nc.vector.tensor_tensor(out=glu[:], in0=t1[:], in1=agg[:], op=OP.mult)
                else:
                    nc.scalar.activation(out=glu[:], in_=agg[:], func=AF.Gelu)
                ops4 = pp_mm.tile([P, G, HID], F32, tag="mmk_ps")
                for gi in range(G):
                    gluT_ps = pp_t.tile([P, P], F32, tag="tp_ps")
                    nc.tensor.transpose(out=gluT_ps[:], in_=glu[:, gi, :],
                                        identity=consts["ident"][:])
                    gluT = wp.tile([P, P], F32, tag="gluT")
                    nc.scalar.copy(out=gluT[:], in_=gluT_ps[:])
                    nc.tensor.matmul(out=ops4[:, gi, :], lhsT=gluT[:],
                                     rhs=consts["Wo"][t][:],
                                     start=True, stop=True)
                # o3 = g*o + (g*b_o) + (1-g)*xs  (bo const already has g*b_o)
                o1g = o1_all[:, gi0:gi0 + G, :]
                nc.vector.tensor_scalar_mul(o1g, ops4[:], float(g_list[t]))
                if not build_l23.bo_trivial:
                    nc.vector.tensor_tensor(
                        out=o1g, in0=o1g,
                        in1=consts["bo"][t][:, None, :].to_broadcast([P, G, HID]),
                        op=OP.add)
                xs_s = wp.tile([P, G, HID], F32, tag="xs_s")
                nc.vector.tensor_scalar_mul(xs_s[:], xs4[:], float(1.0 - g_list[t]))
                nc.vector.tensor_tensor(out=o1g, in0=o1g, in1=xs_s[:], op=OP.add)
                for gi in range(G):
                    stats = sp.tile([P, nc.vector.BN_STATS_DIM], F32, tag="stats")
                    nc.vector.bn_stats(out=stats[:], in_=o1_all[:, gi0 + gi, :])
                    nc.vector.bn_aggr(out=mv_all[:, gi0 + gi, :], in_=stats[:])
                gi0 += G
            # one batched sqrt for all tiles' variances (single table load)
            rstd_all = cp.tile([P, NTL], F32, tag="rstd_all")
            nc.scalar.activation(out=rstd_all[:], in_=mv_all[:, :, 1],
                                 func=AF.Sqrt,
                                 bias=eps_t[:, 0:1])
            nc.vector.reciprocal(rstd_all[:], rstd_all[:])
            # pass 2: normalize + relu + next-layer projections (relu/copy are
            # in every ACT table set, so no further table switches).
            def _xh_relu(i, t, dst_ap):
                xh = wp.tile([P, HID], F32, tag="xh")
                nc.vector.tensor_scalar(
                    out=xh[:], in0=o1_all[:, i, :], scalar1=mv_all[:, i, 0:1],
                    scalar2=rstd_all[:, i:i + 1],
                    op0=OP.subtract, op1=OP.mult)
                if not build_l23.ln_trivial:
                    nc.vector.tensor_tensor(out=xh[:], in0=xh[:], in1=consts["lng"][t][:], op=OP.mult)
                    nc.vector.tensor_tensor(out=xh[:], in0=xh[:], in1=consts["lnb"][t][:], op=OP.add)
                nc.scalar.activation(out=dst_ap, in_=xh[:], func=AF.Relu)

            if not last:
                gi0 = 0
                for grp in _type_groups(c, tiles):
                    t = grp[0][0]
                    G = len(grp)
                    stg = sgp.tile([P, G, _kqv_width(t) * HID], F32, tag="stg")
                    for gi, (_, i_t, r0) in enumerate(grp):
                        _xh_relu(gi0 + gi, t, stg[:, gi, :HID])
                        _kqv_tile(nc, pools, c, consts, t, stg[:, gi, :HID], stg, gi)
                    _kqv_flush(nc, c, outs, t, grp[0][1], G, stg, grp[0][2])
                    gi0 += G
            else:
                for g0i in range(0, len(tiles), 4):
                    grp = tiles[g0i:g0i + 4]
                    G = len(grp)
                    dlst = sp.tile([P, 4, 1], F32, tag="dlst")
                    for gi, (t, i_t, r0) in enumerate(grp):
                        xs_new = wp.tile([P, HID], F32, tag="xs_new")
                        _xh_relu(g0i + gi, t, xs_new[:])
                        xnT_ps = pp_t.tile([P, P], F32, tag="tp_ps")
                        nc.tensor.transpose(out=xnT_ps[:], in_=xs_new[:], identity=consts["ident"][:])
                        xnT = wp.tile([P, P], F32, tag="xnT")
                        nc.scalar.copy(out=xnT[:], in_=xnT_ps[:])
                        d_ps = pp_mm.tile([P, 1], F32, tag="mm_ps")
                        nc.tensor.matmul(out=d_ps[:], lhsT=xnT[:], rhs=consts["whead"][:],
                                         start=True, stop=True)
                        nc.vector.tensor_tensor(out=dlst[:, gi, :], in0=d_ps[:],
                                                in1=consts["bh"][:], op=OP.add)
                    nc.gpsimd.dma_start(
                        dl_o[g0i * P:(g0i + G) * P, :].rearrange(
                            "(g p) f -> p g f", p=P),
                        dlst[:, :G, :])
    nc.compile()
    return nc


build_l23.g_list = None
build_l23.ln_trivial = False
build_l23.bo_trivial = False


# ---------------------------------------------------------------------------
# Runner
# ---------------------------------------------------------------------------

LAUNCH_TIMES_NS = []
TRACE_DIRS = []


def _run(nc, in_maps, cfg):
    backend = os.environ.get("HGT_BACKEND", "hw")
    if backend == "sim":
        from concourse.bass_interp import CoreSim
        results = []
        for m in in_maps:
            sim = CoreSim(nc, trace=False, require_finite=False, require_nnan=False)
            for k, v in m.items():
                sim.tensor(k)[:] = v
            sim.simulate(check_with_hw=False)
            out = {}
            for alloc in nc.m.functions[0].allocations:
                if isinstance(alloc, mybir.MemoryLocationSet) and alloc.kind == "ExternalOutput":
                    name = alloc.memorylocations[0].name
                    out[name] = sim.tensor(name).copy()
            results.append(out)
        return results
    else:
        from concourse.bass_utils import run_bass_kernel_spmd
        trace = os.environ.get("HGT_TRACE", "0") == "1"
        res = run_bass_kernel_spmd(nc, in_maps, core_ids=list(range(cfg.C)),
                                   trace=trace)
        if trace:
            LAUNCH_TIMES_NS.append(res.exec_time_ns)
            it = res.instructions_and_trace
            TRACE_DIRS.append(getattr(it, "trace_path", it))
        return res.results


# ---------------------------------------------------------------------------
# Main entry
# ---------------------------------------------------------------------------

def kernel(**inputs):
    cfg = Cfg()
    return _kernel_impl(cfg, inputs)


def _kernel_impl(cfg, inputs):
    c = cfg
    prm = prep_params(c, inputs)
    g = prep_graph(c, inputs)
    NT, tws = g["NT"], g["tws"]

    # ---- launch 1
    _kqv_tile.bias_zero = not np.asarray(inputs["b_kqv"])[0].any()
    nc1 = build_l1(c)
    in_maps = []
    xa = np.asarray(inputs["x_a"], np.float32)
    xw = np.asarray(inputs["x_w"], np.float32)
    xo = np.asarray(inputs["x_o"], np.float32)

    import ml_dtypes as _mld

    def padxT(x, n, npad):
        out = np.zeros((64, npad), np.float32)
        out[:, :n] = x.T
        return out.astype(np.float16)

    for cc in range(c.C):
        in_maps.append({
            "xta": padxT(xa[cc * c.nac:(cc + 1) * c.nac], c.nac, c.nap),
            "xtw": padxT(xw[cc * c.nwc:(cc + 1) * c.nwc], c.nwc, c.nwp),
            "xto": padxT(xo[cc * c.noc:(cc + 1) * c.noc], c.noc, c.nop),
            "Win": prm["WinB"], "binT": prm["binT"],
            "identb": prm["identb"],
            "Wcat": prm["Wcat0"], "bcat": prm["bcat0"],
            "ident": prm["ident"],
        })
    r1 = _run(nc1, in_maps, c)

    def assemble_tables(res):
        """Build global kd table + per-core q' tables + global ve table."""
        kd_tab = np.empty((c.KD_ROWS, HID), np.float32)
        kd_tab[-1] = 1.0
        ve_tab = np.empty((c.VE_ROWS, HID), np.float32)
        ve_tab[-1] = 0.0
        qp_tabs = []
        for cc in range(c.C):
            kd = res[cc]["kd"]
            ve = res[cc]["ve"]
            # kd local [a|w|o] -> global
            kd_tab[c.KOFF[0] + cc * c.nac:c.KOFF[0] + (cc + 1) * c.nac] = kd[:c.nac]
            kd_tab[c.KOFF[1] + cc * c.nwc:c.KOFF[1] + (cc + 1) * c.nwc] = \
                kd[c.base_local[1]:c.base_local[1] + c.nwc]
            kd_tab[c.KOFF[2] + cc * c.noc:c.KOFF[2] + (cc + 1) * c.noc] = \
                kd[c.base_local[2]:c.base_local[2] + c.noc]
            # ve local slots [a-et2, a-et3, w-et0, o-et1] -> global stacked
            ve_tab[c.VOFF[2] + cc * c.nac:c.VOFF[2] + (cc + 1) * c.nac] = \
                ve[c.QB[0]:c.QB[0] + c.nac]
            ve_tab[c.VOFF[3] + cc * c.nac:c.VOFF[3] + (cc + 1) * c.nac] = \
                ve[c.QB[1]:c.QB[1] + c.nac]
            ve_tab[c.VOFF[0] + cc * c.nwc:c.VOFF[0] + (cc + 1) * c.nwc] = \
                ve[c.QB[2]:c.QB[2] + c.nwc]
            ve_tab[c.VOFF[1] + cc * c.noc:c.VOFF[1] + (cc + 1) * c.noc] = \
                ve[c.QB[3]:c.QB[3] + c.noc]
            # pad-edge q rows are -8.0: with pad k rows = 1.0 the pad alpha is
            # 128 * -8 = -1024 (bf16-safe), exp -> 0.
            qp = np.vstack([res[cc]["qp"], np.full((1, HID), -8.0, np.float32)])
            qp_tabs.append(qp)
        return kd_tab, ve_tab, qp_tabs

    import ml_dtypes
    bf16 = ml_dtypes.bfloat16

    # ---- launches 2 and 3
    # last launch only needs a-dst windows (head reads only a-type nodes)
    NT_a = int(tws[:c.ntile_a].sum())
    res = r1
    for l, last in ((1, False), (2, True)):
        kd_tab, ve_tab, qp_tabs = assemble_tables(res)
        lay = l - 1  # layer params index: launch2 -> layer 0, launch3 -> layer 1
        build_l23.g_list = prm[f"g{lay}"]
        build_l23.bo_trivial = not np.asarray(inputs["b_o"])[lay].any()
        build_l23.ln_trivial = bool(
            (np.asarray(inputs["ln_g"])[lay] == 1).all()
            and not np.asarray(inputs["ln_b"])[lay].any())
        _kqv_tile.bias_zero = (not last) and \
            not np.asarray(inputs["b_kqv"])[lay + 1].any()
        NT_l = NT_a if last else NT
        nc = build_l23(c, NT_l, tws, last)
        in_maps = []
        for cc in range(c.C):
            ti = g["tile_idx"][cc][:NT_l]  # [NT_l, P, 4]
            kg_s = np.ascontiguousarray(
                kd_tab[ti[:, :, 0]].transpose(1, 0, 2)).astype(bf16)
            vg_s = np.ascontiguousarray(
                ve_tab[ti[:, :, 1]].transpose(1, 0, 2)).astype(bf16)
            qg_s = np.ascontiguousarray(
                qp_tabs[cc][ti[:, :, 2]].transpose(1, 0, 2)).astype(bf16)
            dstlT = np.ascontiguousarray(ti[:, :, 3].T.astype(bf16))
            m = {
                "kg": kg_s, "vg": vg_s, "qg": qg_s,
                "xsin": res[cc]["xs"][:c.nap] if last else res[cc]["xs"],
                "dstlT": dstlT,
                "iota": prm["iota"], "ident": prm["ident"],
                "Wo": prm[f"Wo{lay}"], "bo": prm[f"bo{lay}"],
                "lng": prm[f"lng{lay}"], "lnb": prm[f"lnb{lay}"],
                "gs": prm[f"g{lay}"],
            }
            if not last:
                m.update({"Wcat": prm[f"Wcat{lay + 1}"], "bcat": prm[f"bcat{lay + 1}"]})
            else:
                m.update({"whead": prm["whead"], "bh": prm["bh"]})
            in_maps.append(m)
        res = _run(nc, in_maps, c)

    out = np.concatenate([res[cc]["delta"][:c.nac, 0] for cc in range(c.C)])
    return out.astype(np.float32)



# revision 27
# speedup vs baseline: 3.6320x; 1.0085x over previous
"""HGT regressor on 8 Trainium2 NeuronCores (Bass/Tile).

Strategy (graph/data parallel, hint-following):
  - Nodes of each type are partitioned contiguously across the 8 cores
    (a: 12500/core, w: 2500/core, o: 6250/core). Each core owns the edges
    whose *destination* lies in its node shard.
  - Per layer, each core computes K = kqv[:, :128] (raw) and the per-edge-type
    source-side V transform (m_rel folded at source) plus the destination-side
    Q transform (a_rel * p_rel * scale folded into Q) for its own nodes only.
  - The full K / V_et tables are exchanged between layer launches via the host
    (replicated to all cores), i.e. host-mediated all-gather. Q' stays local.
  - Edge phase per core: edges sorted by local destination row, grouped into
    128-node windows; per 128-edge tile: indirect-DMA gathers of K[src],
    V_et[src], Q'_et[dst]; alpha = sum_h(K*Q'); ex = exp(alpha); payload
    [ex*V | ex] is scatter-added into a PSUM window accumulator via a
    one-hot matmul; windows flush densely to a numer/den table in DRAM.
  - Node phase per core: agg = numer/den, gelu, W_o, gated skip, LayerNorm,
    relu, then next-layer projections (or the scalar head in the last layer).
  - Softmax needs no running max: alpha = q'k with these parameter scales is
    O(1); exp cannot overflow, and softmax is shift-invariant anyway.
"""
import os
import sys

sys.path.insert(0, "/opt/trn_rl_repo")

import numpy as np

import concourse.bass as bass
import concourse.mybir as mybir
import concourse.tile as tile
from concourse import bacc

P = 128
H, D, HID = 4, 32, 128
PAY = HID + H  # 132
F32 = mybir.dt.float32
F16 = mybir.dt.float16
BF16 = mybir.dt.bfloat16
I32 = mybir.dt.int32
AF = mybir.ActivationFunctionType
OP = mybir.AluOpType


def _ceil(a, b):
    return (a + b - 1) * b // b if False else -(-a // b) * b


def cdiv(a, b):
    return -(-a // b)


class Cfg:
    """All sizes derived from problem scale; supports mini-scale testing."""

    def __init__(self, NA=100000, NWK=20000, NO=50000, E=150000, C=8):
        self.NA, self.NWK, self.NO, self.E, self.C = NA, NWK, NO, E, C
        assert NA % C == 0 and NWK % C == 0 and NO % C == 0
        self.nac, self.nwc, self.noc = NA // C, NWK // C, NO // C
        self.nap, self.nwp, self.nop = (
            cdiv(self.nac, P) * P,
            cdiv(self.nwc, P) * P,
            cdiv(self.noc, P) * P,
        )
        # local node-row layout (numer/xs/kd rows): [a | w | o], each padded
        self.base_local = (0, self.nap, self.nap + self.nwp)
        self.LOCN = self.nap + self.nwp + self.nop
        self.NWIN = self.LOCN // P
        # per-type tile counts
        self.ntile_a, self.ntile_w, self.ntile_o = (
            self.nap // P,
            self.nwp // P,
            self.nop // P,
        )
        # q' local layout: slots [a-et0, a-et1, w-et2, o-et3]
        self.QB = (0, self.nap, 2 * self.nap, 2 * self.nap + self.nwp)
        self.QTOT = 2 * self.nap + self.nwp + self.nop
        # ve local layout (same bases): slots [a-et2, a-et3, w-et0, o-et1]
        # global kd table layout: [a 0..NA | w | o] + trash
        self.KOFF = (0, NA, NA + NWK)
        self.KD_ROWS = NA + NWK + NO + 1
        # global stacked ve table: [et0 w | et1 o | et2 a | et3 a] + trash
        self.VOFF = (0, NWK, NWK + NO, NWK + NO + NA)
        self.VE_ROWS = NWK + NO + 2 * NA + 1


# edge types: (src_type, dst_type)
ETYPES = ((1, 0), (2, 0), (0, 1), (0, 2))


# ---------------------------------------------------------------------------
# Host-side preprocessing
# ---------------------------------------------------------------------------

def prep_graph(cfg, inputs):
    """Compute per-core edge tile indices. Shared across both layers.

    Returns dict with:
      NT: static tile count (same all cores)
      tws: [NWIN] tiles per window (static across cores)
      tile_idx: [C][NT, P, 4] int32  (kidx, vidx, qidx, dst_local)
    """
    c = cfg
    edges = []  # per et: (src, dst)
    for name_s, name_d in (("src_wa", "dst_wa"), ("src_oa", "dst_oa"),
                           ("src_aw", "dst_aw"), ("src_ao", "dst_ao")):
        edges.append((np.asarray(inputs[name_s]), np.asarray(inputs[name_d])))

    shard_n = (c.nac, c.nwc, c.noc)
    # concat all ets with global indices
    K_TRASH = c.KD_ROWS - 1
    V_TRASH = c.VE_ROWS - 1
    Q_TRASH = c.QTOT  # row appended by host to the q' table

    all_core = []
    koff_by_et = (c.KOFF[1], c.KOFF[2], c.KOFF[0], c.KOFF[0])  # src type offset in kd
    for et, (st, dt) in enumerate(ETYPES):
        src, dst = edges[et]
        kidx = koff_by_et[et] + src
        vidx = c.VOFF[et] + src
        core = dst // shard_n[dt]
        dloc = dst - core * shard_n[dt]  # dst index within its type shard
        # local numer row / q' row
        tb = (c.base_local[0], c.base_local[1], c.base_local[2])[dt]
        row = tb + dloc
        qslot = {0: 0, 1: 1, 2: 2, 3: 3}[et]
        qidx = c.QB[qslot] + dloc
        all_core.append((core, row, kidx, vidx, qidx))

    core_cat = np.concatenate([a[0] for a in all_core])
    row_cat = np.concatenate([a[1] for a in all_core])
    k_cat = np.concatenate([a[2] for a in all_core])
    v_cat = np.concatenate([a[3] for a in all_core])
    q_cat = np.concatenate([a[4] for a in all_core])

    # per-core, per-window edge counts -> static tile structure
    win_cat = row_cat // P
    counts = np.zeros((c.C, c.NWIN), np.int64)
    for cc in range(c.C):
        m = core_cat == cc
        counts[cc] = np.bincount(win_cat[m], minlength=c.NWIN)
    tws = np.maximum(cdiv(counts.max(axis=0), P), 1)  # >=1 tile per window
    NT = int(tws.sum())
    tile_base = np.zeros(c.NWIN, np.int64)
    tile_base[1:] = np.cumsum(tws)[:-1]

    tile_idx = np.zeros((c.C, NT, P, 4), np.int32)
    # fill pads with trash rows -> ex = 0 contributions
    tile_idx[:, :, :, 0] = K_TRASH
    tile_idx[:, :, :, 1] = V_TRASH
    tile_idx[:, :, :, 2] = Q_TRASH
    tile_idx[:, :, :, 3] = 0
    for cc in range(c.C):
        m = core_cat == cc
        rows = row_cat[m]
        order = np.argsort(rows, kind="stable")
        rows = rows[order]
        ks, vs, qs = k_cat[m][order], v_cat[m][order], q_cat[m][order]
        wins = rows // P
        dstl = rows % P
        # position within window
        wstart = np.searchsorted(wins, np.arange(c.NWIN), side="left")
        pos = np.arange(rows.size) - wstart[wins]
        slot_t = pos // P   # tile within window
        slot_p = pos % P    # partition
        gt = tile_base[wins] + slot_t  # global tile id
        tile_idx[cc, gt, slot_p, 0] = ks
        tile_idx[cc, gt, slot_p, 1] = vs
        tile_idx[cc, gt, slot_p, 2] = qs
        tile_idx[cc, gt, slot_p, 3] = dstl
    return {"NT": NT, "tws": tws.astype(np.int64), "tile_idx": tile_idx,
            "tile_base": tile_base}


def blockdiag(M):
    out = np.zeros((HID, HID), np.float32)
    for h in range(H):
        out[h * D:(h + 1) * D, h * D:(h + 1) * D] = M[h]
    return out


def prep_params(cfg, inputs):
    """Fold and lay out all parameters (host, tiny)."""
    scale = np.float32(1.0 / np.sqrt(D))
    a_rel = np.asarray(inputs["a_rel"])
    m_rel = np.asarray(inputs["m_rel"])
    p_rel = np.asarray(inputs["p_rel"])
    import ml_dtypes as _mld
    prm = {}
    rep = lambda v, w: np.broadcast_to(np.asarray(v, np.float32)[None, :], (P, w)).copy()
    for l in range(2):
        BDaT, BDm = [], []
        for et in range(4):
            a_eff = a_rel[l, et] * (p_rel[l, et] * scale)[:, None, None]
            BDaT.append(blockdiag(a_eff).T.copy())
            BDm.append(blockdiag(m_rel[l, et]))
        prm[f"BDaT{l}"] = np.stack(BDaT)  # [4,128,128]
        prm[f"BDm{l}"] = np.stack(BDm)
        prm[f"Wkqv{l}"] = np.asarray(inputs["W_kqv"])[l]       # [3,128,384]
        prm[f"bkqv{l}"] = np.stack([rep(np.asarray(inputs["b_kqv"])[l, t], 3 * HID) for t in range(3)])
        # folded projection weights: per type, slots [kd | q'_ets | v'_ets]
        # q' = xs @ (Wq @ BDaT_et), v' = xs @ (Wv @ BDm_et); biases likewise.
        import ml_dtypes
        q_ets = ((0, 1), (2,), (3,))
        v_ets = ((2, 3), (0,), (1,))
        Wcat = np.zeros((3, HID, 5 * HID), np.float32)
        bcat = np.zeros((3, P, 5 * HID), np.float32)
        for t in range(3):
            Wk = np.asarray(inputs["W_kqv"])[l, t]
            bk = np.asarray(inputs["b_kqv"])[l, t]
            cols = [Wk[:, :HID]]
            bs = [bk[:HID]]
            for et in q_ets[t]:
                cols.append(Wk[:, HID:2 * HID] @ prm[f"BDaT{l}"][et])
                bs.append(bk[HID:2 * HID] @ prm[f"BDaT{l}"][et])
            for et in v_ets[t]:
                cols.append(Wk[:, 2 * HID:] @ prm[f"BDm{l}"][et])
                bs.append(bk[2 * HID:] @ prm[f"BDm{l}"][et])
            cat = np.concatenate(cols, axis=1)
            Wcat[t, :, :cat.shape[1]] = cat
            bcat[t, :, :cat.shape[1]] = np.concatenate(bs)[None, :]
        prm[f"Wcat{l}"] = Wcat.astype(ml_dtypes.bfloat16)
        prm[f"bcat{l}"] = bcat
        g = 1.0 / (1.0 + np.exp(-np.asarray(inputs["skip_p"], np.float64)))  # [2,3]
        # skip gain g folded into Wo: node phase computes o1 = (glu @ g*Wo) + (1-g)*xs
        prm[f"Wo{l}"] = (np.asarray(inputs["W_o"])[l]
                         * g[l].astype(np.float32)[:, None, None])  # [3,128,128]
        prm[f"g{l}"] = g[l].astype(np.float32)
        prm[f"bo{l}"] = np.stack([rep(np.asarray(inputs["b_o"])[l, t] * g[l, t], HID) for t in range(3)])
        prm[f"lng{l}"] = np.stack([rep(np.asarray(inputs["ln_g"])[l, t], HID) for t in range(3)])
        prm[f"lnb{l}"] = np.stack([rep(np.asarray(inputs["ln_b"])[l, t], HID) for t in range(3)])
    # input proj, padded to 128 contraction
    W_in = np.asarray(inputs["W_in"])  # [3,64,128]
    Wp = np.zeros((3, 128, HID), np.float32)
    Wp[:, :64, :] = W_in
    prm["Win"] = Wp
    prm["bin"] = np.stack([rep(np.asarray(inputs["b_in"])[t], HID) for t in range(3)])
    prm["WinB"] = W_in.astype(np.float16)  # [3,64,128]
    prm["binT"] = np.ascontiguousarray(
        np.asarray(inputs["b_in"], np.float32)[:, :, None])  # [3,128,1]
    prm["whead"] = np.asarray(inputs["w_head"], np.float32)  # [128,1]
    prm["bh"] = np.full((P, 1), float(np.asarray(inputs["b_head"])[0] + np.asarray(inputs["base"])[0]), np.float32)
    prm["iota"] = np.broadcast_to(
        np.arange(128, dtype=np.float32).astype(_mld.bfloat16)[None, :],
        (P, 128)).copy()
    prm["ident"] = np.eye(128, dtype=np.float32)
    prm["identb"] = np.eye(128, dtype=np.float16)
    return prm


# ---------------------------------------------------------------------------
# Builders
# ---------------------------------------------------------------------------

_CONST_N = [0]


def _load_const(nc, cp, ap, shape, dtype=F32):
    _CONST_N[0] += 1
    t = cp.tile(list(shape), dtype, tag=f"cst{_CONST_N[0]}")
    nc.sync.dma_start(t[:], ap)
    return t


def _type_tiles(cfg):
    """Yield (t, i_t, r0) for all node tiles: type, tile-in-type, local row base."""
    out = []
    for t, (ntile, b) in enumerate(
        zip((cfg.ntile_a, cfg.ntile_w, cfg.ntile_o), cfg.base_local)
    ):
        for i in range(ntile):
            out.append((t, i, b + i * P))
    return out


def _kqv_width(t):
    """Staging width in slots for type t: [xs | kd | q'_ets | v'_ets]."""
    return 6 if t == 0 else 4


def _kqv_tile(nc, pools, cfg, consts, t, xs_tile, stg, gi):
    """Next-layer projections for one tile via folded weights.

    Writes kd/q'/v' (slots 1..) of stg[:, gi, :]; slot 0 (xs) is written by
    the caller. One transpose + 1-2 bf16 matmuls + 1-2 bias-adds.
    """
    wp, pp_t = pools["wp"], pools["pp_t"]
    xsT_ps = pp_t.tile([P, P], F32, tag="tp_ps")
    nc.tensor.transpose(out=xsT_ps[:], in_=xs_tile[:], identity=consts["ident"][:])
    xsT = wp.tile([P, P], BF16, tag="xsT")
    nc.scalar.copy(out=xsT[:], in_=xsT_ps[:])
    _kqv_mms(nc, pools, cfg, consts, t, xsT[:], stg, gi)


def _kqv_mms(nc, pools, cfg, consts, t, xsT_ap, stg, gi):
    pp_mm = pools["pp_mm"]
    xsT = xsT_ap
    W = consts["Wcat"][t]
    B = consts["bcat"][t]
    nw = (_kqv_width(t) - 1) * HID  # matmul output width (kd + q + v slots)
    n1 = min(nw, 4 * HID)
    ps1 = pp_mm.tile([P, 4 * HID], F32, tag="mmk_ps")
    nc.tensor.matmul(out=ps1[:, :n1], lhsT=xsT[:], rhs=W[:, :n1],
                     start=True, stop=True)
    if _kqv_tile.bias_zero:
        # biases are all zero: plain copy, on the (less busy) scalar engine
        nc.scalar.copy(out=stg[:, gi, HID:HID + n1], in_=ps1[:, :n1])
    else:
        nc.vector.tensor_tensor(out=stg[:, gi, HID:HID + n1], in0=ps1[:, :n1],
                                in1=B[:, :n1], op=OP.add)
    if nw > n1:
        ps2 = pp_mm.tile([P, HID], F32, tag="mm_ps")
        nc.tensor.matmul(out=ps2[:], lhsT=xsT[:], rhs=W[:, n1:nw],
                         start=True, stop=True)
        if _kqv_tile.bias_zero:
            nc.vector.tensor_copy(out=stg[:, gi, HID + n1:HID + nw], in_=ps2[:])
        else:
            nc.vector.tensor_tensor(out=stg[:, gi, HID + n1:HID + nw],
                                    in0=ps2[:], in1=B[:, n1:nw], op=OP.add)


_kqv_tile.bias_zero = False


def _kqv_flush(nc, cfg, outs, t, i0, G, stg, r0base, write_xs=True):
    """DMA a group of G tiles' staged [xs|kd|q|v] slots to their tables."""
    kd_o, qp_o, ve_o = outs["kd"], outs["qp"], outs["ve"]
    xs_o = outs.get("xs")
    rt0 = i0 * P

    def wr(dst, lo):
        nc.gpsimd.dma_start(dst.rearrange("(g p) f -> p g f", p=P),
                            stg[:, :, lo * HID:(lo + 1) * HID])

    if write_xs and xs_o is not None:
        wr(xs_o[r0base:r0base + G * P, :], 0)
    wr(kd_o[r0base:r0base + G * P, :], 1)
    q_slots = ((0, 1), (2,), (3,))[t]
    v_slots = ((0, 1), (2,), (3,))[t]
    for j, sl in enumerate(q_slots):
        wr(qp_o[cfg.QB[sl] + rt0:cfg.QB[sl] + rt0 + G * P, :], 2 + j)
    for j, sl in enumerate(v_slots):
        wr(ve_o[cfg.QB[sl] + rt0:cfg.QB[sl] + rt0 + G * P, :],
           2 + len(q_slots) + j)


def _type_groups(cfg, tiles, G=4):
    """Split the ordered tile list into same-type groups of <= G."""
    groups = []
    cur = []
    for tt in tiles:
        if cur and (tt[0] != cur[0][0] or len(cur) == G):
            groups.append(cur)
            cur = []
        cur.append(tt)
    if cur:
        groups.append(cur)
    return groups


def build_l1(cfg):
    """Launch 1: input proj + relu -> xs1; kqv chain -> kd/q'/ve tables."""
    nc = bacc.Bacc("TRN2", target_bir_lowering=False, debug=False,
                   num_devices=cfg.C)
    c = cfg
    xa = nc.dram_tensor("xta", [64, c.nap], F16, kind="ExternalInput").ap()
    xw = nc.dram_tensor("xtw", [64, c.nwp], F16, kind="ExternalInput").ap()
    xo = nc.dram_tensor("xto", [64, c.nop], F16, kind="ExternalInput").ap()
    Win = nc.dram_tensor("Win", [3, 64, HID], F16, kind="ExternalInput").ap()
    binp = nc.dram_tensor("binT", [3, P, 1], F32, kind="ExternalInput").ap()
    Wcat_d = nc.dram_tensor("Wcat", [3, P, 5 * HID], BF16, kind="ExternalInput").ap()
    bcat_d = nc.dram_tensor("bcat", [3, P, 5 * HID], F32, kind="ExternalInput").ap()
    ident_d = nc.dram_tensor("ident", [P, P], F32, kind="ExternalInput").ap()
    identb_d = nc.dram_tensor("identb", [P, P], F16, kind="ExternalInput").ap()

    xs_o = nc.dram_tensor("xs", [c.LOCN, P], F32, kind="ExternalOutput").ap()
    kd_o = nc.dram_tensor("kd", [c.LOCN, P], F32, kind="ExternalOutput").ap()
    qp_o = nc.dram_tensor("qp", [c.QTOT, P], F32, kind="ExternalOutput").ap()
    ve_o = nc.dram_tensor("ve", [c.QTOT, P], F32, kind="ExternalOutput").ap()

    xin = (xa, xw, xo)
    with tile.TileContext(nc) as tc:
        with tc.tile_pool(name="consts", bufs=1) as cp, \
             tc.tile_pool(name="work", bufs=4) as wp, \
             tc.tile_pool(name="stgp", bufs=2) as sgp, \
             tc.tile_pool(name="ppt", bufs=2, space="PSUM") as pp_t, \
             tc.tile_pool(name="ppmm", bufs=2, space="PSUM") as pp_mm:
            consts = {
                "ident": _load_const(nc, cp, ident_d[:, :], (P, P)),
                "identb": _load_const(nc, cp, identb_d[:, :], (P, P), F16),
                "Win": [_load_const(nc, cp, Win[t], (64, HID), F16) for t in range(3)],
                "bin": [_load_const(nc, cp, binp[t], (P, 1)) for t in range(3)],
                "Wcat": [_load_const(nc, cp, Wcat_d[t], (P, 5 * HID), BF16) for t in range(3)],
                "bcat": [_load_const(nc, cp, bcat_d[t], (P, 5 * HID)) for t in range(3)],
            }
            pools = {"cp": cp, "wp": wp, "pp_t": pp_t, "pp_mm": pp_mm}
            outs = {"kd": kd_o, "qp": qp_o, "ve": ve_o, "xs": xs_o}
            for grp in _type_groups(c, _type_tiles(c)):
                t = grp[0][0]
                G = len(grp)
                i0 = grp[0][1]
                stg = sgp.tile([P, G, _kqv_width(t) * HID], F32, tag="stg")
                # one projection matmul + one relu for the whole group:
                # xsT = relu(Win.T @ xT) directly feature-major (no input
                # transpose; xT comes pre-transposed from the host).
                xt = wp.tile([64, G * P], F16, tag="x_in")
                nc.sync.dma_start(xt[:], xin[t][:, i0 * P:(i0 + G) * P])
                pj_ps = pp_mm.tile([P, G, P], F32, tag="mmk_ps")
                nc.tensor.matmul(
                    out=pj_ps[:].rearrange("p g n -> p (g n)"),
                    lhsT=consts["Win"][t][:], rhs=xt[:],
                    start=True, stop=True)
                xsT_all = wp.tile([P, G, P], F16, tag="xsT_all")
                nc.scalar.activation(out=xsT_all[:], in_=pj_ps[:], func=AF.Relu,
                                     bias=consts["bin"][t][:, 0:1])
                for gi, (_, i_t, r0) in enumerate(grp):
                    xs_ps = pp_t.tile([P, P], F16, tag="tpb_ps")
                    nc.tensor.transpose(out=xs_ps[:], in_=xsT_all[:, gi, :],
                                        identity=consts["identb"][:])
                    nc.scalar.copy(out=stg[:, gi, :HID], in_=xs_ps[:])
                    _kqv_mms(nc, pools, c, consts, t, xsT_all[:, gi, :], stg, gi)
                _kqv_flush(nc, c, outs, t, grp[0][1], G, stg, grp[0][2])
    nc.compile()
    return nc


def build_l23(cfg, NT, tws, last):
    """Launches 2/3: edge phase + node phase (+ head if last)."""
    nc = bacc.Bacc("TRN2", target_bir_lowering=False, debug=False,
                   num_devices=cfg.C)
    c = cfg
    kg_d = nc.dram_tensor("kg", [P, NT, HID], BF16, kind="ExternalInput").ap()
    vg_d = nc.dram_tensor("vg", [P, NT, HID], BF16, kind="ExternalInput").ap()
    qg_d = nc.dram_tensor("qg", [P, NT, HID], BF16, kind="ExternalInput").ap()
    NWIN_E = c.ntile_a if last else c.NWIN   # only a-dst windows feed the head
    LOCN_E = c.nap if last else c.LOCN
    xs_in = nc.dram_tensor("xsin", [LOCN_E, P], F32, kind="ExternalInput").ap()
    ti_t = nc.dram_tensor("dstlT", [P, NT], BF16, kind="ExternalInput").ap()
    iota_d = nc.dram_tensor("iota", [P, P], BF16, kind="ExternalInput").ap()
    ident_d = nc.dram_tensor("ident", [P, P], F32, kind="ExternalInput").ap()
    Wo_d = nc.dram_tensor("Wo", [3, P, HID], F32, kind="ExternalInput").ap()
    bo_d = nc.dram_tensor("bo", [3, P, HID], F32, kind="ExternalInput").ap()
    lng_d = nc.dram_tensor("lng", [3, P, HID], F32, kind="ExternalInput").ap()
    lnb_d = nc.dram_tensor("lnb", [3, P, HID], F32, kind="ExternalInput").ap()
    gs_d = nc.dram_tensor("gs", [3], F32, kind="ExternalInput").ap()  # unused on-device; values baked via bo/g mul
    if not last:
        Wcat_d = nc.dram_tensor("Wcat", [3, P, 5 * HID], BF16, kind="ExternalInput").ap()
        bcat_d = nc.dram_tensor("bcat", [3, P, 5 * HID], F32, kind="ExternalInput").ap()
    else:
        wh_d = nc.dram_tensor("whead", [P, 1], F32, kind="ExternalInput").ap()
        bh_d = nc.dram_tensor("bh", [P, 1], F32, kind="ExternalInput").ap()

    if not last:
        xs_o = nc.dram_tensor("xs", [c.LOCN, P], F32, kind="ExternalOutput").ap()
        kd_o = nc.dram_tensor("kd", [c.LOCN, P], F32, kind="ExternalOutput").ap()
        qp_o = nc.dram_tensor("qp", [c.QTOT, P], F32, kind="ExternalOutput").ap()
        ve_o = nc.dram_tensor("ve", [c.QTOT, P], F32, kind="ExternalOutput").ap()
    else:
        dl_o = nc.dram_tensor("delta", [c.nap, 1], F32, kind="ExternalOutput").ap()

    # gains folded on host: bo tile already contains g*b_o. g itself baked as consts below.
    g_vals = None  # set in kernel() via attribute hack? no: pass via build arg
    g_list = build_l23.g_list  # [3] floats for this layer

    with tile.TileContext(nc) as tc:
        with tc.tile_pool(name="consts", bufs=1) as cp, \
             tc.tile_pool(name="idx", bufs=2) as idxp, \
             tc.tile_pool(name="gat", bufs=2) as gp, \
             tc.tile_pool(name="ework", bufs=2) as ewp, \
             tc.tile_pool(name="nwork", bufs=3) as wp, \
             tc.tile_pool(name="stgp", bufs=2) as sgp, \
             tc.tile_pool(name="small", bufs=4) as sp, \
             tc.tile_pool(name="flush", bufs=2) as fp, \
             tc.tile_pool(name="dram", bufs=1, space="DRAM") as dp, \
             tc.tile_pool(name="ppe", bufs=2, space="PSUM") as pp_e, \
             tc.tile_pool(name="ppt", bufs=2, space="PSUM") as pp_t, \
             tc.tile_pool(name="ppmm", bufs=2, space="PSUM") as pp_mm:

            numer = dp.tile([LOCN_E, PAY], F32)
            eps_t = cp.tile([P, 1], F32, tag="lneps")
            nc.vector.memset(eps_t[:], 1e-5)

            consts = {
                "iota": _load_const(nc, cp, iota_d[:, :], (P, P), BF16),
                "ident": _load_const(nc, cp, ident_d[:, :], (P, P)),
                "Wo": [_load_const(nc, cp, Wo_d[t], (P, HID)) for t in range(3)],
                "bo": [_load_const(nc, cp, bo_d[t], (P, HID)) for t in range(3)],
                "lng": [_load_const(nc, cp, lng_d[t], (P, HID)) for t in range(3)],
                "lnb": [_load_const(nc, cp, lnb_d[t], (P, HID)) for t in range(3)],
            }
            if not last:
                consts.update({
                    "Wcat": [_load_const(nc, cp, Wcat_d[t], (P, 5 * HID), BF16) for t in range(3)],
                    "bcat": [_load_const(nc, cp, bcat_d[t], (P, 5 * HID)) for t in range(3)],
                })
            else:
                consts["whead"] = _load_const(nc, cp, wh_d[:, :], (P, 1))
                consts["bh"] = _load_const(nc, cp, bh_d[:, :], (P, 1))

            # ---------------- edge phase ----------------
            # streams are host-pre-gathered per edge (bf16). Windows are
            # grouped greedily into supers (<= SUPER_T tiles); per super one
            # DMA per stream + one batched op per DVE stage; scatter stays a
            # per-tile bf16 one-hot matmul into the window's PSUM accumulator.
            SUPER_T = 20
            supers = []  # (g0, [T_w...], w0)
            gtile = 0
            w = 0
            while w < NWIN_E:
                g0 = gtile
                ts = []
                w0 = w
                while w < NWIN_E and len(ts) < 4 and \
                        sum(ts) + int(tws[w]) <= SUPER_T:
                    ts.append(int(tws[w]))
                    gtile += int(tws[w])
                    w += 1
                supers.append((g0, ts, w0))
            for g0, ts, w0 in supers:
                TS = sum(ts)
                kgt = gp.tile([P, TS, HID], BF16, tag="kgt")
                nc.sync.dma_start(kgt[:], kg_d[:, g0:g0 + TS, :])
                vgt = gp.tile([P, TS, HID], BF16, tag="vgt")
                nc.sync.dma_start(vgt[:], vg_d[:, g0:g0 + TS, :])
                qgt = gp.tile([P, TS, HID], BF16, tag="qgt")
                nc.scalar.dma_start(qgt[:], qg_d[:, g0:g0 + TS, :])
                dstl = idxp.tile([P, TS], BF16, tag="dstl")
                nc.scalar.dma_start(dstl[:], ti_t[:, g0:g0 + TS])

                prod = ewp.tile([P, TS, HID], BF16, tag="prod")
                nc.vector.tensor_tensor(out=prod[:], in0=kgt[:], in1=qgt[:],
                                        op=OP.mult)
                alpha = ewp.tile([P, TS, H], F32, tag="alpha")
                nc.vector.tensor_reduce(
                    out=alpha[:],
                    in_=prod[:].rearrange("p t (h d) -> p t h d", h=H),
                    axis=mybir.AxisListType.X, op=OP.add)
                payload = ewp.tile([P, TS, PAY], BF16, tag="payload")
                ex = payload[:, :, HID:HID + H]
                nc.scalar.activation(out=ex, in_=alpha[:], func=AF.Exp)
                nc.vector.tensor_tensor(
                    out=payload[:, :, :HID].rearrange("p t (h d) -> p t h d", h=H),
                    in0=vgt[:].rearrange("p t (h d) -> p t h d", h=H),
                    in1=ex[:, :, :, None].to_broadcast([P, TS, H, D]),
                    op=OP.mult)
                onehot = ewp.tile([P, TS, P], BF16, tag="onehot")
                nc.vector.tensor_tensor(
                    out=onehot[:],
                    in0=dstl[:, :, None].to_broadcast([P, TS, P]),
                    in1=consts["iota"][:, None, :].to_broadcast([P, TS, P]),
                    op=OP.is_equal)
                fl = fp.tile([P, len(ts), PAY], F32, tag="fl")
                toff = 0
                for wi, T in enumerate(ts):
                    psum_w = pp_e.tile([P, PAY], F32, tag="psw")
                    for t in range(toff, toff + T):
                        nc.tensor.matmul(out=psum_w[:], lhsT=onehot[:, t, :],
                                         rhs=payload[:, t, :],
                                         start=(t == toff),
                                         stop=(t == toff + T - 1))
                    nc.vector.tensor_copy(out=fl[:, wi, :], in_=psum_w[:])
                    toff += T
                nc.sync.dma_start(
                    numer[w0 * P:(w0 + len(ts)) * P, :].rearrange(
                        "(t p) f -> p t f", p=P),
                    fl[:])

            # ---------------- node phase ----------------
            pools = {"cp": cp, "wp": wp, "pp_t": pp_t, "pp_mm": pp_mm}
            outs = None if last else {"kd": kd_o, "qp": qp_o, "ve": ve_o, "xs": xs_o}
            tiles = [x for x in _type_tiles(c) if (not last) or x[0] == 0]
            NTL = len(tiles)
            # pass 1: all tiles up through the skip-add + LN stats; gelu is the
            # only table-based ACT function here so the scalar engine loads
            # the gelu table once instead of thrashing gelu<->sqrt per tile.
            o1_all = cp.tile([P, NTL, HID], F16, tag="o1_all")
            mv_all = cp.tile([P, NTL, 2], F32, tag="mv_all")
            gi0 = 0
            for grp in _type_groups(c, tiles):
                t = grp[0][0]
                G = len(grp)
                r00 = grp[0][2]
                nm4 = wp.tile([P, G, PAY], F32, tag="nm")
                nc.scalar.dma_start(
                    nm4[:],
                    numer[r00:r00 + G * P, :].rearrange("(g p) f -> p g f", p=P))
                xs4 = wp.tile([P, G, HID], F32, tag="xs_ld")
                nc.scalar.dma_start(
                    xs4[:],
                    xs_in[r00:r00 + G * P, :].rearrange("(g p) f -> p g f", p=P))
                den = sp.tile([P, G, H], F32, tag="den")
                nc.vector.tensor_scalar_add(den[:], nm4[:, :, HID:HID + H], 1e-16)
                rec = sp.tile([P, G, H], F32, tag="rec")
                nc.vector.reciprocal(rec[:], den[:])
                agg = wp.tile([P, G, HID], F32, tag="agg")
                nc.vector.tensor_tensor(
                    out=agg[:].rearrange("p g (h d) -> p g h d", h=H),
                    in0=nm4[:, :, :HID].rearrange("p g (h d) -> p g h d", h=H),
                    in1=rec[:, :, :, None].to_broadcast([P, G, H, D]),
                    op=OP.mult)
                glu = wp.tile([P, G, HID], F32, tag="glu")
                if os.environ.get("HGT_BACKEND", "hw") == "sim":
                    # CoreSim has no Gelu LUT: tanh approximation (dev only)
                    t1 = wp.tile([P, G, HID], F32, tag="gelu_t1")
                    nc.vector.tensor_tensor(out=t1[:], in0=agg[:], in1=agg[:], op=OP.mult)
                    nc.vector.tensor_tensor(out=t1[:], in0=t1[:], in1=agg[:], op=OP.mult)
                    nc.vector.tensor_scalar(out=t1[:], in0=t1[:], scalar1=0.044715,
                                            scalar2=None, op0=OP.mult)
                    nc.vector.tensor_tensor(out=t1[:], in0=t1[:], in1=agg[:], op=OP.add)
                    nc.scalar.activation(out=t1[:], in_=t1[:], func=AF.Tanh,
                                         scale=0.7978845608028654)
                    nc.vector.tensor_scalar(out=t1[:], in0=t1[:], scalar1=0.5,
                                            scalar2=0.5, op0=OP.mult, op1=OP.add)
                    nc.vector.tensor_tensor(out=glu[:], in0=t1[:], in1=agg[:], op=OP.mult)
                else:
                    nc.scalar.activation(out=glu[:], in_=agg[:], func=AF.Gelu)
                ops4 = pp_mm.tile([P, G, HID], F32, tag="mmk_ps")
                for gi in range(G):
                    gluT_ps = pp_t.tile([P, P], F32, tag="tp_ps")
                    nc.tensor.transpose(out=gluT_ps[:], in_=glu[:, gi, :],
                                        identity=consts["ident"][:])
                    gluT = wp.tile([P, P], F32, tag="gluT")
                    nc.scalar.copy(out=gluT[:], in_=gluT_ps[:])
                    nc.tensor.matmul(out=ops4[:, gi, :], lhsT=gluT[:],
                                     rhs=consts["Wo"][t][:],
                                     start=True, stop=True)
                # o3 = g*o + (g*b_o) + (1-g)*xs  (bo const already has g*b_o)
                o1g = o1_all[:, gi0:gi0 + G, :]
                xs_s = wp.tile([P, G, HID], F32, tag="xs_s")
                nc.vector.tensor_scalar_mul(xs_s[:], xs4[:], float(1.0 - g_list[t]))
                if build_l23.bo_trivial:
                    nc.vector.tensor_tensor(out=o1g, in0=ops4[:], in1=xs_s[:],
                                            op=OP.add)
                else:
                    nc.vector.tensor_tensor(
                        out=o1g, in0=ops4[:],
                        in1=consts["bo"][t][:, None, :].to_broadcast([P, G, HID]),
                        op=OP.add)
                    nc.vector.tensor_tensor(out=o1g, in0=o1g, in1=xs_s[:], op=OP.add)
                for gi in range(G):
                    stats = sp.tile([P, nc.vector.BN_STATS_DIM], F32, tag="stats")
                    nc.vector.bn_stats(out=stats[:], in_=o1_all[:, gi0 + gi, :])
                    nc.vector.bn_aggr(out=mv_all[:, gi0 + gi, :], in_=stats[:])
                gi0 += G
            # one batched sqrt for all tiles' variances (single table load)
            rstd_all = cp.tile([P, NTL], F32, tag="rstd_all")
            nc.scalar.activation(out=rstd_all[:], in_=mv_all[:, :, 1],
                                 func=AF.Sqrt,
                                 bias=eps_t[:, 0:1])
            nc.vector.reciprocal(rstd_all[:], rstd_all[:])
            # pass 2: normalize + relu + next-layer projections (relu/copy are
            # in every ACT table set, so no further table switches).
            def _xh_relu(i, t, dst_ap):
                xh = wp.tile([P, HID], F32, tag="xh")
                nc.vector.tensor_scalar(
                    out=xh[:], in0=o1_all[:, i, :], scalar1=mv_all[:, i, 0:1],
                    scalar2=rstd_all[:, i:i + 1],
                    op0=OP.subtract, op1=OP.mult)
                if not build_l23.ln_trivial:
                    nc.vector.tensor_tensor(out=xh[:], in0=xh[:], in1=consts["lng"][t][:], op=OP.mult)
                    nc.vector.tensor_tensor(out=xh[:], in0=xh[:], in1=consts["lnb"][t][:], op=OP.add)
                nc.scalar.activation(out=dst_ap, in_=xh[:], func=AF.Relu)

            if not last:
                gi0 = 0
                for grp in _type_groups(c, tiles):
                    t = grp[0][0]
                    G = len(grp)
                    stg = sgp.tile([P, G, _kqv_width(t) * HID], F32, tag="stg")
                    for gi, (_, i_t, r0) in enumerate(grp):
                        _xh_relu(gi0 + gi, t, stg[:, gi, :HID])
                        _kqv_tile(nc, pools, c, consts, t, stg[:, gi, :HID], stg, gi)
                    _kqv_flush(nc, c, outs, t, grp[0][1], G, stg, grp[0][2])
                    gi0 += G
            else:
                for g0i in range(0, len(tiles), 4):
                    grp = tiles[g0i:g0i + 4]
                    G = len(grp)
                    dlst = sp.tile([P, 4, 1], F32, tag="dlst")
                    for gi, (t, i_t, r0) in enumerate(grp):
                        xs_new = wp.tile([P, HID], F32, tag="xs_new")
                        _xh_relu(g0i + gi, t, xs_new[:])
                        xnT_ps = pp_t.tile([P, P], F32, tag="tp_ps")
                        nc.tensor.transpose(out=xnT_ps[:], in_=xs_new[:], identity=consts["ident"][:])
                        xnT = wp.tile([P, P], F32, tag="xnT")
                        nc.scalar.copy(out=xnT[:], in_=xnT_ps[:])
                        d_ps = pp_mm.tile([P, 1], F32, tag="mm_ps")
                        nc.tensor.matmul(out=d_ps[:], lhsT=xnT[:], rhs=consts["whead"][:],
                                         start=True, stop=True)
                        nc.vector.tensor_tensor(out=dlst[:, gi, :], in0=d_ps[:],
                                                in1=consts["bh"][:], op=OP.add)
                    nc.gpsimd.dma_start(
                        dl_o[g0i * P:(g0i + G) * P, :].rearrange(
                            "(g p) f -> p g f", p=P),
                        dlst[:, :G, :])
    nc.compile()
    return nc


build_l23.g_list = None
build_l23.ln_trivial = False
build_l23.bo_trivial = False


# ---------------------------------------------------------------------------
# Runner
# ---------------------------------------------------------------------------

LAUNCH_TIMES_NS = []
TRACE_DIRS = []


def _run(nc, in_maps, cfg):
    backend = os.environ.get("HGT_BACKEND", "hw")
    if backend == "sim":
        from concourse.bass_interp import CoreSim
        results = []
        for m in in_maps:
            sim = CoreSim(nc, trace=False, require_finite=False, require_nnan=False)
            for k, v in m.items():
                sim.tensor(k)[:] = v
            sim.simulate(check_with_hw=False)
            out = {}
            for alloc in nc.m.functions[0].allocations:
                if isinstance(alloc, mybir.MemoryLocationSet) and alloc.kind == "ExternalOutput":
                    name = alloc.memorylocations[0].name
                    out[name] = sim.tensor(name).copy()
            results.append(out)
        return results
    else:
        from concourse.bass_utils import run_bass_kernel_spmd
        trace = os.environ.get("HGT_TRACE", "0") == "1"
        res = run_bass_kernel_spmd(nc, in_maps, core_ids=list(range(cfg.C)),
                                   trace=trace)
        if trace:
            LAUNCH_TIMES_NS.append(res.exec_time_ns)
            it = res.instructions_and_trace
            TRACE_DIRS.append(getattr(it, "trace_path", it))
        return res.results


# ---------------------------------------------------------------------------
# Main entry
# ---------------------------------------------------------------------------

def kernel(**inputs):
    cfg = Cfg()
    return _kernel_impl(cfg, inputs)


def _kernel_impl(cfg, inputs):
    c = cfg
    prm = prep_params(c, inputs)
    g = prep_graph(c, inputs)
    NT, tws = g["NT"], g["tws"]

    # ---- launch 1
    _kqv_tile.bias_zero = not np.asarray(inputs["b_kqv"])[0].any()
    nc1 = build_l1(c)
    in_maps = []
    xa = np.asarray(inputs["x_a"], np.float32)
    xw = np.asarray(inputs["x_w"], np.float32)
    xo = np.asarray(inputs["x_o"], np.float32)

    import ml_dtypes as _mld

    def padxT(x, n, npad):
        out = np.zeros((64, npad), np.float32)
        out[:, :n] = x.T
        return out.astype(np.float16)

    for cc in range(c.C):
        in_maps.append({
            "xta": padxT(xa[cc * c.nac:(cc + 1) * c.nac], c.nac, c.nap),
            "xtw": padxT(xw[cc * c.nwc:(cc + 1) * c.nwc], c.nwc, c.nwp),
            "xto": padxT(xo[cc * c.noc:(cc + 1) * c.noc], c.noc, c.nop),
            "Win": prm["WinB"], "binT": prm["binT"],
            "identb": prm["identb"],
            "Wcat": prm["Wcat0"], "bcat": prm["bcat0"],
            "ident": prm["ident"],
        })
    r1 = _run(nc1, in_maps, c)

    def assemble_tables(res):
        """Build global kd table + per-core q' tables + global ve table."""
        kd_tab = np.empty((c.KD_ROWS, HID), np.float32)
        kd_tab[-1] = 1.0
        ve_tab = np.empty((c.VE_ROWS, HID), np.float32)
        ve_tab[-1] = 0.0
        qp_tabs = []
        for cc in range(c.C):
            kd = res[cc]["kd"]
            ve = res[cc]["ve"]
            # kd local [a|w|o] -> global
            kd_tab[c.KOFF[0] + cc * c.nac:c.KOFF[0] + (cc + 1) * c.nac] = kd[:c.nac]
            kd_tab[c.KOFF[1] + cc * c.nwc:c.KOFF[1] + (cc + 1) * c.nwc] = \
                kd[c.base_local[1]:c.base_local[1] + c.nwc]
            kd_tab[c.KOFF[2] + cc * c.noc:c.KOFF[2] + (cc + 1) * c.noc] = \
                kd[c.base_local[2]:c.base_local[2] + c.noc]
            # ve local slots [a-et2, a-et3, w-et0, o-et1] -> global stacked
            ve_tab[c.VOFF[2] + cc * c.nac:c.VOFF[2] + (cc + 1) * c.nac] = \
                ve[c.QB[0]:c.QB[0] + c.nac]
            ve_tab[c.VOFF[3] + cc * c.nac:c.VOFF[3] + (cc + 1) * c.nac] = \
                ve[c.QB[1]:c.QB[1] + c.nac]
            ve_tab[c.VOFF[0] + cc * c.nwc:c.VOFF[0] + (cc + 1) * c.nwc] = \
                ve[c.QB[2]:c.QB[2] + c.nwc]
            ve_tab[c.VOFF[1] + cc * c.noc:c.VOFF[1] + (cc + 1) * c.noc] = \
                ve[c.QB[3]:c.QB[3] + c.noc]
            # pad-edge q rows are -8.0: with pad k rows = 1.0 the pad alpha is
            # 128 * -8 = -1024 (bf16-safe), exp -> 0.
            qp = np.vstack([res[cc]["qp"], np.full((1, HID), -8.0, np.float32)])
            qp_tabs.append(qp)
        return kd_tab, ve_tab, qp_tabs

    import ml_dtypes
    bf16 = ml_dtypes.bfloat16

    # ---- launches 2 and 3
    # last launch only needs a-dst windows (head reads only a-type nodes)
    NT_a = int(tws[:c.ntile_a].sum())
    res = r1
    for l, last in ((1, False), (2, True)):
        kd_tab, ve_tab, qp_tabs = assemble_tables(res)
        lay = l - 1  # layer params index: launch2 -> layer 0, launch3 -> layer 1
        build_l23.g_list = prm[f"g{lay}"]
        build_l23.bo_trivial = not np.asarray(inputs["b_o"])[lay].any()
        build_l23.ln_trivial = bool(
            (np.asarray(inputs["ln_g"])[lay] == 1).all()
            and not np.asarray(inputs["ln_b"])[lay].any())
        _kqv_tile.bias_zero = (not last) and \
            not np.asarray(inputs["b_kqv"])[lay + 1].any()
        NT_l = NT_a if last else NT
        nc = build_l23(c, NT_l, tws, last)
        in_maps = []
        for cc in range(c.C):
            ti = g["tile_idx"][cc][:NT_l]  # [NT_l, P, 4]
            kg_s = np.ascontiguousarray(
                kd_tab[ti[:, :, 0]].transpose(1, 0, 2)).astype(bf16)
            vg_s = np.ascontiguousarray(
                ve_tab[ti[:, :, 1]].transpose(1, 0, 2)).astype(bf16)
            qg_s = np.ascontiguousarray(
                qp_tabs[cc][ti[:, :, 2]].transpose(1, 0, 2)).astype(bf16)
            dstlT = np.ascontiguousarray(ti[:, :, 3].T.astype(bf16))
            m = {
                "kg": kg_s, "vg": vg_s, "qg": qg_s,
                "xsin": res[cc]["xs"][:c.nap] if last else res[cc]["xs"],
                "dstlT": dstlT,
                "iota": prm["iota"], "ident": prm["ident"],
                "Wo": prm[f"Wo{lay}"], "bo": prm[f"bo{lay}"],
                "lng": prm[f"lng{lay}"], "lnb": prm[f"lnb{lay}"],
                "gs": prm[f"g{lay}"],
            }
            if not last:
                m.update({"Wcat": prm[f"Wcat{lay + 1}"], "bcat": prm[f"bcat{lay + 1}"]})
            else:
                m.update({"whead": prm["whead"], "bh": prm["bh"]})
            in_maps.append(m)
        res = _run(nc, in_maps, c)

    out = np.concatenate([res[cc]["delta"][:c.nac, 0] for cc in range(c.C)])
    return out.astype(np.float32)



# revision 28
# speedup vs baseline: 3.7708x; 1.0382x over previous
"""HGT regressor on 8 Trainium2 NeuronCores (Bass/Tile).

Strategy (graph/data parallel, hint-following):
  - Nodes of each type are partitioned contiguously across the 8 cores
    (a: 12500/core, w: 2500/core, o: 6250/core). Each core owns the edges
    whose *destination* lies in its node shard.
  - Per layer, each core computes K = kqv[:, :128] (raw) and the per-edge-type
    source-side V transform (m_rel folded at source) plus the destination-side
    Q transform (a_rel * p_rel * scale folded into Q) for its own nodes only.
  - The full K / V_et tables are exchanged between layer launches via the host
    (replicated to all cores), i.e. host-mediated all-gather. Q' stays local.
  - Edge phase per core: edges sorted by local destination row, grouped into
    128-node windows; per 128-edge tile: indirect-DMA gathers of K[src],
    V_et[src], Q'_et[dst]; alpha = sum_h(K*Q'); ex = exp(alpha); payload
    [ex*V | ex] is scatter-added into a PSUM window accumulator via a
    one-hot matmul; windows flush densely to a numer/den table in DRAM.
  - Node phase per core: agg = numer/den, gelu, W_o, gated skip, LayerNorm,
    relu, then next-layer projections (or the scalar head in the last layer).
  - Softmax needs no running max: alpha = q'k with these parameter scales is
    O(1); exp cannot overflow, and softmax is shift-invariant anyway.
"""
import os
import sys

sys.path.insert(0, "/opt/trn_rl_repo")

import numpy as np

import concourse.bass as bass
import concourse.mybir as mybir
import concourse.tile as tile
from concourse import bacc

P = 128
H, D, HID = 4, 32, 128
PAY = HID + H  # 132
F32 = mybir.dt.float32
F16 = mybir.dt.float16
BF16 = mybir.dt.bfloat16
I32 = mybir.dt.int32
AF = mybir.ActivationFunctionType
OP = mybir.AluOpType


def _ceil(a, b):
    return (a + b - 1) * b // b if False else -(-a // b) * b


def cdiv(a, b):
    return -(-a // b)


class Cfg:
    """All sizes derived from problem scale; supports mini-scale testing."""

    def __init__(self, NA=100000, NWK=20000, NO=50000, E=150000, C=8):
        self.NA, self.NWK, self.NO, self.E, self.C = NA, NWK, NO, E, C
        assert NA % C == 0 and NWK % C == 0 and NO % C == 0
        self.nac, self.nwc, self.noc = NA // C, NWK // C, NO // C
        self.nap, self.nwp, self.nop = (
            cdiv(self.nac, P) * P,
            cdiv(self.nwc, P) * P,
            cdiv(self.noc, P) * P,
        )
        # local node-row layout (numer/xs/kd rows): [a | w | o], each padded
        self.base_local = (0, self.nap, self.nap + self.nwp)
        self.LOCN = self.nap + self.nwp + self.nop
        self.NWIN = self.LOCN // P
        # per-type tile counts
        self.ntile_a, self.ntile_w, self.ntile_o = (
            self.nap // P,
            self.nwp // P,
            self.nop // P,
        )
        # q' local layout: slots [a-et0, a-et1, w-et2, o-et3]
        self.QB = (0, self.nap, 2 * self.nap, 2 * self.nap + self.nwp)
        self.QTOT = 2 * self.nap + self.nwp + self.nop
        # ve local layout (same bases): slots [a-et2, a-et3, w-et0, o-et1]
        # global kd table layout: [a 0..NA | w | o] + trash
        self.KOFF = (0, NA, NA + NWK)
        self.KD_ROWS = NA + NWK + NO + 1
        # global stacked ve table: [et0 w | et1 o | et2 a | et3 a] + trash
        self.VOFF = (0, NWK, NWK + NO, NWK + NO + NA)
        self.VE_ROWS = NWK + NO + 2 * NA + 1


# edge types: (src_type, dst_type)
ETYPES = ((1, 0), (2, 0), (0, 1), (0, 2))

# (d, h)-interleaved feature order: new col d*H+h <- old col h*D+d
DH_PERM = np.arange(HID).reshape(H, D).T.flatten()


# ---------------------------------------------------------------------------
# Host-side preprocessing
# ---------------------------------------------------------------------------

def prep_graph(cfg, inputs):
    """Compute per-core edge tile indices. Shared across both layers.

    Returns dict with:
      NT: static tile count (same all cores)
      tws: [NWIN] tiles per window (static across cores)
      tile_idx: [C][NT, P, 4] int32  (kidx, vidx, qidx, dst_local)
    """
    c = cfg
    edges = []  # per et: (src, dst)
    for name_s, name_d in (("src_wa", "dst_wa"), ("src_oa", "dst_oa"),
                           ("src_aw", "dst_aw"), ("src_ao", "dst_ao")):
        edges.append((np.asarray(inputs[name_s]), np.asarray(inputs[name_d])))

    shard_n = (c.nac, c.nwc, c.noc)
    # concat all ets with global indices
    K_TRASH = c.KD_ROWS - 1
    V_TRASH = c.VE_ROWS - 1
    Q_TRASH = c.QTOT  # row appended by host to the q' table

    all_core = []
    koff_by_et = (c.KOFF[1], c.KOFF[2], c.KOFF[0], c.KOFF[0])  # src type offset in kd
    for et, (st, dt) in enumerate(ETYPES):
        src, dst = edges[et]
        kidx = koff_by_et[et] + src
        vidx = c.VOFF[et] + src
        core = dst // shard_n[dt]
        dloc = dst - core * shard_n[dt]  # dst index within its type shard
        # local numer row / q' row
        tb = (c.base_local[0], c.base_local[1], c.base_local[2])[dt]
        row = tb + dloc
        qslot = {0: 0, 1: 1, 2: 2, 3: 3}[et]
        qidx = c.QB[qslot] + dloc
        all_core.append((core, row, kidx, vidx, qidx))

    core_cat = np.concatenate([a[0] for a in all_core])
    row_cat = np.concatenate([a[1] for a in all_core])
    k_cat = np.concatenate([a[2] for a in all_core])
    v_cat = np.concatenate([a[3] for a in all_core])
    q_cat = np.concatenate([a[4] for a in all_core])

    # per-core, per-window edge counts -> static tile structure
    win_cat = row_cat // P
    counts = np.zeros((c.C, c.NWIN), np.int64)
    for cc in range(c.C):
        m = core_cat == cc
        counts[cc] = np.bincount(win_cat[m], minlength=c.NWIN)
    tws = np.maximum(cdiv(counts.max(axis=0), P), 1)  # >=1 tile per window
    NT = int(tws.sum())
    tile_base = np.zeros(c.NWIN, np.int64)
    tile_base[1:] = np.cumsum(tws)[:-1]

    tile_idx = np.zeros((c.C, NT, P, 4), np.int32)
    # fill pads with trash rows -> ex = 0 contributions
    tile_idx[:, :, :, 0] = K_TRASH
    tile_idx[:, :, :, 1] = V_TRASH
    tile_idx[:, :, :, 2] = Q_TRASH
    tile_idx[:, :, :, 3] = 0
    for cc in range(c.C):
        m = core_cat == cc
        rows = row_cat[m]
        order = np.argsort(rows, kind="stable")
        rows = rows[order]
        ks, vs, qs = k_cat[m][order], v_cat[m][order], q_cat[m][order]
        wins = rows // P
        dstl = rows % P
        # position within window
        wstart = np.searchsorted(wins, np.arange(c.NWIN), side="left")
        pos = np.arange(rows.size) - wstart[wins]
        slot_t = pos // P   # tile within window
        slot_p = pos % P    # partition
        gt = tile_base[wins] + slot_t  # global tile id
        tile_idx[cc, gt, slot_p, 0] = ks
        tile_idx[cc, gt, slot_p, 1] = vs
        tile_idx[cc, gt, slot_p, 2] = qs
        tile_idx[cc, gt, slot_p, 3] = dstl
    return {"NT": NT, "tws": tws.astype(np.int64), "tile_idx": tile_idx,
            "tile_base": tile_base}


def blockdiag(M):
    out = np.zeros((HID, HID), np.float32)
    for h in range(H):
        out[h * D:(h + 1) * D, h * D:(h + 1) * D] = M[h]
    return out


def prep_params(cfg, inputs):
    """Fold and lay out all parameters (host, tiny)."""
    scale = np.float32(1.0 / np.sqrt(D))
    a_rel = np.asarray(inputs["a_rel"])
    m_rel = np.asarray(inputs["m_rel"])
    p_rel = np.asarray(inputs["p_rel"])
    import ml_dtypes as _mld
    prm = {}
    rep = lambda v, w: np.broadcast_to(np.asarray(v, np.float32)[None, :], (P, w)).copy()
    for l in range(2):
        BDaT, BDm = [], []
        for et in range(4):
            a_eff = a_rel[l, et] * (p_rel[l, et] * scale)[:, None, None]
            BDaT.append(blockdiag(a_eff).T.copy())
            BDm.append(blockdiag(m_rel[l, et]))
        prm[f"BDaT{l}"] = np.stack(BDaT)  # [4,128,128]
        prm[f"BDm{l}"] = np.stack(BDm)
        prm[f"Wkqv{l}"] = np.asarray(inputs["W_kqv"])[l]       # [3,128,384]
        prm[f"bkqv{l}"] = np.stack([rep(np.asarray(inputs["b_kqv"])[l, t], 3 * HID) for t in range(3)])
        # folded projection weights: per type, slots [kd | q'_ets | v'_ets]
        # q' = xs @ (Wq @ BDaT_et), v' = xs @ (Wv @ BDm_et); biases likewise.
        import ml_dtypes
        q_ets = ((0, 1), (2,), (3,))
        v_ets = ((2, 3), (0,), (1,))
        Wcat = np.zeros((3, HID, 5 * HID), np.float32)
        bcat = np.zeros((3, P, 5 * HID), np.float32)
        for t in range(3):
            Wk = np.asarray(inputs["W_kqv"])[l, t]
            bk = np.asarray(inputs["b_kqv"])[l, t]
            cols = [Wk[:, :HID]]
            bs = [bk[:HID]]
            for et in q_ets[t]:
                cols.append(Wk[:, HID:2 * HID] @ prm[f"BDaT{l}"][et])
                bs.append(bk[HID:2 * HID] @ prm[f"BDaT{l}"][et])
            for et in v_ets[t]:
                cols.append(Wk[:, 2 * HID:] @ prm[f"BDm{l}"][et])
                bs.append(bk[2 * HID:] @ prm[f"BDm{l}"][et])
            cat = np.concatenate(cols, axis=1)
            Wcat[t, :, :cat.shape[1]] = cat
            bcat[t, :, :cat.shape[1]] = np.concatenate(bs)[None, :]
        prm[f"Wcat{l}"] = Wcat.astype(ml_dtypes.bfloat16)
        prm[f"bcat{l}"] = bcat
        g = 1.0 / (1.0 + np.exp(-np.asarray(inputs["skip_p"], np.float64)))  # [2,3]
        # skip gain g folded into Wo; rows permuted to the (d, h)-interleaved
        # feature order the aggregated V stream arrives in.
        prm[f"Wo{l}"] = (np.asarray(inputs["W_o"])[l]
                         * g[l].astype(np.float32)[:, None, None])[
                             :, DH_PERM, :]  # [3,128,128]
        prm[f"g{l}"] = g[l].astype(np.float32)
        prm[f"bo{l}"] = np.stack([rep(np.asarray(inputs["b_o"])[l, t] * g[l, t], HID) for t in range(3)])
        prm[f"lng{l}"] = np.stack([rep(np.asarray(inputs["ln_g"])[l, t], HID) for t in range(3)])
        prm[f"lnb{l}"] = np.stack([rep(np.asarray(inputs["ln_b"])[l, t], HID) for t in range(3)])
    # input proj, padded to 128 contraction
    W_in = np.asarray(inputs["W_in"])  # [3,64,128]
    Wp = np.zeros((3, 128, HID), np.float32)
    Wp[:, :64, :] = W_in
    prm["Win"] = Wp
    prm["bin"] = np.stack([rep(np.asarray(inputs["b_in"])[t], HID) for t in range(3)])
    prm["WinB"] = W_in.astype(np.float16)  # [3,64,128]
    prm["binT"] = np.ascontiguousarray(
        np.asarray(inputs["b_in"], np.float32)[:, :, None])  # [3,128,1]
    prm["whead"] = np.asarray(inputs["w_head"], np.float32)  # [128,1]
    prm["bh"] = np.full((P, 1), float(np.asarray(inputs["b_head"])[0] + np.asarray(inputs["base"])[0]), np.float32)
    prm["iota"] = np.broadcast_to(
        np.arange(128, dtype=np.float32).astype(_mld.bfloat16)[None, :],
        (P, 128)).copy()
    prm["ident"] = np.eye(128, dtype=np.float32)
    prm["identb"] = np.eye(128, dtype=np.float16)
    return prm


# ---------------------------------------------------------------------------
# Builders
# ---------------------------------------------------------------------------

_CONST_N = [0]


def _load_const(nc, cp, ap, shape, dtype=F32):
    _CONST_N[0] += 1
    t = cp.tile(list(shape), dtype, tag=f"cst{_CONST_N[0]}")
    nc.sync.dma_start(t[:], ap)
    return t


def _type_tiles(cfg):
    """Yield (t, i_t, r0) for all node tiles: type, tile-in-type, local row base."""
    out = []
    for t, (ntile, b) in enumerate(
        zip((cfg.ntile_a, cfg.ntile_w, cfg.ntile_o), cfg.base_local)
    ):
        for i in range(ntile):
            out.append((t, i, b + i * P))
    return out


def _kqv_width(t):
    """Staging width in slots for type t: [xs | kd | q'_ets | v'_ets]."""
    return 6 if t == 0 else 4


def _kqv_tile(nc, pools, cfg, consts, t, xs_tile, stg, gi):
    """Next-layer projections for one tile via folded weights.

    Writes kd/q'/v' (slots 1..) of stg[:, gi, :]; slot 0 (xs) is written by
    the caller. One transpose + 1-2 bf16 matmuls + 1-2 bias-adds.
    """
    wp, pp_t = pools["wp"], pools["pp_t"]
    xsT_ps = pp_t.tile([P, P], F32, tag="tp_ps")
    nc.tensor.transpose(out=xsT_ps[:], in_=xs_tile[:], identity=consts["ident"][:])
    xsT = wp.tile([P, P], BF16, tag="xsT")
    nc.scalar.copy(out=xsT[:], in_=xsT_ps[:])
    _kqv_mms(nc, pools, cfg, consts, t, xsT[:], stg, gi)


def _kqv_mms(nc, pools, cfg, consts, t, xsT_ap, stg, gi):
    pp_mm = pools["pp_mm"]
    xsT = xsT_ap
    W = consts["Wcat"][t]
    B = consts["bcat"][t]
    nw = (_kqv_width(t) - 1) * HID  # matmul output width (kd + q + v slots)
    n1 = min(nw, 4 * HID)
    ps1 = pp_mm.tile([P, 4 * HID], F32, tag="mmk_ps")
    nc.tensor.matmul(out=ps1[:, :n1], lhsT=xsT[:], rhs=W[:, :n1],
                     start=True, stop=True)
    if _kqv_tile.bias_zero:
        # biases are all zero: plain copy, on the (less busy) scalar engine
        nc.scalar.copy(out=stg[:, gi, HID:HID + n1], in_=ps1[:, :n1])
    else:
        nc.vector.tensor_tensor(out=stg[:, gi, HID:HID + n1], in0=ps1[:, :n1],
                                in1=B[:, :n1], op=OP.add)
    if nw > n1:
        ps2 = pp_mm.tile([P, HID], F32, tag="mm_ps")
        nc.tensor.matmul(out=ps2[:], lhsT=xsT[:], rhs=W[:, n1:nw],
                         start=True, stop=True)
        if _kqv_tile.bias_zero:
            nc.vector.tensor_copy(out=stg[:, gi, HID + n1:HID + nw], in_=ps2[:])
        else:
            nc.vector.tensor_tensor(out=stg[:, gi, HID + n1:HID + nw],
                                    in0=ps2[:], in1=B[:, n1:nw], op=OP.add)


_kqv_tile.bias_zero = False


def _kqv_flush(nc, cfg, outs, t, i0, G, stg, r0base, write_xs=True):
    """DMA a group of G tiles' staged [xs|kd|q|v] slots to their tables."""
    kd_o, qp_o, ve_o = outs["kd"], outs["qp"], outs["ve"]
    xs_o = outs.get("xs")
    rt0 = i0 * P

    def wr(dst, lo):
        nc.gpsimd.dma_start(dst.rearrange("(g p) f -> p g f", p=P),
                            stg[:, :, lo * HID:(lo + 1) * HID])

    if write_xs and xs_o is not None:
        wr(xs_o[r0base:r0base + G * P, :], 0)
    wr(kd_o[r0base:r0base + G * P, :], 1)
    q_slots = ((0, 1), (2,), (3,))[t]
    v_slots = ((0, 1), (2,), (3,))[t]
    for j, sl in enumerate(q_slots):
        wr(qp_o[cfg.QB[sl] + rt0:cfg.QB[sl] + rt0 + G * P, :], 2 + j)
    for j, sl in enumerate(v_slots):
        wr(ve_o[cfg.QB[sl] + rt0:cfg.QB[sl] + rt0 + G * P, :],
           2 + len(q_slots) + j)


def _type_groups(cfg, tiles, G=4):
    """Split the ordered tile list into same-type groups of <= G."""
    groups = []
    cur = []
    for tt in tiles:
        if cur and (tt[0] != cur[0][0] or len(cur) == G):
            groups.append(cur)
            cur = []
        cur.append(tt)
    if cur:
        groups.append(cur)
    return groups


def build_l1(cfg):
    """Launch 1: input proj + relu -> xs1; kqv chain -> kd/q'/ve tables."""
    nc = bacc.Bacc("TRN2", target_bir_lowering=False, debug=False,
                   num_devices=cfg.C)
    c = cfg
    xa = nc.dram_tensor("xta", [64, c.nap], F16, kind="ExternalInput").ap()
    xw = nc.dram_tensor("xtw", [64, c.nwp], F16, kind="ExternalInput").ap()
    xo = nc.dram_tensor("xto", [64, c.nop], F16, kind="ExternalInput").ap()
    Win = nc.dram_tensor("Win", [3, 64, HID], F16, kind="ExternalInput").ap()
    binp = nc.dram_tensor("binT", [3, P, 1], F32, kind="ExternalInput").ap()
    Wcat_d = nc.dram_tensor("Wcat", [3, P, 5 * HID], BF16, kind="ExternalInput").ap()
    bcat_d = nc.dram_tensor("bcat", [3, P, 5 * HID], F32, kind="ExternalInput").ap()
    ident_d = nc.dram_tensor("ident", [P, P], F32, kind="ExternalInput").ap()
    identb_d = nc.dram_tensor("identb", [P, P], F16, kind="ExternalInput").ap()

    xs_o = nc.dram_tensor("xs", [c.LOCN, P], F32, kind="ExternalOutput").ap()
    kd_o = nc.dram_tensor("kd", [c.LOCN, P], F32, kind="ExternalOutput").ap()
    qp_o = nc.dram_tensor("qp", [c.QTOT, P], F32, kind="ExternalOutput").ap()
    ve_o = nc.dram_tensor("ve", [c.QTOT, P], F32, kind="ExternalOutput").ap()

    xin = (xa, xw, xo)
    with tile.TileContext(nc) as tc:
        with tc.tile_pool(name="consts", bufs=1) as cp, \
             tc.tile_pool(name="work", bufs=4) as wp, \
             tc.tile_pool(name="stgp", bufs=2) as sgp, \
             tc.tile_pool(name="ppt", bufs=2, space="PSUM") as pp_t, \
             tc.tile_pool(name="ppmm", bufs=2, space="PSUM") as pp_mm:
            consts = {
                "ident": _load_const(nc, cp, ident_d[:, :], (P, P)),
                "identb": _load_const(nc, cp, identb_d[:, :], (P, P), F16),
                "Win": [_load_const(nc, cp, Win[t], (64, HID), F16) for t in range(3)],
                "bin": [_load_const(nc, cp, binp[t], (P, 1)) for t in range(3)],
                "Wcat": [_load_const(nc, cp, Wcat_d[t], (P, 5 * HID), BF16) for t in range(3)],
                "bcat": [_load_const(nc, cp, bcat_d[t], (P, 5 * HID)) for t in range(3)],
            }
            pools = {"cp": cp, "wp": wp, "pp_t": pp_t, "pp_mm": pp_mm}
            outs = {"kd": kd_o, "qp": qp_o, "ve": ve_o, "xs": xs_o}
            for grp in _type_groups(c, _type_tiles(c)):
                t = grp[0][0]
                G = len(grp)
                i0 = grp[0][1]
                stg = sgp.tile([P, G, _kqv_width(t) * HID], F32, tag="stg")
                # one projection matmul + one relu for the whole group:
                # xsT = relu(Win.T @ xT) directly feature-major (no input
                # transpose; xT comes pre-transposed from the host).
                xt = wp.tile([64, G * P], F16, tag="x_in")
                nc.sync.dma_start(xt[:], xin[t][:, i0 * P:(i0 + G) * P])
                pj_ps = pp_mm.tile([P, G, P], F32, tag="mmk_ps")
                nc.tensor.matmul(
                    out=pj_ps[:].rearrange("p g n -> p (g n)"),
                    lhsT=consts["Win"][t][:], rhs=xt[:],
                    start=True, stop=True)
                xsT_all = wp.tile([P, G, P], F16, tag="xsT_all")
                nc.scalar.activation(out=xsT_all[:], in_=pj_ps[:], func=AF.Relu,
                                     bias=consts["bin"][t][:, 0:1])
                for gi, (_, i_t, r0) in enumerate(grp):
                    xs_ps = pp_t.tile([P, P], F16, tag="tpb_ps")
                    nc.tensor.transpose(out=xs_ps[:], in_=xsT_all[:, gi, :],
                                        identity=consts["identb"][:])
                    nc.scalar.copy(out=stg[:, gi, :HID], in_=xs_ps[:])
                    _kqv_mms(nc, pools, c, consts, t, xsT_all[:, gi, :], stg, gi)
                _kqv_flush(nc, c, outs, t, grp[0][1], G, stg, grp[0][2])
    nc.compile()
    return nc


def build_l23(cfg, NT, tws, last):
    """Launches 2/3: edge phase + node phase (+ head if last)."""
    nc = bacc.Bacc("TRN2", target_bir_lowering=False, debug=False,
                   num_devices=cfg.C)
    c = cfg
    kg_d = nc.dram_tensor("kg", [P, NT, HID], BF16, kind="ExternalInput").ap()
    vg_d = nc.dram_tensor("vg", [P, NT, HID], BF16, kind="ExternalInput").ap()
    qg_d = nc.dram_tensor("qg", [P, NT, HID], BF16, kind="ExternalInput").ap()
    NWIN_E = c.ntile_a if last else c.NWIN   # only a-dst windows feed the head
    LOCN_E = c.nap if last else c.LOCN
    xs_in = nc.dram_tensor("xsin", [LOCN_E, P], F32, kind="ExternalInput").ap()
    ti_t = nc.dram_tensor("dstlT", [P, NT], BF16, kind="ExternalInput").ap()
    iota_d = nc.dram_tensor("iota", [P, P], BF16, kind="ExternalInput").ap()
    ident_d = nc.dram_tensor("ident", [P, P], F32, kind="ExternalInput").ap()
    Wo_d = nc.dram_tensor("Wo", [3, P, HID], F32, kind="ExternalInput").ap()
    bo_d = nc.dram_tensor("bo", [3, P, HID], F32, kind="ExternalInput").ap()
    lng_d = nc.dram_tensor("lng", [3, P, HID], F32, kind="ExternalInput").ap()
    lnb_d = nc.dram_tensor("lnb", [3, P, HID], F32, kind="ExternalInput").ap()
    gs_d = nc.dram_tensor("gs", [3], F32, kind="ExternalInput").ap()  # unused on-device; values baked via bo/g mul
    if not last:
        Wcat_d = nc.dram_tensor("Wcat", [3, P, 5 * HID], BF16, kind="ExternalInput").ap()
        bcat_d = nc.dram_tensor("bcat", [3, P, 5 * HID], F32, kind="ExternalInput").ap()
    else:
        wh_d = nc.dram_tensor("whead", [P, 1], F32, kind="ExternalInput").ap()
        bh_d = nc.dram_tensor("bh", [P, 1], F32, kind="ExternalInput").ap()

    if not last:
        xs_o = nc.dram_tensor("xs", [c.LOCN, P], F32, kind="ExternalOutput").ap()
        kd_o = nc.dram_tensor("kd", [c.LOCN, P], F32, kind="ExternalOutput").ap()
        qp_o = nc.dram_tensor("qp", [c.QTOT, P], F32, kind="ExternalOutput").ap()
        ve_o = nc.dram_tensor("ve", [c.QTOT, P], F32, kind="ExternalOutput").ap()
    else:
        dl_o = nc.dram_tensor("delta", [c.nap, 1], F32, kind="ExternalOutput").ap()

    # gains folded on host: bo tile already contains g*b_o. g itself baked as consts below.
    g_vals = None  # set in kernel() via attribute hack? no: pass via build arg
    g_list = build_l23.g_list  # [3] floats for this layer

    with tile.TileContext(nc) as tc:
        with tc.tile_pool(name="consts", bufs=1) as cp, \
             tc.tile_pool(name="idx", bufs=2) as idxp, \
             tc.tile_pool(name="gat", bufs=2) as gp, \
             tc.tile_pool(name="ework", bufs=2) as ewp, \
             tc.tile_pool(name="nwork", bufs=3) as wp, \
             tc.tile_pool(name="stgp", bufs=2) as sgp, \
             tc.tile_pool(name="small", bufs=4) as sp, \
             tc.tile_pool(name="flush", bufs=2) as fp, \
             tc.tile_pool(name="dram", bufs=1, space="DRAM") as dp, \
             tc.tile_pool(name="ppe", bufs=2, space="PSUM") as pp_e, \
             tc.tile_pool(name="ppt", bufs=2, space="PSUM") as pp_t, \
             tc.tile_pool(name="ppmm", bufs=2, space="PSUM") as pp_mm:

            numer = dp.tile([LOCN_E, PAY], F32)
            eps_t = cp.tile([P, 1], F32, tag="lneps")
            nc.vector.memset(eps_t[:], 1e-5)

            consts = {
                "iota": _load_const(nc, cp, iota_d[:, :], (P, P), BF16),
                "ident": _load_const(nc, cp, ident_d[:, :], (P, P)),
                "Wo": [_load_const(nc, cp, Wo_d[t], (P, HID)) for t in range(3)],
                "bo": [_load_const(nc, cp, bo_d[t], (P, HID)) for t in range(3)],
                "lng": [_load_const(nc, cp, lng_d[t], (P, HID)) for t in range(3)],
                "lnb": [_load_const(nc, cp, lnb_d[t], (P, HID)) for t in range(3)],
            }
            if not last:
                consts.update({
                    "Wcat": [_load_const(nc, cp, Wcat_d[t], (P, 5 * HID), BF16) for t in range(3)],
                    "bcat": [_load_const(nc, cp, bcat_d[t], (P, 5 * HID)) for t in range(3)],
                })
            else:
                consts["whead"] = _load_const(nc, cp, wh_d[:, :], (P, 1))
                consts["bh"] = _load_const(nc, cp, bh_d[:, :], (P, 1))

            # ---------------- edge phase ----------------
            # streams are host-pre-gathered per edge (bf16). Windows are
            # grouped greedily into supers (<= SUPER_T tiles); per super one
            # DMA per stream + one batched op per DVE stage; scatter stays a
            # per-tile bf16 one-hot matmul into the window's PSUM accumulator.
            SUPER_T = 20
            supers = []  # (g0, [T_w...], w0)
            gtile = 0
            w = 0
            while w < NWIN_E:
                g0 = gtile
                ts = []
                w0 = w
                while w < NWIN_E and len(ts) < 4 and \
                        sum(ts) + int(tws[w]) <= SUPER_T:
                    ts.append(int(tws[w]))
                    gtile += int(tws[w])
                    w += 1
                supers.append((g0, ts, w0))
            for g0, ts, w0 in supers:
                TS = sum(ts)
                kgt = gp.tile([P, TS, HID], BF16, tag="kgt")
                nc.sync.dma_start(kgt[:], kg_d[:, g0:g0 + TS, :])
                vgt = gp.tile([P, TS, HID], BF16, tag="vgt")
                nc.sync.dma_start(vgt[:], vg_d[:, g0:g0 + TS, :])
                qgt = gp.tile([P, TS, HID], BF16, tag="qgt")
                nc.scalar.dma_start(qgt[:], qg_d[:, g0:g0 + TS, :])
                dstl = idxp.tile([P, TS], BF16, tag="dstl")
                nc.scalar.dma_start(dstl[:], ti_t[:, g0:g0 + TS])

                prod = ewp.tile([P, TS, HID], BF16, tag="prod")
                nc.vector.tensor_tensor(out=prod[:], in0=kgt[:], in1=qgt[:],
                                        op=OP.mult)
                alpha = ewp.tile([P, TS, H], F32, tag="alpha")
                nc.vector.tensor_reduce(
                    out=alpha[:],
                    in_=prod[:].rearrange("p t (h d) -> p t h d", h=H),
                    axis=mybir.AxisListType.X, op=OP.add)
                payload = ewp.tile([P, TS, PAY], BF16, tag="payload")
                ex = payload[:, :, HID:HID + H]
                nc.scalar.activation(out=ex, in_=alpha[:], func=AF.Exp)
                # vg stream columns are host-interleaved (d, h): the ex
                # broadcast lands on the middle axis and the innermost stays
                # contiguous, so the DVE runs this in 2x mode.
                nc.vector.tensor_tensor(
                    out=payload[:, :, :HID].rearrange("p t (d h) -> p t d h", h=H),
                    in0=vgt[:].rearrange("p t (d h) -> p t d h", h=H),
                    in1=ex[:, :, None, :].to_broadcast([P, TS, D, H]),
                    op=OP.mult)
                onehot = ewp.tile([P, TS, P], BF16, tag="onehot")
                nc.vector.tensor_tensor(
                    out=onehot[:],
                    in0=dstl[:, :, None].to_broadcast([P, TS, P]),
                    in1=consts["iota"][:, None, :].to_broadcast([P, TS, P]),
                    op=OP.is_equal)
                fl = fp.tile([P, len(ts), PAY], F32, tag="fl")
                toff = 0
                for wi, T in enumerate(ts):
                    psum_w = pp_e.tile([P, PAY], F32, tag="psw")
                    for t in range(toff, toff + T):
                        nc.tensor.matmul(out=psum_w[:], lhsT=onehot[:, t, :],
                                         rhs=payload[:, t, :],
                                         start=(t == toff),
                                         stop=(t == toff + T - 1))
                    nc.scalar.copy(out=fl[:, wi, :], in_=psum_w[:])
                    toff += T
                nc.sync.dma_start(
                    numer[w0 * P:(w0 + len(ts)) * P, :].rearrange(
                        "(t p) f -> p t f", p=P),
                    fl[:])

            # ---------------- node phase ----------------
            pools = {"cp": cp, "wp": wp, "pp_t": pp_t, "pp_mm": pp_mm}
            outs = None if last else {"kd": kd_o, "qp": qp_o, "ve": ve_o, "xs": xs_o}
            tiles = [x for x in _type_tiles(c) if (not last) or x[0] == 0]
            NTL = len(tiles)
            # pass 1: all tiles up through the skip-add + LN stats; gelu is the
            # only table-based ACT function here so the scalar engine loads
            # the gelu table once instead of thrashing gelu<->sqrt per tile.
            o1_all = cp.tile([P, NTL, HID], F16, tag="o1_all")
            mv_all = cp.tile([P, NTL, 2], F32, tag="mv_all")
            gi0 = 0
            for grp in _type_groups(c, tiles):
                t = grp[0][0]
                G = len(grp)
                r00 = grp[0][2]
                nm4 = wp.tile([P, G, PAY], F32, tag="nm")
                nc.scalar.dma_start(
                    nm4[:],
                    numer[r00:r00 + G * P, :].rearrange("(g p) f -> p g f", p=P))
                xs4 = wp.tile([P, G, HID], F32, tag="xs_ld")
                nc.scalar.dma_start(
                    xs4[:],
                    xs_in[r00:r00 + G * P, :].rearrange("(g p) f -> p g f", p=P))
                den = sp.tile([P, G, H], F32, tag="den")
                nc.vector.tensor_scalar_add(den[:], nm4[:, :, HID:HID + H], 1e-16)
                rec = sp.tile([P, G, H], F32, tag="rec")
                nc.vector.reciprocal(rec[:], den[:])
                agg = wp.tile([P, G, HID], F32, tag="agg")
                nc.vector.tensor_tensor(
                    out=agg[:].rearrange("p g (d h) -> p g d h", h=H),
                    in0=nm4[:, :, :HID].rearrange("p g (d h) -> p g d h", h=H),
                    in1=rec[:, :, None, :].to_broadcast([P, G, D, H]),
                    op=OP.mult)
                glu = wp.tile([P, G, HID], F32, tag="glu")
                if os.environ.get("HGT_BACKEND", "hw") == "sim":
                    # CoreSim has no Gelu LUT: tanh approximation (dev only)
                    t1 = wp.tile([P, G, HID], F32, tag="gelu_t1")
                    nc.vector.tensor_tensor(out=t1[:], in0=agg[:], in1=agg[:], op=OP.mult)
                    nc.vector.tensor_tensor(out=t1[:], in0=t1[:], in1=agg[:], op=OP.mult)
                    nc.vector.tensor_scalar(out=t1[:], in0=t1[:], scalar1=0.044715,
                                            scalar2=None, op0=OP.mult)
                    nc.vector.tensor_tensor(out=t1[:], in0=t1[:], in1=agg[:], op=OP.add)
                    nc.scalar.activation(out=t1[:], in_=t1[:], func=AF.Tanh,
                                         scale=0.7978845608028654)
                    nc.vector.tensor_scalar(out=t1[:], in0=t1[:], scalar1=0.5,
                                            scalar2=0.5, op0=OP.mult, op1=OP.add)
                    nc.vector.tensor_tensor(out=glu[:], in0=t1[:], in1=agg[:], op=OP.mult)
                else:
                    nc.scalar.activation(out=glu[:], in_=agg[:], func=AF.Gelu)
                ops4 = pp_mm.tile([P, G, HID], F32, tag="mmk_ps")
                for gi in range(G):
                    gluT_ps = pp_t.tile([P, P], F32, tag="tp_ps")
                    nc.tensor.transpose(out=gluT_ps[:], in_=glu[:, gi, :],
                                        identity=consts["ident"][:])
                    gluT = wp.tile([P, P], F32, tag="gluT")
                    nc.scalar.copy(out=gluT[:], in_=gluT_ps[:])
                    nc.tensor.matmul(out=ops4[:, gi, :], lhsT=gluT[:],
                                     rhs=consts["Wo"][t][:],
                                     start=True, stop=True)
                # o3 = g*o + (g*b_o) + (1-g)*xs  (bo const already has g*b_o)
                o1g = o1_all[:, gi0:gi0 + G, :]
                xs_s = wp.tile([P, G, HID], F32, tag="xs_s")
                nc.vector.tensor_scalar_mul(xs_s[:], xs4[:], float(1.0 - g_list[t]))
                if build_l23.bo_trivial:
                    nc.vector.tensor_tensor(out=o1g, in0=ops4[:], in1=xs_s[:],
                                            op=OP.add)
                else:
                    nc.vector.tensor_tensor(
                        out=o1g, in0=ops4[:],
                        in1=consts["bo"][t][:, None, :].to_broadcast([P, G, HID]),
                        op=OP.add)
                    nc.vector.tensor_tensor(out=o1g, in0=o1g, in1=xs_s[:], op=OP.add)
                for gi in range(G):
                    stats = sp.tile([P, nc.vector.BN_STATS_DIM], F32, tag="stats")
                    nc.vector.bn_stats(out=stats[:], in_=o1_all[:, gi0 + gi, :])
                    nc.vector.bn_aggr(out=mv_all[:, gi0 + gi, :], in_=stats[:])
                gi0 += G
            # one batched sqrt for all tiles' variances (single table load)
            rstd_all = cp.tile([P, NTL], F32, tag="rstd_all")
            nc.scalar.activation(out=rstd_all[:], in_=mv_all[:, :, 1],
                                 func=AF.Sqrt,
                                 bias=eps_t[:, 0:1])
            nc.vector.reciprocal(rstd_all[:], rstd_all[:])
            # pass 2: normalize + relu + next-layer projections (relu/copy are
            # in every ACT table set, so no further table switches).
            def _xh_relu(i, t, dst_ap):
                xh = wp.tile([P, HID], F32, tag="xh")
                nc.vector.tensor_scalar(
                    out=xh[:], in0=o1_all[:, i, :], scalar1=mv_all[:, i, 0:1],
                    scalar2=rstd_all[:, i:i + 1],
                    op0=OP.subtract, op1=OP.mult)
                if not build_l23.ln_trivial:
                    nc.vector.tensor_tensor(out=xh[:], in0=xh[:], in1=consts["lng"][t][:], op=OP.mult)
                    nc.vector.tensor_tensor(out=xh[:], in0=xh[:], in1=consts["lnb"][t][:], op=OP.add)
                nc.scalar.activation(out=dst_ap, in_=xh[:], func=AF.Relu)

            if not last:
                gi0 = 0
                for grp in _type_groups(c, tiles):
                    t = grp[0][0]
                    G = len(grp)
                    stg = sgp.tile([P, G, _kqv_width(t) * HID], F32, tag="stg")
                    for gi, (_, i_t, r0) in enumerate(grp):
                        _xh_relu(gi0 + gi, t, stg[:, gi, :HID])
                        _kqv_tile(nc, pools, c, consts, t, stg[:, gi, :HID], stg, gi)
                    _kqv_flush(nc, c, outs, t, grp[0][1], G, stg, grp[0][2])
                    gi0 += G
            else:
                for g0i in range(0, len(tiles), 4):
                    grp = tiles[g0i:g0i + 4]
                    G = len(grp)
                    dlst = sp.tile([P, 4, 1], F32, tag="dlst")
                    for gi, (t, i_t, r0) in enumerate(grp):
                        xs_new = wp.tile([P, HID], F32, tag="xs_new")
                        _xh_relu(g0i + gi, t, xs_new[:])
                        xnT_ps = pp_t.tile([P, P], F32, tag="tp_ps")
                        nc.tensor.transpose(out=xnT_ps[:], in_=xs_new[:], identity=consts["ident"][:])
                        xnT = wp.tile([P, P], F32, tag="xnT")
                        nc.scalar.copy(out=xnT[:], in_=xnT_ps[:])
                        d_ps = pp_mm.tile([P, 1], F32, tag="mm_ps")
                        nc.tensor.matmul(out=d_ps[:], lhsT=xnT[:], rhs=consts["whead"][:],
                                         start=True, stop=True)
                        nc.vector.tensor_tensor(out=dlst[:, gi, :], in0=d_ps[:],
                                                in1=consts["bh"][:], op=OP.add)
                    nc.gpsimd.dma_start(
                        dl_o[g0i * P:(g0i + G) * P, :].rearrange(
                            "(g p) f -> p g f", p=P),
                        dlst[:, :G, :])
    nc.compile()
    return nc


build_l23.g_list = None
build_l23.ln_trivial = False
build_l23.bo_trivial = False


# ---------------------------------------------------------------------------
# Runner
# ---------------------------------------------------------------------------

LAUNCH_TIMES_NS = []
TRACE_DIRS = []


def _run(nc, in_maps, cfg):
    backend = os.environ.get("HGT_BACKEND", "hw")
    if backend == "sim":
        from concourse.bass_interp import CoreSim
        results = []
        for m in in_maps:
            sim = CoreSim(nc, trace=False, require_finite=False, require_nnan=False)
            for k, v in m.items():
                sim.tensor(k)[:] = v
            sim.simulate(check_with_hw=False)
            out = {}
            for alloc in nc.m.functions[0].allocations:
                if isinstance(alloc, mybir.MemoryLocationSet) and alloc.kind == "ExternalOutput":
                    name = alloc.memorylocations[0].name
                    out[name] = sim.tensor(name).copy()
            results.append(out)
        return results
    else:
        from concourse.bass_utils import run_bass_kernel_spmd
        trace = os.environ.get("HGT_TRACE", "0") == "1"
        res = run_bass_kernel_spmd(nc, in_maps, core_ids=list(range(cfg.C)),
                                   trace=trace)
        if trace:
            LAUNCH_TIMES_NS.append(res.exec_time_ns)
            it = res.instructions_and_trace
            TRACE_DIRS.append(getattr(it, "trace_path", it))
        return res.results


# ---------------------------------------------------------------------------
# Main entry
# ---------------------------------------------------------------------------

def kernel(**inputs):
    cfg = Cfg()
    return _kernel_impl(cfg, inputs)


def _kernel_impl(cfg, inputs):
    c = cfg
    prm = prep_params(c, inputs)
    g = prep_graph(c, inputs)
    NT, tws = g["NT"], g["tws"]

    # ---- launch 1
    _kqv_tile.bias_zero = not np.asarray(inputs["b_kqv"])[0].any()
    nc1 = build_l1(c)
    in_maps = []
    xa = np.asarray(inputs["x_a"], np.float32)
    xw = np.asarray(inputs["x_w"], np.float32)
    xo = np.asarray(inputs["x_o"], np.float32)

    import ml_dtypes as _mld

    def padxT(x, n, npad):
        out = np.zeros((64, npad), np.float32)
        out[:, :n] = x.T
        return out.astype(np.float16)

    for cc in range(c.C):
        in_maps.append({
            "xta": padxT(xa[cc * c.nac:(cc + 1) * c.nac], c.nac, c.nap),
            "xtw": padxT(xw[cc * c.nwc:(cc + 1) * c.nwc], c.nwc, c.nwp),
            "xto": padxT(xo[cc * c.noc:(cc + 1) * c.noc], c.noc, c.nop),
            "Win": prm["WinB"], "binT": prm["binT"],
            "identb": prm["identb"],
            "Wcat": prm["Wcat0"], "bcat": prm["bcat0"],
            "ident": prm["ident"],
        })
    r1 = _run(nc1, in_maps, c)

    def assemble_tables(res):
        """Build global kd table + per-core q' tables + global ve table."""
        kd_tab = np.empty((c.KD_ROWS, HID), np.float32)
        kd_tab[-1] = 1.0
        ve_tab = np.empty((c.VE_ROWS, HID), np.float32)
        ve_tab[-1] = 0.0
        qp_tabs = []
        for cc in range(c.C):
            kd = res[cc]["kd"]
            ve = res[cc]["ve"]
            # kd local [a|w|o] -> global
            kd_tab[c.KOFF[0] + cc * c.nac:c.KOFF[0] + (cc + 1) * c.nac] = kd[:c.nac]
            kd_tab[c.KOFF[1] + cc * c.nwc:c.KOFF[1] + (cc + 1) * c.nwc] = \
                kd[c.base_local[1]:c.base_local[1] + c.nwc]
            kd_tab[c.KOFF[2] + cc * c.noc:c.KOFF[2] + (cc + 1) * c.noc] = \
                kd[c.base_local[2]:c.base_local[2] + c.noc]
            # ve local slots [a-et2, a-et3, w-et0, o-et1] -> global stacked
            ve_tab[c.VOFF[2] + cc * c.nac:c.VOFF[2] + (cc + 1) * c.nac] = \
                ve[c.QB[0]:c.QB[0] + c.nac]
            ve_tab[c.VOFF[3] + cc * c.nac:c.VOFF[3] + (cc + 1) * c.nac] = \
                ve[c.QB[1]:c.QB[1] + c.nac]
            ve_tab[c.VOFF[0] + cc * c.nwc:c.VOFF[0] + (cc + 1) * c.nwc] = \
                ve[c.QB[2]:c.QB[2] + c.nwc]
            ve_tab[c.VOFF[1] + cc * c.noc:c.VOFF[1] + (cc + 1) * c.noc] = \
                ve[c.QB[3]:c.QB[3] + c.noc]
            # pad-edge q rows are -8.0: with pad k rows = 1.0 the pad alpha is
            # 128 * -8 = -1024 (bf16-safe), exp -> 0.
            qp = np.vstack([res[cc]["qp"], np.full((1, HID), -8.0, np.float32)])
            qp_tabs.append(qp)
        return kd_tab, ve_tab, qp_tabs

    import ml_dtypes
    bf16 = ml_dtypes.bfloat16

    # ---- launches 2 and 3
    # last launch only needs a-dst windows (head reads only a-type nodes)
    NT_a = int(tws[:c.ntile_a].sum())
    res = r1
    for l, last in ((1, False), (2, True)):
        kd_tab, ve_tab, qp_tabs = assemble_tables(res)
        lay = l - 1  # layer params index: launch2 -> layer 0, launch3 -> layer 1
        build_l23.g_list = prm[f"g{lay}"]
        build_l23.bo_trivial = not np.asarray(inputs["b_o"])[lay].any()
        build_l23.ln_trivial = bool(
            (np.asarray(inputs["ln_g"])[lay] == 1).all()
            and not np.asarray(inputs["ln_b"])[lay].any())
        _kqv_tile.bias_zero = (not last) and \
            not np.asarray(inputs["b_kqv"])[lay + 1].any()
        NT_l = NT_a if last else NT
        nc = build_l23(c, NT_l, tws, last)
        in_maps = []
        for cc in range(c.C):
            ti = g["tile_idx"][cc][:NT_l]  # [NT_l, P, 4]
            kg_s = np.ascontiguousarray(
                kd_tab[ti[:, :, 0]].transpose(1, 0, 2)).astype(bf16)
            vg_s = np.ascontiguousarray(
                ve_tab[ti[:, :, 1]][:, :, DH_PERM].transpose(1, 0, 2)).astype(bf16)
            qg_s = np.ascontiguousarray(
                qp_tabs[cc][ti[:, :, 2]].transpose(1, 0, 2)).astype(bf16)
            dstlT = np.ascontiguousarray(ti[:, :, 3].T.astype(bf16))
            m = {
                "kg": kg_s, "vg": vg_s, "qg": qg_s,
                "xsin": res[cc]["xs"][:c.nap] if last else res[cc]["xs"],
                "dstlT": dstlT,
                "iota": prm["iota"], "ident": prm["ident"],
                "Wo": prm[f"Wo{lay}"], "bo": prm[f"bo{lay}"],
                "lng": prm[f"lng{lay}"], "lnb": prm[f"lnb{lay}"],
                "gs": prm[f"g{lay}"],
            }
            if not last:
                m.update({"Wcat": prm[f"Wcat{lay + 1}"], "bcat": prm[f"bcat{lay + 1}"]})
            else:
                m.update({"whead": prm["whead"], "bh": prm["bh"]})
            in_maps.append(m)
        res = _run(nc, in_maps, c)

    out = np.concatenate([res[cc]["delta"][:c.nac, 0] for cc in range(c.C)])
    return out.astype(np.float32)



# revision 29
# speedup vs baseline: 3.9905x; 1.0583x over previous
"""HGT regressor on 8 Trainium2 NeuronCores (Bass/Tile).

Strategy (graph/data parallel, hint-following):
  - Nodes of each type are partitioned contiguously across the 8 cores
    (a: 12500/core, w: 2500/core, o: 6250/core). Each core owns the edges
    whose *destination* lies in its node shard.
  - Per layer, each core computes K = kqv[:, :128] (raw) and the per-edge-type
    source-side V transform (m_rel folded at source) plus the destination-side
    Q transform (a_rel * p_rel * scale folded into Q) for its own nodes only.
  - The full K / V_et tables are exchanged between layer launches via the host
    (replicated to all cores), i.e. host-mediated all-gather. Q' stays local.
  - Edge phase per core: edges sorted by local destination row, grouped into
    128-node windows; per 128-edge tile: indirect-DMA gathers of K[src],
    V_et[src], Q'_et[dst]; alpha = sum_h(K*Q'); ex = exp(alpha); payload
    [ex*V | ex] is scatter-added into a PSUM window accumulator via a
    one-hot matmul; windows flush densely to a numer/den table in DRAM.
  - Node phase per core: agg = numer/den, gelu, W_o, gated skip, LayerNorm,
    relu, then next-layer projections (or the scalar head in the last layer).
  - Softmax needs no running max: alpha = q'k with these parameter scales is
    O(1); exp cannot overflow, and softmax is shift-invariant anyway.
"""
import os
import sys

sys.path.insert(0, "/opt/trn_rl_repo")

import numpy as np

import concourse.bass as bass
import concourse.mybir as mybir
import concourse.tile as tile
from concourse import bacc

P = 128
H, D, HID = 4, 32, 128
PAY = HID + H  # 132
F32 = mybir.dt.float32
F16 = mybir.dt.float16
BF16 = mybir.dt.bfloat16
I32 = mybir.dt.int32
AF = mybir.ActivationFunctionType
OP = mybir.AluOpType


def _ceil(a, b):
    return (a + b - 1) * b // b if False else -(-a // b) * b


def cdiv(a, b):
    return -(-a // b)


class Cfg:
    """All sizes derived from problem scale; supports mini-scale testing."""

    def __init__(self, NA=100000, NWK=20000, NO=50000, E=150000, C=8):
        self.NA, self.NWK, self.NO, self.E, self.C = NA, NWK, NO, E, C
        assert NA % C == 0 and NWK % C == 0 and NO % C == 0
        self.nac, self.nwc, self.noc = NA // C, NWK // C, NO // C
        self.nap, self.nwp, self.nop = (
            cdiv(self.nac, P) * P,
            cdiv(self.nwc, P) * P,
            cdiv(self.noc, P) * P,
        )
        # local node-row layout (numer/xs/kd rows): [a | w | o], each padded
        self.base_local = (0, self.nap, self.nap + self.nwp)
        self.LOCN = self.nap + self.nwp + self.nop
        self.NWIN = self.LOCN // P
        # per-type tile counts
        self.ntile_a, self.ntile_w, self.ntile_o = (
            self.nap // P,
            self.nwp // P,
            self.nop // P,
        )
        # q' local layout: slots [a-et0, a-et1, w-et2, o-et3]
        self.QB = (0, self.nap, 2 * self.nap, 2 * self.nap + self.nwp)
        self.QTOT = 2 * self.nap + self.nwp + self.nop
        # ve local layout (same bases): slots [a-et2, a-et3, w-et0, o-et1]
        # global kd table layout: [a 0..NA | w | o] + trash
        self.KOFF = (0, NA, NA + NWK)
        self.KD_ROWS = NA + NWK + NO + 1
        # global stacked ve table: [et0 w | et1 o | et2 a | et3 a] + trash
        self.VOFF = (0, NWK, NWK + NO, NWK + NO + NA)
        self.VE_ROWS = NWK + NO + 2 * NA + 1


# edge types: (src_type, dst_type)
ETYPES = ((1, 0), (2, 0), (0, 1), (0, 2))

# (d, h)-interleaved feature order: new col d*H+h <- old col h*D+d
DH_PERM = np.arange(HID).reshape(H, D).T.flatten()


# ---------------------------------------------------------------------------
# Host-side preprocessing
# ---------------------------------------------------------------------------

def prep_graph(cfg, inputs):
    """Compute per-core edge tile indices. Shared across both layers.

    Returns dict with:
      NT: static tile count (same all cores)
      tws: [NWIN] tiles per window (static across cores)
      tile_idx: [C][NT, P, 4] int32  (kidx, vidx, qidx, dst_local)
    """
    c = cfg
    edges = []  # per et: (src, dst)
    for name_s, name_d in (("src_wa", "dst_wa"), ("src_oa", "dst_oa"),
                           ("src_aw", "dst_aw"), ("src_ao", "dst_ao")):
        edges.append((np.asarray(inputs[name_s]), np.asarray(inputs[name_d])))

    shard_n = (c.nac, c.nwc, c.noc)
    # concat all ets with global indices
    K_TRASH = c.KD_ROWS - 1
    V_TRASH = c.VE_ROWS - 1
    Q_TRASH = c.QTOT  # row appended by host to the q' table

    all_core = []
    koff_by_et = (c.KOFF[1], c.KOFF[2], c.KOFF[0], c.KOFF[0])  # src type offset in kd
    for et, (st, dt) in enumerate(ETYPES):
        src, dst = edges[et]
        kidx = koff_by_et[et] + src
        vidx = c.VOFF[et] + src
        core = dst // shard_n[dt]
        dloc = dst - core * shard_n[dt]  # dst index within its type shard
        # local numer row / q' row
        tb = (c.base_local[0], c.base_local[1], c.base_local[2])[dt]
        row = tb + dloc
        qslot = {0: 0, 1: 1, 2: 2, 3: 3}[et]
        qidx = c.QB[qslot] + dloc
        all_core.append((core, row, kidx, vidx, qidx))

    core_cat = np.concatenate([a[0] for a in all_core])
    row_cat = np.concatenate([a[1] for a in all_core])
    k_cat = np.concatenate([a[2] for a in all_core])
    v_cat = np.concatenate([a[3] for a in all_core])
    q_cat = np.concatenate([a[4] for a in all_core])

    # per-core, per-window edge counts -> static tile structure
    win_cat = row_cat // P
    counts = np.zeros((c.C, c.NWIN), np.int64)
    for cc in range(c.C):
        m = core_cat == cc
        counts[cc] = np.bincount(win_cat[m], minlength=c.NWIN)
    tws = np.maximum(cdiv(counts.max(axis=0), P), 1)  # >=1 tile per window
    NT = int(tws.sum())
    tile_base = np.zeros(c.NWIN, np.int64)
    tile_base[1:] = np.cumsum(tws)[:-1]

    tile_idx = np.zeros((c.C, NT, P, 4), np.int32)
    # fill pads with trash rows -> ex = 0 contributions
    tile_idx[:, :, :, 0] = K_TRASH
    tile_idx[:, :, :, 1] = V_TRASH
    tile_idx[:, :, :, 2] = Q_TRASH
    tile_idx[:, :, :, 3] = 0
    for cc in range(c.C):
        m = core_cat == cc
        rows = row_cat[m]
        order = np.argsort(rows, kind="stable")
        rows = rows[order]
        ks, vs, qs = k_cat[m][order], v_cat[m][order], q_cat[m][order]
        wins = rows // P
        dstl = rows % P
        # position within window
        wstart = np.searchsorted(wins, np.arange(c.NWIN), side="left")
        pos = np.arange(rows.size) - wstart[wins]
        slot_t = pos // P   # tile within window
        slot_p = pos % P    # partition
        gt = tile_base[wins] + slot_t  # global tile id
        tile_idx[cc, gt, slot_p, 0] = ks
        tile_idx[cc, gt, slot_p, 1] = vs
        tile_idx[cc, gt, slot_p, 2] = qs
        tile_idx[cc, gt, slot_p, 3] = dstl
    return {"NT": NT, "tws": tws.astype(np.int64), "tile_idx": tile_idx,
            "tile_base": tile_base}


def blockdiag(M):
    out = np.zeros((HID, HID), np.float32)
    for h in range(H):
        out[h * D:(h + 1) * D, h * D:(h + 1) * D] = M[h]
    return out


def prep_params(cfg, inputs):
    """Fold and lay out all parameters (host, tiny)."""
    scale = np.float32(1.0 / np.sqrt(D))
    a_rel = np.asarray(inputs["a_rel"])
    m_rel = np.asarray(inputs["m_rel"])
    p_rel = np.asarray(inputs["p_rel"])
    import ml_dtypes as _mld
    prm = {}
    rep = lambda v, w: np.broadcast_to(np.asarray(v, np.float32)[None, :], (P, w)).copy()
    for l in range(2):
        BDaT, BDm = [], []
        for et in range(4):
            a_eff = a_rel[l, et] * (p_rel[l, et] * scale)[:, None, None]
            BDaT.append(blockdiag(a_eff).T.copy())
            BDm.append(blockdiag(m_rel[l, et]))
        prm[f"BDaT{l}"] = np.stack(BDaT)  # [4,128,128]
        prm[f"BDm{l}"] = np.stack(BDm)
        prm[f"Wkqv{l}"] = np.asarray(inputs["W_kqv"])[l]       # [3,128,384]
        prm[f"bkqv{l}"] = np.stack([rep(np.asarray(inputs["b_kqv"])[l, t], 3 * HID) for t in range(3)])
        # folded projection weights: per type, slots [kd | q'_ets | v'_ets]
        # q' = xs @ (Wq @ BDaT_et), v' = xs @ (Wv @ BDm_et); biases likewise.
        import ml_dtypes
        q_ets = ((0, 1), (2,), (3,))
        v_ets = ((2, 3), (0,), (1,))
        Wcat = np.zeros((3, HID, 5 * HID), np.float32)
        bcat = np.zeros((3, P, 5 * HID), np.float32)
        for t in range(3):
            Wk = np.asarray(inputs["W_kqv"])[l, t]
            bk = np.asarray(inputs["b_kqv"])[l, t]
            cols = [Wk[:, :HID]]
            bs = [bk[:HID]]
            for et in q_ets[t]:
                cols.append(Wk[:, HID:2 * HID] @ prm[f"BDaT{l}"][et])
                bs.append(bk[HID:2 * HID] @ prm[f"BDaT{l}"][et])
            for et in v_ets[t]:
                cols.append(Wk[:, 2 * HID:] @ prm[f"BDm{l}"][et])
                bs.append(bk[2 * HID:] @ prm[f"BDm{l}"][et])
            cat = np.concatenate(cols, axis=1)
            Wcat[t, :, :cat.shape[1]] = cat
            bcat[t, :, :cat.shape[1]] = np.concatenate(bs)[None, :]
        prm[f"Wcat{l}"] = Wcat.astype(ml_dtypes.bfloat16)
        prm[f"bcat{l}"] = bcat
        g = 1.0 / (1.0 + np.exp(-np.asarray(inputs["skip_p"], np.float64)))  # [2,3]
        # skip gain g folded into Wo; rows permuted to the (d, h)-interleaved
        # feature order the aggregated V stream arrives in.
        prm[f"Wo{l}"] = (np.asarray(inputs["W_o"])[l]
                         * g[l].astype(np.float32)[:, None, None])[
                             :, DH_PERM, :]  # [3,128,128]
        prm[f"g{l}"] = g[l].astype(np.float32)
        prm[f"bo{l}"] = np.stack([rep(np.asarray(inputs["b_o"])[l, t] * g[l, t], HID) for t in range(3)])
        prm[f"lng{l}"] = np.stack([rep(np.asarray(inputs["ln_g"])[l, t], HID) for t in range(3)])
        prm[f"lnb{l}"] = np.stack([rep(np.asarray(inputs["ln_b"])[l, t], HID) for t in range(3)])
    # input proj, padded to 128 contraction
    W_in = np.asarray(inputs["W_in"])  # [3,64,128]
    Wp = np.zeros((3, 128, HID), np.float32)
    Wp[:, :64, :] = W_in
    prm["Win"] = Wp
    prm["bin"] = np.stack([rep(np.asarray(inputs["b_in"])[t], HID) for t in range(3)])
    prm["WinB"] = W_in.astype(np.float16)  # [3,64,128]
    prm["binT"] = np.ascontiguousarray(
        np.asarray(inputs["b_in"], np.float32)[:, :, None])  # [3,128,1]
    prm["whead"] = np.asarray(inputs["w_head"], np.float32)  # [128,1]
    prm["bh"] = np.full((P, 1), float(np.asarray(inputs["b_head"])[0] + np.asarray(inputs["base"])[0]), np.float32)
    prm["iota"] = np.broadcast_to(
        np.arange(128, dtype=np.float32).astype(_mld.bfloat16)[None, :],
        (P, 128)).copy()
    prm["ident"] = np.eye(128, dtype=np.float32)
    prm["identb"] = np.eye(128, dtype=np.float16)
    return prm


# ---------------------------------------------------------------------------
# Builders
# ---------------------------------------------------------------------------

_CONST_N = [0]


def _load_const(nc, cp, ap, shape, dtype=F32):
    _CONST_N[0] += 1
    t = cp.tile(list(shape), dtype, tag=f"cst{_CONST_N[0]}")
    nc.sync.dma_start(t[:], ap)
    return t


def _type_tiles(cfg):
    """Yield (t, i_t, r0) for all node tiles: type, tile-in-type, local row base."""
    out = []
    for t, (ntile, b) in enumerate(
        zip((cfg.ntile_a, cfg.ntile_w, cfg.ntile_o), cfg.base_local)
    ):
        for i in range(ntile):
            out.append((t, i, b + i * P))
    return out


def _kqv_width(t):
    """Staging width in slots for type t: [xs | kd | q'_ets | v'_ets]."""
    return 6 if t == 0 else 4


def _kqv_tile(nc, pools, cfg, consts, t, xs_tile, stg, gi):
    """Next-layer projections for one tile via folded weights.

    Writes kd/q'/v' (slots 1..) of stg[:, gi, :]; slot 0 (xs) is written by
    the caller. One transpose + 1-2 bf16 matmuls + 1-2 bias-adds.
    """
    wp, pp_t = pools["wp"], pools["pp_t"]
    xsT_ps = pp_t.tile([P, P], F32, tag="tp_ps")
    nc.tensor.transpose(out=xsT_ps[:], in_=xs_tile[:], identity=consts["ident"][:])
    xsT = wp.tile([P, P], BF16, tag="xsT")
    nc.scalar.copy(out=xsT[:], in_=xsT_ps[:])
    _kqv_mms(nc, pools, cfg, consts, t, xsT[:], stg, gi)


def _kqv_mms(nc, pools, cfg, consts, t, xsT_ap, stg, gi):
    pp_mm = pools["pp_mm"]
    xsT = xsT_ap
    W = consts["Wcat"][t]
    B = consts["bcat"][t]
    nw = (_kqv_width(t) - 1) * HID  # matmul output width (kd + q + v slots)
    n1 = min(nw, 4 * HID)
    ps1 = pp_mm.tile([P, 4 * HID], F32, tag="mmk_ps")
    nc.tensor.matmul(out=ps1[:, :n1], lhsT=xsT[:], rhs=W[:, :n1],
                     start=True, stop=True)
    if _kqv_tile.bias_zero:
        # biases are all zero: plain copy, on the (less busy) scalar engine
        nc.scalar.copy(out=stg[:, gi, HID:HID + n1], in_=ps1[:, :n1])
    else:
        nc.vector.tensor_tensor(out=stg[:, gi, HID:HID + n1], in0=ps1[:, :n1],
                                in1=B[:, :n1], op=OP.add)
    if nw > n1:
        ps2 = pp_mm.tile([P, HID], F32, tag="mm_ps")
        nc.tensor.matmul(out=ps2[:], lhsT=xsT[:], rhs=W[:, n1:nw],
                         start=True, stop=True)
        if _kqv_tile.bias_zero:
            nc.vector.tensor_copy(out=stg[:, gi, HID + n1:HID + nw], in_=ps2[:])
        else:
            nc.vector.tensor_tensor(out=stg[:, gi, HID + n1:HID + nw],
                                    in0=ps2[:], in1=B[:, n1:nw], op=OP.add)


_kqv_tile.bias_zero = False


def _kqv_flush(nc, cfg, outs, t, i0, G, stg, r0base, write_xs=True):
    """DMA a group of G tiles' staged [xs|kd|q|v] slots to their tables."""
    kd_o, qp_o, ve_o = outs["kd"], outs["qp"], outs["ve"]
    xs_o = outs.get("xs")
    rt0 = i0 * P

    def wr(dst, lo):
        nc.gpsimd.dma_start(dst.rearrange("(g p) f -> p g f", p=P),
                            stg[:, :, lo * HID:(lo + 1) * HID])

    if write_xs and xs_o is not None:
        wr(xs_o[r0base:r0base + G * P, :], 0)
    wr(kd_o[r0base:r0base + G * P, :], 1)
    q_slots = ((0, 1), (2,), (3,))[t]
    v_slots = ((0, 1), (2,), (3,))[t]
    for j, sl in enumerate(q_slots):
        wr(qp_o[cfg.QB[sl] + rt0:cfg.QB[sl] + rt0 + G * P, :], 2 + j)
    for j, sl in enumerate(v_slots):
        wr(ve_o[cfg.QB[sl] + rt0:cfg.QB[sl] + rt0 + G * P, :],
           2 + len(q_slots) + j)


def _type_groups(cfg, tiles, G=4):
    """Split the ordered tile list into same-type groups of <= G."""
    groups = []
    cur = []
    for tt in tiles:
        if cur and (tt[0] != cur[0][0] or len(cur) == G):
            groups.append(cur)
            cur = []
        cur.append(tt)
    if cur:
        groups.append(cur)
    return groups


def build_l1(cfg):
    """Launch 1: input proj + relu -> xs1; kqv chain -> kd/q'/ve tables."""
    nc = bacc.Bacc("TRN2", target_bir_lowering=False, debug=False,
                   num_devices=cfg.C)
    c = cfg
    xa = nc.dram_tensor("xta", [64, c.nap], F16, kind="ExternalInput").ap()
    xw = nc.dram_tensor("xtw", [64, c.nwp], F16, kind="ExternalInput").ap()
    xo = nc.dram_tensor("xto", [64, c.nop], F16, kind="ExternalInput").ap()
    Win = nc.dram_tensor("Win", [3, 64, HID], F16, kind="ExternalInput").ap()
    binp = nc.dram_tensor("binT", [3, P, 1], F32, kind="ExternalInput").ap()
    Wcat_d = nc.dram_tensor("Wcat", [3, P, 5 * HID], BF16, kind="ExternalInput").ap()
    bcat_d = nc.dram_tensor("bcat", [3, P, 5 * HID], F32, kind="ExternalInput").ap()
    ident_d = nc.dram_tensor("ident", [P, P], F32, kind="ExternalInput").ap()
    identb_d = nc.dram_tensor("identb", [P, P], F16, kind="ExternalInput").ap()

    xs_o = nc.dram_tensor("xs", [c.LOCN, P], F32, kind="ExternalOutput").ap()
    kd_o = nc.dram_tensor("kd", [c.LOCN, P], F32, kind="ExternalOutput").ap()
    qp_o = nc.dram_tensor("qp", [c.QTOT, P], F32, kind="ExternalOutput").ap()
    ve_o = nc.dram_tensor("ve", [c.QTOT, P], F32, kind="ExternalOutput").ap()

    xin = (xa, xw, xo)
    with tile.TileContext(nc) as tc:
        with tc.tile_pool(name="consts", bufs=1) as cp, \
             tc.tile_pool(name="work", bufs=4) as wp, \
             tc.tile_pool(name="stgp", bufs=2) as sgp, \
             tc.tile_pool(name="ppt", bufs=2, space="PSUM") as pp_t, \
             tc.tile_pool(name="ppmm", bufs=2, space="PSUM") as pp_mm:
            consts = {
                "ident": _load_const(nc, cp, ident_d[:, :], (P, P)),
                "identb": _load_const(nc, cp, identb_d[:, :], (P, P), F16),
                "Win": [_load_const(nc, cp, Win[t], (64, HID), F16) for t in range(3)],
                "bin": [_load_const(nc, cp, binp[t], (P, 1)) for t in range(3)],
                "Wcat": [_load_const(nc, cp, Wcat_d[t], (P, 5 * HID), BF16) for t in range(3)],
                "bcat": [_load_const(nc, cp, bcat_d[t], (P, 5 * HID)) for t in range(3)],
            }
            pools = {"cp": cp, "wp": wp, "pp_t": pp_t, "pp_mm": pp_mm}
            outs = {"kd": kd_o, "qp": qp_o, "ve": ve_o, "xs": xs_o}
            for grp in _type_groups(c, _type_tiles(c)):
                t = grp[0][0]
                G = len(grp)
                i0 = grp[0][1]
                stg = sgp.tile([P, G, _kqv_width(t) * HID], F32, tag="stg")
                # one projection matmul + one relu for the whole group:
                # xsT = relu(Win.T @ xT) directly feature-major (no input
                # transpose; xT comes pre-transposed from the host).
                xt = wp.tile([64, G * P], F16, tag="x_in")
                nc.sync.dma_start(xt[:], xin[t][:, i0 * P:(i0 + G) * P])
                pj_ps = pp_mm.tile([P, G, P], F32, tag="mmk_ps")
                nc.tensor.matmul(
                    out=pj_ps[:].rearrange("p g n -> p (g n)"),
                    lhsT=consts["Win"][t][:], rhs=xt[:],
                    start=True, stop=True)
                xsT_all = wp.tile([P, G, P], F16, tag="xsT_all")
                nc.scalar.activation(out=xsT_all[:], in_=pj_ps[:], func=AF.Relu,
                                     bias=consts["bin"][t][:, 0:1])
                for gi, (_, i_t, r0) in enumerate(grp):
                    xs_ps = pp_t.tile([P, P], F16, tag="tpb_ps")
                    nc.tensor.transpose(out=xs_ps[:], in_=xsT_all[:, gi, :],
                                        identity=consts["identb"][:])
                    nc.scalar.copy(out=stg[:, gi, :HID], in_=xs_ps[:])
                    _kqv_mms(nc, pools, c, consts, t, xsT_all[:, gi, :], stg, gi)
                _kqv_flush(nc, c, outs, t, grp[0][1], G, stg, grp[0][2])
    nc.compile()
    return nc


def build_l23(cfg, NT, tws, last):
    """Launches 2/3: edge phase + node phase (+ head if last)."""
    nc = bacc.Bacc("TRN2", target_bir_lowering=False, debug=False,
                   num_devices=cfg.C)
    c = cfg
    kg_d = nc.dram_tensor("kg", [P, NT, HID], BF16, kind="ExternalInput").ap()
    vg_d = nc.dram_tensor("vg", [P, NT, HID], BF16, kind="ExternalInput").ap()
    qg_d = nc.dram_tensor("qg", [P, NT, HID], BF16, kind="ExternalInput").ap()
    NWIN_E = c.ntile_a if last else c.NWIN   # only a-dst windows feed the head
    LOCN_E = c.nap if last else c.LOCN
    xs_in = nc.dram_tensor("xsin", [LOCN_E, P], F32, kind="ExternalInput").ap()
    ti_t = nc.dram_tensor("dstlT", [P, NT], BF16, kind="ExternalInput").ap()
    iota_d = nc.dram_tensor("iota", [P, P], BF16, kind="ExternalInput").ap()
    ident_d = nc.dram_tensor("ident", [P, P], F32, kind="ExternalInput").ap()
    Wo_d = nc.dram_tensor("Wo", [3, P, HID], F32, kind="ExternalInput").ap()
    bo_d = nc.dram_tensor("bo", [3, P, HID], F32, kind="ExternalInput").ap()
    lng_d = nc.dram_tensor("lng", [3, P, HID], F32, kind="ExternalInput").ap()
    lnb_d = nc.dram_tensor("lnb", [3, P, HID], F32, kind="ExternalInput").ap()
    gs_d = nc.dram_tensor("gs", [3], F32, kind="ExternalInput").ap()  # unused on-device; values baked via bo/g mul
    if not last:
        Wcat_d = nc.dram_tensor("Wcat", [3, P, 5 * HID], BF16, kind="ExternalInput").ap()
        bcat_d = nc.dram_tensor("bcat", [3, P, 5 * HID], F32, kind="ExternalInput").ap()
    else:
        wh_d = nc.dram_tensor("whead", [P, 1], F32, kind="ExternalInput").ap()
        bh_d = nc.dram_tensor("bh", [P, 1], F32, kind="ExternalInput").ap()

    if not last:
        xs_o = nc.dram_tensor("xs", [c.LOCN, P], F32, kind="ExternalOutput").ap()
        kd_o = nc.dram_tensor("kd", [c.LOCN, P], F32, kind="ExternalOutput").ap()
        qp_o = nc.dram_tensor("qp", [c.QTOT, P], F32, kind="ExternalOutput").ap()
        ve_o = nc.dram_tensor("ve", [c.QTOT, P], F32, kind="ExternalOutput").ap()
    else:
        dl_o = nc.dram_tensor("delta", [c.nap, 1], F32, kind="ExternalOutput").ap()

    # gains folded on host: bo tile already contains g*b_o. g itself baked as consts below.
    g_vals = None  # set in kernel() via attribute hack? no: pass via build arg
    g_list = build_l23.g_list  # [3] floats for this layer

    with tile.TileContext(nc) as tc:
        with tc.tile_pool(name="consts", bufs=1) as cp, \
             tc.tile_pool(name="idx", bufs=2) as idxp, \
             tc.tile_pool(name="gat", bufs=2) as gp, \
             tc.tile_pool(name="ework", bufs=2) as ewp, \
             tc.tile_pool(name="nwork", bufs=3) as wp, \
             tc.tile_pool(name="stgp", bufs=2) as sgp, \
             tc.tile_pool(name="small", bufs=4) as sp, \
             tc.tile_pool(name="flush", bufs=2) as fp, \
             tc.tile_pool(name="dram", bufs=1, space="DRAM") as dp, \
             tc.tile_pool(name="ppe", bufs=2, space="PSUM") as pp_e, \
             tc.tile_pool(name="ppt", bufs=2, space="PSUM") as pp_t, \
             tc.tile_pool(name="ppmm", bufs=2, space="PSUM") as pp_mm:

            numer = dp.tile([LOCN_E, PAY], F16)
            eps_t = cp.tile([P, 1], F32, tag="lneps")
            nc.vector.memset(eps_t[:], 1e-5)

            consts = {
                "iota": _load_const(nc, cp, iota_d[:, :], (P, P), BF16),
                "ident": _load_const(nc, cp, ident_d[:, :], (P, P)),
                "Wo": [_load_const(nc, cp, Wo_d[t], (P, HID)) for t in range(3)],
                "bo": [_load_const(nc, cp, bo_d[t], (P, HID)) for t in range(3)],
                "lng": [_load_const(nc, cp, lng_d[t], (P, HID)) for t in range(3)],
                "lnb": [_load_const(nc, cp, lnb_d[t], (P, HID)) for t in range(3)],
            }
            if not last:
                consts.update({
                    "Wcat": [_load_const(nc, cp, Wcat_d[t], (P, 5 * HID), BF16) for t in range(3)],
                    "bcat": [_load_const(nc, cp, bcat_d[t], (P, 5 * HID)) for t in range(3)],
                })
            else:
                consts["whead"] = _load_const(nc, cp, wh_d[:, :], (P, 1))
                consts["bh"] = _load_const(nc, cp, bh_d[:, :], (P, 1))

            # ---------------- edge phase ----------------
            # streams are host-pre-gathered per edge (bf16). Windows are
            # grouped greedily into supers (<= SUPER_T tiles); per super one
            # DMA per stream + one batched op per DVE stage; scatter stays a
            # per-tile bf16 one-hot matmul into the window's PSUM accumulator.
            SUPER_T = 24
            supers = []  # (g0, [T_w...], w0)
            gtile = 0
            w = 0
            while w < NWIN_E:
                g0 = gtile
                ts = []
                w0 = w
                while w < NWIN_E and len(ts) < 5 and \
                        sum(ts) + int(tws[w]) <= SUPER_T:
                    ts.append(int(tws[w]))
                    gtile += int(tws[w])
                    w += 1
                supers.append((g0, ts, w0))
            for g0, ts, w0 in supers:
                TS = sum(ts)
                kgt = gp.tile([P, TS, HID], BF16, tag="kgt")
                nc.sync.dma_start(kgt[:], kg_d[:, g0:g0 + TS, :])
                vgt = gp.tile([P, TS, HID], BF16, tag="vgt")
                nc.sync.dma_start(vgt[:], vg_d[:, g0:g0 + TS, :])
                qgt = gp.tile([P, TS, HID], BF16, tag="qgt")
                nc.scalar.dma_start(qgt[:], qg_d[:, g0:g0 + TS, :])
                dstl = idxp.tile([P, TS], BF16, tag="dstl")
                nc.scalar.dma_start(dstl[:], ti_t[:, g0:g0 + TS])

                prod = ewp.tile([P, TS, HID], BF16, tag="prod")
                nc.vector.tensor_tensor(out=prod[:], in0=kgt[:], in1=qgt[:],
                                        op=OP.mult)
                alpha = ewp.tile([P, TS, H], F32, tag="alpha")
                nc.vector.tensor_reduce(
                    out=alpha[:],
                    in_=prod[:].rearrange("p t (h d) -> p t h d", h=H),
                    axis=mybir.AxisListType.X, op=OP.add)
                payload = ewp.tile([P, TS, PAY], BF16, tag="payload")
                ex = payload[:, :, HID:HID + H]
                nc.scalar.activation(out=ex, in_=alpha[:], func=AF.Exp)
                # vg stream columns are host-interleaved (d, h): the ex
                # broadcast lands on the middle axis and the innermost stays
                # contiguous, so the DVE runs this in 2x mode.
                nc.vector.tensor_tensor(
                    out=payload[:, :, :HID].rearrange("p t (d h) -> p t d h", h=H),
                    in0=vgt[:].rearrange("p t (d h) -> p t d h", h=H),
                    in1=ex[:, :, None, :].to_broadcast([P, TS, D, H]),
                    op=OP.mult)
                onehot = ewp.tile([P, TS, P], BF16, tag="onehot")
                nc.vector.tensor_tensor(
                    out=onehot[:],
                    in0=dstl[:, :, None].to_broadcast([P, TS, P]),
                    in1=consts["iota"][:, None, :].to_broadcast([P, TS, P]),
                    op=OP.is_equal)
                fl = fp.tile([P, len(ts), PAY], F16, tag="fl")
                toff = 0
                for wi, T in enumerate(ts):
                    psum_w = pp_e.tile([P, PAY], F32, tag="psw")
                    for t in range(toff, toff + T):
                        nc.tensor.matmul(out=psum_w[:], lhsT=onehot[:, t, :],
                                         rhs=payload[:, t, :],
                                         start=(t == toff),
                                         stop=(t == toff + T - 1))
                    nc.scalar.copy(out=fl[:, wi, :], in_=psum_w[:])
                    toff += T
                nc.sync.dma_start(
                    numer[w0 * P:(w0 + len(ts)) * P, :].rearrange(
                        "(t p) f -> p t f", p=P),
                    fl[:])

            # ---------------- node phase ----------------
            pools = {"cp": cp, "wp": wp, "pp_t": pp_t, "pp_mm": pp_mm}
            outs = None if last else {"kd": kd_o, "qp": qp_o, "ve": ve_o, "xs": xs_o}
            tiles = [x for x in _type_tiles(c) if (not last) or x[0] == 0]
            NTL = len(tiles)
            # pass 1: all tiles up through the skip-add + LN stats; gelu is the
            # only table-based ACT function here so the scalar engine loads
            # the gelu table once instead of thrashing gelu<->sqrt per tile.
            o1_all = cp.tile([P, NTL, HID], F16, tag="o1_all")
            mv_all = cp.tile([P, NTL, 2], F32, tag="mv_all")
            gi0 = 0
            for grp in _type_groups(c, tiles):
                t = grp[0][0]
                G = len(grp)
                r00 = grp[0][2]
                nm4 = wp.tile([P, G, PAY], F16, tag="nm")
                nc.scalar.dma_start(
                    nm4[:],
                    numer[r00:r00 + G * P, :].rearrange("(g p) f -> p g f", p=P))
                xs4 = wp.tile([P, G, HID], F32, tag="xs_ld")
                nc.scalar.dma_start(
                    xs4[:],
                    xs_in[r00:r00 + G * P, :].rearrange("(g p) f -> p g f", p=P))
                den = sp.tile([P, G, H], F32, tag="den")
                nc.vector.tensor_scalar_add(den[:], nm4[:, :, HID:HID + H], 1e-16)
                rec = sp.tile([P, G, H], F32, tag="rec")
                nc.vector.reciprocal(rec[:], den[:])
                agg = wp.tile([P, G, HID], F32, tag="agg")
                nc.vector.tensor_tensor(
                    out=agg[:].rearrange("p g (d h) -> p g d h", h=H),
                    in0=nm4[:, :, :HID].rearrange("p g (d h) -> p g d h", h=H),
                    in1=rec[:, :, None, :].to_broadcast([P, G, D, H]),
                    op=OP.mult)
                glu = wp.tile([P, G, HID], F32, tag="glu")
                if os.environ.get("HGT_BACKEND", "hw") == "sim":
                    # CoreSim has no Gelu LUT: tanh approximation (dev only)
                    t1 = wp.tile([P, G, HID], F32, tag="gelu_t1")
                    nc.vector.tensor_tensor(out=t1[:], in0=agg[:], in1=agg[:], op=OP.mult)
                    nc.vector.tensor_tensor(out=t1[:], in0=t1[:], in1=agg[:], op=OP.mult)
                    nc.vector.tensor_scalar(out=t1[:], in0=t1[:], scalar1=0.044715,
                                            scalar2=None, op0=OP.mult)
                    nc.vector.tensor_tensor(out=t1[:], in0=t1[:], in1=agg[:], op=OP.add)
                    nc.scalar.activation(out=t1[:], in_=t1[:], func=AF.Tanh,
                                         scale=0.7978845608028654)
                    nc.vector.tensor_scalar(out=t1[:], in0=t1[:], scalar1=0.5,
                                            scalar2=0.5, op0=OP.mult, op1=OP.add)
                    nc.vector.tensor_tensor(out=glu[:], in0=t1[:], in1=agg[:], op=OP.mult)
                else:
                    nc.scalar.activation(out=glu[:], in_=agg[:], func=AF.Gelu)
                ops4 = pp_mm.tile([P, G, HID], F32, tag="mmk_ps")
                for gi in range(G):
                    gluT_ps = pp_t.tile([P, P], F32, tag="tp_ps")
                    nc.tensor.transpose(out=gluT_ps[:], in_=glu[:, gi, :],
                                        identity=consts["ident"][:])
                    gluT = wp.tile([P, P], F32, tag="gluT")
                    nc.scalar.copy(out=gluT[:], in_=gluT_ps[:])
                    nc.tensor.matmul(out=ops4[:, gi, :], lhsT=gluT[:],
                                     rhs=consts["Wo"][t][:],
                                     start=True, stop=True)
                # o3 = g*o + (g*b_o) + (1-g)*xs  (bo const already has g*b_o)
                o1g = o1_all[:, gi0:gi0 + G, :]
                xs_s = wp.tile([P, G, HID], F32, tag="xs_s")
                nc.vector.tensor_scalar_mul(xs_s[:], xs4[:], float(1.0 - g_list[t]))
                if build_l23.bo_trivial:
                    nc.vector.tensor_tensor(out=o1g, in0=ops4[:], in1=xs_s[:],
                                            op=OP.add)
                else:
                    nc.vector.tensor_tensor(
                        out=o1g, in0=ops4[:],
                        in1=consts["bo"][t][:, None, :].to_broadcast([P, G, HID]),
                        op=OP.add)
                    nc.vector.tensor_tensor(out=o1g, in0=o1g, in1=xs_s[:], op=OP.add)
                for gi in range(G):
                    stats = sp.tile([P, nc.vector.BN_STATS_DIM], F32, tag="stats")
                    nc.vector.bn_stats(out=stats[:], in_=o1_all[:, gi0 + gi, :])
                    nc.vector.bn_aggr(out=mv_all[:, gi0 + gi, :], in_=stats[:])
                gi0 += G
            # one batched sqrt for all tiles' variances (single table load)
            rstd_all = cp.tile([P, NTL], F32, tag="rstd_all")
            nc.scalar.activation(out=rstd_all[:], in_=mv_all[:, :, 1],
                                 func=AF.Sqrt,
                                 bias=eps_t[:, 0:1])
            nc.vector.reciprocal(rstd_all[:], rstd_all[:])
            # pass 2: normalize + relu + next-layer projections (relu/copy are
            # in every ACT table set, so no further table switches).
            def _xh_relu(i, t, dst_ap):
                xh = wp.tile([P, HID], F32, tag="xh")
                nc.vector.tensor_scalar(
                    out=xh[:], in0=o1_all[:, i, :], scalar1=mv_all[:, i, 0:1],
                    scalar2=rstd_all[:, i:i + 1],
                    op0=OP.subtract, op1=OP.mult)
                if not build_l23.ln_trivial:
                    nc.vector.tensor_tensor(out=xh[:], in0=xh[:], in1=consts["lng"][t][:], op=OP.mult)
                    nc.vector.tensor_tensor(out=xh[:], in0=xh[:], in1=consts["lnb"][t][:], op=OP.add)
                nc.scalar.activation(out=dst_ap, in_=xh[:], func=AF.Relu)

            if not last:
                gi0 = 0
                for grp in _type_groups(c, tiles):
                    t = grp[0][0]
                    G = len(grp)
                    stg = sgp.tile([P, G, _kqv_width(t) * HID], F32, tag="stg")
                    for gi, (_, i_t, r0) in enumerate(grp):
                        _xh_relu(gi0 + gi, t, stg[:, gi, :HID])
                        _kqv_tile(nc, pools, c, consts, t, stg[:, gi, :HID], stg, gi)
                    _kqv_flush(nc, c, outs, t, grp[0][1], G, stg, grp[0][2])
                    gi0 += G
            else:
                for g0i in range(0, len(tiles), 4):
                    grp = tiles[g0i:g0i + 4]
                    G = len(grp)
                    dlst = sp.tile([P, 4, 1], F32, tag="dlst")
                    for gi, (t, i_t, r0) in enumerate(grp):
                        xs_new = wp.tile([P, HID], F32, tag="xs_new")
                        _xh_relu(g0i + gi, t, xs_new[:])
                        xnT_ps = pp_t.tile([P, P], F32, tag="tp_ps")
                        nc.tensor.transpose(out=xnT_ps[:], in_=xs_new[:], identity=consts["ident"][:])
                        xnT = wp.tile([P, P], F32, tag="xnT")
                        nc.scalar.copy(out=xnT[:], in_=xnT_ps[:])
                        d_ps = pp_mm.tile([P, 1], F32, tag="mm_ps")
                        nc.tensor.matmul(out=d_ps[:], lhsT=xnT[:], rhs=consts["whead"][:],
                                         start=True, stop=True)
                        nc.vector.tensor_tensor(out=dlst[:, gi, :], in0=d_ps[:],
                                                in1=consts["bh"][:], op=OP.add)
                    nc.gpsimd.dma_start(
                        dl_o[g0i * P:(g0i + G) * P, :].rearrange(
                            "(g p) f -> p g f", p=P),
                        dlst[:, :G, :])
    nc.compile()
    return nc


build_l23.g_list = None
build_l23.ln_trivial = False
build_l23.bo_trivial = False


# ---------------------------------------------------------------------------
# Runner
# ---------------------------------------------------------------------------

LAUNCH_TIMES_NS = []
TRACE_DIRS = []


def _run(nc, in_maps, cfg):
    backend = os.environ.get("HGT_BACKEND", "hw")
    if backend == "sim":
        from concourse.bass_interp import CoreSim
        results = []
        for m in in_maps:
            sim = CoreSim(nc, trace=False, require_finite=False, require_nnan=False)
            for k, v in m.items():
                sim.tensor(k)[:] = v
            sim.simulate(check_with_hw=False)
            out = {}
            for alloc in nc.m.functions[0].allocations:
                if isinstance(alloc, mybir.MemoryLocationSet) and alloc.kind == "ExternalOutput":
                    name = alloc.memorylocations[0].name
                    out[name] = sim.tensor(name).copy()
            results.append(out)
        return results
    else:
        from concourse.bass_utils import run_bass_kernel_spmd
        trace = os.environ.get("HGT_TRACE", "0") == "1"
        res = run_bass_kernel_spmd(nc, in_maps, core_ids=list(range(cfg.C)),
                                   trace=trace)
        if trace:
            LAUNCH_TIMES_NS.append(res.exec_time_ns)
            it = res.instructions_and_trace
            TRACE_DIRS.append(getattr(it, "trace_path", it))
        return res.results


# ---------------------------------------------------------------------------
# Main entry
# ---------------------------------------------------------------------------

def kernel(**inputs):
    cfg = Cfg()
    return _kernel_impl(cfg, inputs)


def _kernel_impl(cfg, inputs):
    c = cfg
    prm = prep_params(c, inputs)
    g = prep_graph(c, inputs)
    NT, tws = g["NT"], g["tws"]

    # ---- launch 1
    _kqv_tile.bias_zero = not np.asarray(inputs["b_kqv"])[0].any()
    nc1 = build_l1(c)
    in_maps = []
    xa = np.asarray(inputs["x_a"], np.float32)
    xw = np.asarray(inputs["x_w"], np.float32)
    xo = np.asarray(inputs["x_o"], np.float32)

    import ml_dtypes as _mld

    def padxT(x, n, npad):
        out = np.zeros((64, npad), np.float32)
        out[:, :n] = x.T
        return out.astype(np.float16)

    for cc in range(c.C):
        in_maps.append({
            "xta": padxT(xa[cc * c.nac:(cc + 1) * c.nac], c.nac, c.nap),
            "xtw": padxT(xw[cc * c.nwc:(cc + 1) * c.nwc], c.nwc, c.nwp),
            "xto": padxT(xo[cc * c.noc:(cc + 1) * c.noc], c.noc, c.nop),
            "Win": prm["WinB"], "binT": prm["binT"],
            "identb": prm["identb"],
            "Wcat": prm["Wcat0"], "bcat": prm["bcat0"],
            "ident": prm["ident"],
        })
    r1 = _run(nc1, in_maps, c)

    def assemble_tables(res):
        """Build global kd table + per-core q' tables + global ve table."""
        kd_tab = np.empty((c.KD_ROWS, HID), np.float32)
        kd_tab[-1] = 1.0
        ve_tab = np.empty((c.VE_ROWS, HID), np.float32)
        ve_tab[-1] = 0.0
        qp_tabs = []
        for cc in range(c.C):
            kd = res[cc]["kd"]
            ve = res[cc]["ve"]
            # kd local [a|w|o] -> global
            kd_tab[c.KOFF[0] + cc * c.nac:c.KOFF[0] + (cc + 1) * c.nac] = kd[:c.nac]
            kd_tab[c.KOFF[1] + cc * c.nwc:c.KOFF[1] + (cc + 1) * c.nwc] = \
                kd[c.base_local[1]:c.base_local[1] + c.nwc]
            kd_tab[c.KOFF[2] + cc * c.noc:c.KOFF[2] + (cc + 1) * c.noc] = \
                kd[c.base_local[2]:c.base_local[2] + c.noc]
            # ve local slots [a-et2, a-et3, w-et0, o-et1] -> global stacked
            ve_tab[c.VOFF[2] + cc * c.nac:c.VOFF[2] + (cc + 1) * c.nac] = \
                ve[c.QB[0]:c.QB[0] + c.nac]
            ve_tab[c.VOFF[3] + cc * c.nac:c.VOFF[3] + (cc + 1) * c.nac] = \
                ve[c.QB[1]:c.QB[1] + c.nac]
            ve_tab[c.VOFF[0] + cc * c.nwc:c.VOFF[0] + (cc + 1) * c.nwc] = \
                ve[c.QB[2]:c.QB[2] + c.nwc]
            ve_tab[c.VOFF[1] + cc * c.noc:c.VOFF[1] + (cc + 1) * c.noc] = \
                ve[c.QB[3]:c.QB[3] + c.noc]
            # pad-edge q rows are -8.0: with pad k rows = 1.0 the pad alpha is
            # 128 * -8 = -1024 (bf16-safe), exp -> 0.
            qp = np.vstack([res[cc]["qp"], np.full((1, HID), -8.0, np.float32)])
            qp_tabs.append(qp)
        return kd_tab, ve_tab, qp_tabs

    import ml_dtypes
    bf16 = ml_dtypes.bfloat16

    # ---- launches 2 and 3
    # last launch only needs a-dst windows (head reads only a-type nodes)
    NT_a = int(tws[:c.ntile_a].sum())
    res = r1
    for l, last in ((1, False), (2, True)):
        kd_tab, ve_tab, qp_tabs = assemble_tables(res)
        lay = l - 1  # layer params index: launch2 -> layer 0, launch3 -> layer 1
        build_l23.g_list = prm[f"g{lay}"]
        build_l23.bo_trivial = not np.asarray(inputs["b_o"])[lay].any()
        build_l23.ln_trivial = bool(
            (np.asarray(inputs["ln_g"])[lay] == 1).all()
            and not np.asarray(inputs["ln_b"])[lay].any())
        _kqv_tile.bias_zero = (not last) and \
            not np.asarray(inputs["b_kqv"])[lay + 1].any()
        NT_l = NT_a if last else NT
        nc = build_l23(c, NT_l, tws, last)
        in_maps = []
        for cc in range(c.C):
            ti = g["tile_idx"][cc][:NT_l]  # [NT_l, P, 4]
            kg_s = np.ascontiguousarray(
                kd_tab[ti[:, :, 0]].transpose(1, 0, 2)).astype(bf16)
            vg_s = np.ascontiguousarray(
                ve_tab[ti[:, :, 1]][:, :, DH_PERM].transpose(1, 0, 2)).astype(bf16)
            qg_s = np.ascontiguousarray(
                qp_tabs[cc][ti[:, :, 2]].transpose(1, 0, 2)).astype(bf16)
            dstlT = np.ascontiguousarray(ti[:, :, 3].T.astype(bf16))
            m = {
                "kg": kg_s, "vg": vg_s, "qg": qg_s,
                "xsin": res[cc]["xs"][:c.nap] if last else res[cc]["xs"],
                "dstlT": dstlT,
                "iota": prm["iota"], "ident": prm["ident"],
                "Wo": prm[f"Wo{lay}"], "bo": prm[f"bo{lay}"],
                "lng": prm[f"lng{lay}"], "lnb": prm[f"lnb{lay}"],
                "gs": prm[f"g{lay}"],
            }
            if not last:
                m.update({"Wcat": prm[f"Wcat{lay + 1}"], "bcat": prm[f"bcat{lay + 1}"]})
            else:
                m.update({"whead": prm["whead"], "bh": prm["bh"]})
            in_maps.append(m)
        res = _run(nc, in_maps, c)

    out = np.concatenate([res[cc]["delta"][:c.nac, 0] for cc in range(c.C)])
    return out.astype(np.float32)

